# revision 1
# baseline (speedup 1.0000x reference)
"""CustomPoseLoss Trainium2 kernel.

loss = mean((pred-target)^2) + 0.5 * mean((R(pred)-R(target))^2)
where R(M) = sign(det M) * polar(M) for each 3x3 matrix (row of 9).

Implementation: closed-form polar decomposition per row, fully vectorized as
channel-plane arithmetic on the Vector/Scalar engines:
  S = M^T M, normalized by tr(S)/3; eigenvalues of S via Cardano
  (acos/cos evaluated as polynomials so only the sqrt LUT set is needed);
  W^-1 = (S + s2 I) adj(N) / det(N) with N = ssig*S + Pg*I  (Cayley-Hamilton
  inverse-sqrt);  R = sign(det) * M W^-1 / sqrt(m).
det(N) is formed from the eigenvalue product (positive, cancellation-free)
and clamped, so near-singular rows stay bounded.

Sharding: pure data parallel over 8 cores; each core reduces its shard to
[128, 2, NCHUNK] partial sums (mse, rot), host combines in float64.
"""

import numpy as np

B = 1048576
N_CORES = 8
ROWS_PER_CORE = B // N_CORES          # 131072
P = 128
ROWS_PER_PART = ROWS_PER_CORE // P    # 1024
T = 256                               # rows per partition per chunk
NCHUNK = ROWS_PER_PART // T           # 4
EPS_D = 1e-5

ACOS_A = (1.5707288, -0.2121144, 0.0742610, -0.0187293)   # A&S 4.4.45
HALF_SQRT3 = 0.8660254037844386


class Regs:
    """[128, 2, T] fp32 plane slots with explicit reuse (SBUF is capped)."""

    def __init__(self, pool, dtype, prefix="v", shape=None):
        self.pool = pool
        self.dtype = dtype
        self.prefix = prefix
        self.shape = shape or [P, 2 * T]
        self.free_tags = []
        self.n = 0
        self.tag_of = {}

    def alloc(self):
        if self.free_tags:
            tag = self.free_tags.pop()
        else:
            self.n += 1
            tag = f"{self.prefix}{self.n}"
        tl = self.pool.tile(self.shape, self.dtype, tag=tag)
        self.tag_of[id(tl)] = tag
        return tl

    def free(self, *tiles):
        for tl in tiles:
            self.free_tags.append(self.tag_of.pop(id(tl)))


LN3 = float(np.log(3.0))
LN6 = float(np.log(6.0))
LN2 = float(np.log(2.0))
EPS_W = 6e-3


def _build_chunk(nc, regs, regs16, praw, traw, acc_mse_col, acc_rot_col,
                 X, D, Sm, QS, Nm, Am, T1m, RT, dR, bias0, mybir):
    Alu = mybir.AluOpType
    Act = mybir.ActivationFunctionType
    L = 2 * T

    def mul(o, a, b):
        nc.vector.tensor_tensor(out=o, in0=a, in1=b, op=Alu.mult)

    def add(o, a, b):
        nc.vector.tensor_tensor(out=o, in0=a, in1=b, op=Alu.add)

    def sub(o, a, b):
        nc.vector.tensor_tensor(out=o, in0=a, in1=b, op=Alu.subtract)

    def vs(o, a, s1, op0, s2=None, op1=None):
        if s2 is None:
            nc.vector.tensor_scalar(out=o, in0=a, scalar1=float(s1),
                                    scalar2=None, op0=getattr(Alu, op0))
        else:
            nc.vector.tensor_scalar(out=o, in0=a, scalar1=float(s1),
                                    scalar2=float(s2), op0=getattr(Alu, op0),
                                    op1=getattr(Alu, op1))

    def stt(o, a, s, b, op0, op1):
        nc.vector.scalar_tensor_tensor(out=o, in0=a, scalar=float(s), in1=b,
                                       op0=getattr(Alu, op0),
                                       op1=getattr(Alu, op1))

    def act(o, a, func, scale=1.0, bias=None, accum_out=None):
        if func == "Copy":
            nc.scalar.activation(out=o, in_=a, func=Act.Copy, bias=0.0,
                                 scale=float(scale), accum_out=accum_out)
        else:
            nc.scalar.activation(out=o, in_=a, func=getattr(Act, func),
                                 bias=bias0[:, 0:1] if bias is None else bias,
                                 scale=float(scale), accum_out=accum_out)

    def bc(plane, k):
        # broadcast [P, L] plane across k sub-planes -> [P, k, L]
        return bass_mod.AP(tensor=plane.tensor, offset=plane.offset,
                           ap=[plane.ap[0], [0, k], plane.ap[1]])

    A = regs.alloc          # fp32 [P, L] planes
    H = regs16.alloc        # fp16 [P, L] planes

    # ---- cast+deinterleave both inputs into X[P, 9, 2T] (f16) ----
    rvp = praw.rearrange("p (n c) -> p n c", c=9)
    rvt = traw.rearrange("p (n c) -> p n c", c=9)
    xin_p = bass_mod.AP(tensor=rvp.tensor, offset=rvp.offset,
                        ap=[rvp.ap[0], rvp.ap[2], rvp.ap[1]])
    xin_t = bass_mod.AP(tensor=rvt.tensor, offset=rvt.offset,
                        ap=[rvt.ap[0], rvt.ap[2], rvt.ap[1]])
    act(X[:, :, 0:T], xin_p, "Copy")
    act(X[:, :, T:L], xin_t, "Copy")
    x = [X[:, c, :] for c in range(9)]          # [P, L] f16 unit-stride

    # ---- mse: D = pred - target (f16), accum sum(D^2) on ACT ----
    sub(D, X[:, :, 0:T], X[:, :, T:L])
    Df = D.rearrange("p c n -> p (c n)")
    act(Df, Df, "Square", accum_out=acc_mse_col)

    # ---- S = M^T M (f16): order [s00,s11,s22,s01,s02,s12] ----
    tmp16 = H()
    for i in range(3):
        sd = Sm[:, i, :]
        mul(sd, x[i], x[i])
        mul(tmp16, x[i+3], x[i+3]); add(sd, sd, tmp16)
        mul(tmp16, x[i+6], x[i+6]); add(sd, sd, tmp16)
    for oi, (ia, ib) in enumerate((((0,3,6),(1,4,7)), ((0,3,6),(2,5,8)),
                                   ((1,4,7),(2,5,8)))):
        so = Sm[:, 3+oi, :]
        mul(so, x[ia[0]], x[ib[0]])
        mul(tmp16, x[ia[1]], x[ib[1]]); add(so, so, tmp16)
        mul(tmp16, x[ia[2]], x[ib[2]]); add(so, so, tmp16)
    # tr and normalization scale q = 3/tr (ln domain)
    tr16 = H()
    add(tr16, Sm[:, 0, :], Sm[:, 1, :]); add(tr16, tr16, Sm[:, 2, :])
    vs(tr16, tr16, 6e-5, "max")
    lnt = A(); act(lnt, tr16, "Ln")
    q16 = H(); act(q16, lnt, "Exp", scale=-1.0, bias=_c(nc, LN3))
    regs16.free(tr16)
    nc.vector.tensor_tensor(out=Sm[:, :, :], in0=Sm[:, :, :], in1=bc(q16, 6),
                            op=Alu.mult)
    regs16.free(q16)

    # ---- det(M) fp32 from raw (strided channel views) ----
    xr = [None] * 9
    for c in range(9):
        ap_p = rvp[:, :, c]
        ap_t = rvt[:, :, c]
        xr[c] = (ap_p, ap_t)
    cA, cB, det, tmpd = A(), A(), A(), A()
    def rmul(o, i, j):
        # o[:, :T] = pred_ch_i*pred_ch_j ; o[:, T:] = target halves
        mul(o[:, 0:T], xr[i][0], xr[j][0])
        mul(o[:, T:L], xr[i][1], xr[j][1])
    def rmul2(o, i, co):
        mul(o[:, 0:T], xr[i][0], co[:, 0:T])
        mul(o[:, T:L], xr[i][1], co[:, T:L])
    rmul(cA, 4, 8); rmul(cB, 5, 7); sub(cA, cA, cB)
    rmul2(det, 0, cA)
    rmul(cA, 5, 6); rmul(cB, 3, 8); sub(cA, cA, cB)
    rmul2(tmpd, 1, cA); add(det, det, tmpd)
    rmul(cA, 3, 7); rmul(cB, 4, 6); sub(cA, cA, cB)
    rmul2(tmpd, 2, cA); add(det, det, tmpd)
    regs.free(cA)
    sgd = A(); act(sgd, det, "Sign")
    lnad = A(); act(cB, det, "Abs")
    act(lnad, cB, "Ln")
    regs.free(cB, det, tmpd)
    # Pg = exp(lnad + 1.5*(ln3 - lnt));  dets = Pg^2
    lnPg = A()
    stt(lnPg, lnt, -1.5, lnad, "mult", "add")
    regs.free(lnad)
    Pg32 = A(); act(Pg32, lnPg, "Exp", scale=1.0, bias=_c(nc, 1.5 * LN3))
    Pg16 = H(); act(Pg16, Pg32, "Copy")
    dets = A(); act(dets, Pg32, "Square")
    regs.free(lnPg)

    # ---- tr(S^2) fp32 from normalized f16 S ----
    act(QS, Sm, "Square")
    u1, u2 = A(), A()
    add(u1, QS[:, 0, :], QS[:, 1, :]); add(u1, u1, QS[:, 2, :])
    add(u2, QS[:, 3, :], QS[:, 4, :]); add(u2, u2, QS[:, 5, :])
    trS2 = A()
    stt(trS2, u2, 2.0, u1, "mult", "add")
    # p and 1/(2 p^3) via ln/exp
    trK2, p, ip3h = u1, A(), u2            # reuse u1/u2 slots
    vs(trK2, trS2, -3.0, "add", 1e-30, "max")
    lnk = A(); act(lnk, trK2, "Ln")
    act(p, lnk, "Exp", scale=0.5, bias=_c(nc, -0.5 * LN6))
    act(ip3h, lnk, "Exp", scale=-1.5, bias=_c(nc, 1.5 * LN6 - LN2))
    vs(ip3h, ip3h, 1e30, "min")
    regs.free(lnk)
    # arg
    detK, arg = A(), A()
    stt(detK, trS2, 0.5, dets, "mult", "add")
    vs(detK, detK, -2.5, "add")
    mul(arg, detK, ip3h)
    vs(arg, arg, 1.0, "min", -1.0, "max")
    regs.free(detK, trS2, dets, u2)   # u2 == ip3h
    # ---- th3 = acos(arg) ----
    y, om, h = A(), A(), A()
    act(y, arg, "Abs")
    vs(om, y, -1.0, "mult", 1.0, "add")
    lnom = A(); act(lnom, om, "Ln")
    act(om, lnom, "Exp", scale=0.5)              # sqrt(1-y)
    regs.free(lnom)
    vs(h, y, ACOS_A[3], "mult", ACOS_A[2], "add")
    mul(h, h, y); vs(h, h, ACOS_A[1], "add")
    mul(h, h, y); vs(h, h, ACOS_A[0], "add")
    mul(h, h, om)
    sg_a, th3 = y, om                            # reuse slots
    act(sg_a, arg, "Sign")
    vs(arg, sg_a, -np.pi/2, "mult", np.pi/2, "add")
    mul(th3, sg_a, h); add(th3, th3, arg)
    regs.free(h, arg, y)    # th3 == om stays
    # ---- cos((th3+2pik)/3) ----
    z, c0p = A(), A()
    act(z, th3, "Square", scale=1.0/3.0)
    vs(c0p, z, 1.0/40320.0, "mult", -1.0/720.0, "add")
    mul(c0p, c0p, z); vs(c0p, c0p, 1.0/24.0, "add")
    mul(c0p, c0p, z); vs(c0p, c0p, -0.5, "add")
    mul(c0p, c0p, z); vs(c0p, c0p, 1.0, "add")
    regs.free(z, om)   # om == th3
    s0, uc1, c1p, c2p = A(), A(), A(), A()
    act(s0, c0p, "Square")
    vs(s0, s0, -1.0, "mult", 1.0, "add")
    vs(s0, s0, 0.0, "max")
    lns = A(); act(lns, s0, "Ln")
    act(s0, lns, "Exp", scale=0.5)
    regs.free(lns)
    vs(uc1, c0p, -0.5, "mult")
    vs(s0, s0, HALF_SQRT3, "mult")
    sub(c1p, uc1, s0)
    add(c2p, uc1, s0)
    regs.free(s0, uc1)
    # ---- lambda_k, g_k = sqrt(lambda_k) ----
    tp = A()
    vs(tp, p, 2.0, "mult")
    regs.free(p)
    lam, g = [], []
    for ck in (c0p, c1p, c2p):
        lk, gk, lnl = A(), A(), A()
        mul(lk, tp, ck)
        vs(lk, lk, 1.0, "add", 1e-35, "max")
        act(lnl, lk, "Ln")
        act(gk, lnl, "Exp", scale=0.5)
        regs.free(lnl)
        lam.append(lk); g.append(gk)
    regs.free(tp, c0p, c1p, c2p)
    g01, ssig, s2i, tmp2 = A(), A(), A(), A()
    add(g01, g[0], g[1])
    add(ssig, g01, g[2])
    mul(s2i, g[0], g[1]); mul(tmp2, g[2], g01); add(s2i, s2i, tmp2)
    regs.free(g01, tmp2, *g)
    # ---- w = sign/(sqrt(m)*detN) via ln domain, clamped ----
    t_, nuk, lnn = A(), A(), A()
    mul(nuk, ssig, lam[0]); add(nuk, nuk, Pg32)
    act(t_, nuk, "Ln")
    mul(nuk, ssig, lam[1]); add(nuk, nuk, Pg32)
    act(lnn, nuk, "Ln"); add(t_, t_, lnn)
    mul(nuk, ssig, lam[2]); add(nuk, nuk, Pg32)
    act(lnn, nuk, "Ln"); add(t_, t_, lnn)
    stt(t_, lnt, 0.5, t_, "mult", "add")
    vs(t_, t_, float(np.log(EPS_W) + 0.5*LN3), "max")
    wmag = lnn                                  # reuse
    act(wmag, t_, "Exp", scale=-1.0, bias=_c(nc, 0.5 * LN3))
    w32 = A()
    mul(w32, wmag, sgd)
    regs.free(t_, nuk, lnn, sgd, lnt, *lam)
    ssig16, s2i16 = H(), H()
    act(ssig16, ssig, "Copy")
    act(s2i16, s2i, "Copy")
    regs.free(ssig, s2i, Pg32)

    # ---- N = ssig*S + Pg*I (f16, batched) ----
    nc.vector.tensor_tensor(out=Nm[:, :, :], in0=Sm[:, :, :],
                            in1=bc(ssig16, 6), op=Alu.mult)
    nc.vector.tensor_tensor(out=Nm[:, 0:3, :], in0=Nm[:, 0:3, :],
                            in1=bc(Pg16, 3), op=Alu.add)
    regs16.free(Pg16, ssig16)
    # A' diagonal (Am = S_diag + s2)
    nc.vector.tensor_tensor(out=Am[:, :, :], in0=Sm[:, 0:3, :],
                            in1=bc(s2i16, 3), op=Alu.add)
    regs16.free(s2i16)
    # ---- adj(N) (f16) -> stored into QS? no: reuse Nm? need both. use T1m? no.
    n00, n11, n22 = (Nm[:, i, :] for i in range(3))
    n01, n02, n12 = (Nm[:, i, :] for i in range(3, 6))
    aj = [H() for _ in range(6)]
    a00, a01, a02, a11, a12, a22 = aj
    def cof(o, a, b, c, dd):
        mul(o, a, b); mul(tmp16, c, dd); sub(o, o, tmp16)
    cof(a00, n11, n22, n12, n12)
    cof(a01, n02, n12, n01, n22)
    cof(a02, n01, n12, n02, n11)
    cof(a11, n00, n22, n02, n02)
    cof(a12, n01, n02, n00, n12)
    cof(a22, n00, n11, n01, n01)
    # ---- T1 = A' adjN (f16); rows of A': (b0,s01,s02),(s01,b1,s12),(s02,s12,b2)
    b0, b1, b2 = (Am[:, i, :] for i in range(3))
    s01p, s02p, s12p = Sm[:, 3, :], Sm[:, 4, :], Sm[:, 5, :]
    def mm3(o, r0, r1, r2, k0, k1, k2):
        mul(o, r0, k0)
        mul(tmp16, r1, k1); add(o, o, tmp16)
        mul(tmp16, r2, k2); add(o, o, tmp16)
    mm3(T1m[:, 0, :], b0, s01p, s02p, a00, a01, a02)
    mm3(T1m[:, 1, :], b0, s01p, s02p, a01, a11, a12)
    mm3(T1m[:, 2, :], b0, s01p, s02p, a02, a12, a22)
    mm3(T1m[:, 3, :], s01p, b1, s12p, a01, a11, a12)
    mm3(T1m[:, 4, :], s01p, b1, s12p, a02, a12, a22)
    mm3(T1m[:, 5, :], s02p, s12p, b2, a02, a12, a22)
    regs16.free(*aj)
    # ---- T2 = w*T1 in fp32 (QS tile is dead; reuse it) ----
    T2f = QS
    nc.vector.tensor_tensor(out=T2f[:, :, :], in0=T1m[:, :, :],
                            in1=bc(w32, 6), op=Alu.mult)
    regs.free(w32)
    t00, t01, t02 = T2f[:, 0, :], T2f[:, 1, :], T2f[:, 2, :]
    t11, t12, t22 = T2f[:, 3, :], T2f[:, 4, :], T2f[:, 5, :]
    T2 = [[t00, t01, t02], [t01, t11, t12], [t02, t12, t22]]
    # ---- R = M*T2 (fp32 out), clamp junk rows, dR, accumulate ----
    tmpr = regs.alloc()
    for i in range(3):
        for j in range(3):
            o = RT[:, 3*i+j, :]
            mul(o, x[3*i], T2[0][j])
            mul(tmpr, x[3*i+1], T2[1][j]); add(o, o, tmpr)
            mul(tmpr, x[3*i+2], T2[2][j]); add(o, o, tmpr)
    regs.free(tmpr)
    RTf = RT.rearrange("p c n -> p (c n)")
    nc.vector.tensor_scalar(out=RTf, in0=RTf, scalar1=8.0, scalar2=-8.0,
                            op0=Alu.min, op1=Alu.max)
    sub(dR, RT[:, :, 0:T], RT[:, :, T:L])
    dRf = dR.rearrange("p c n -> p (c n)")
    act(dRf, dRf, "Square", accum_out=acc_rot_col)
    regs16.free(tmp16)



_CONST_STATE = {}
bass_mod = None


def _c(nc, v):
    """[P,1] fp32 constant AP, DVE-memset once (keeps ACT single-wait)."""
    key = float(np.float32(v))
    consts = _CONST_STATE.setdefault(id(nc), {})
    if key not in consts:
        pool = _CONST_STATE[(id(nc), "pool")]
        from concourse import mybir
        t = pool.tile([P, 1], mybir.dt.float32, tag=f"c{len(consts)}")
        nc.vector.memset(t, key)
        consts[key] = t
    return consts[key][:, 0:1]


def _build_nc():
    global bass_mod
    import concourse.bass as bass
    import concourse.tile as tile
    from concourse import mybir
    bass_mod = bass

    f32 = mybir.dt.float32
    f16 = mybir.dt.float16
    nc = bass.Bass()
    pred = nc.dram_tensor("pred", [ROWS_PER_CORE, 9], f32, kind="ExternalInput")
    targ = nc.dram_tensor("target", [ROWS_PER_CORE, 9], f32, kind="ExternalInput")
    out = nc.dram_tensor("partials", [P, 2 * NCHUNK], f32, kind="ExternalOutput")

    predv = pred.rearrange("(p n) c -> p n c", p=P)    # [128, 1024, 9]
    targv = targ.rearrange("(p n) c -> p n c", p=P)

    with tile.TileContext(nc) as tc:
        with (
            tc.tile_pool(name="raw", bufs=1) as rawp,
            tc.tile_pool(name="pl", bufs=1) as pl,
            tc.tile_pool(name="acc", bufs=1) as accp,
        ):
            acc = accp.tile([P, 2 * NCHUNK], f32, tag="acc")
            bias0 = accp.tile([P, 1], f32, tag="bias0")
            nc.vector.memset(bias0, 0.0)
            _CONST_STATE[(id(nc), "pool")] = accp
            regs = Regs(pl, f32, prefix="v", shape=[P, 2 * T])
            regs16 = Regs(pl, f16, prefix="h", shape=[P, 2 * T])
            L = 2 * T
            praw_all = rawp.tile([P, ROWS_PER_PART * 9], f32, tag="praw")
            traw_all = rawp.tile([P, ROWS_PER_PART * 9], f32, tag="traw")
            # two-piece load: chunk-0 compute overlaps the bulk transfer
            nc.sync.dma_start(out=praw_all[:, 0:T*9], in_=predv[:, 0:T, :])
            nc.sync.dma_start(out=traw_all[:, 0:T*9], in_=targv[:, 0:T, :])
            nc.sync.dma_start(out=praw_all[:, T*9:], in_=predv[:, T:, :])
            nc.sync.dma_start(out=traw_all[:, T*9:], in_=targv[:, T:, :])
            for k in range(NCHUNK):
                praw = praw_all[:, k*T*9:(k+1)*T*9]
                traw = traw_all[:, k*T*9:(k+1)*T*9]
                X = pl.tile([P, 9, L], f16, tag=f"X{k%2}")
                D = pl.tile([P, 9, T], f16, tag="D")
                Sm = pl.tile([P, 6, L], f16, tag=f"Sm{k%2}")
                QS = pl.tile([P, 6, L], f32, tag="QS")
                Nm = pl.tile([P, 6, L], f16, tag="Nm")
                Am = pl.tile([P, 3, L], f16, tag="Am")
                T1m = pl.tile([P, 6, L], f16, tag="T1m")
                RT = pl.tile([P, 9, L], f32, tag="RT")
                dR = pl.tile([P, 9, T], f32, tag="dRt")
                _build_chunk(nc, regs, regs16, praw, traw,
                             acc[:, k:k+1], acc[:, NCHUNK+k:NCHUNK+k+1],
                             X, D, Sm, QS, Nm, Am, T1m, RT, dR, bias0, mybir)
            nc.sync.dma_start(out=out[:, :], in_=acc)
    return nc


def _elide_implied_waits(nc):
    """Drop semaphore waits already implied by program order or transitively
    by earlier waits (vector-clock propagation).  Tile's per-instruction wait
    emission is not transitively minimal, and walrus can encode only one sync
    wait on Activation/DMA instructions (and ~4 on control instructions), so
    the redundant waits both break codegen and waste sequencer time.

    Model: each semaphore s carries a snapshot VC at every increment value;
    an engine's observed VC advances via its own instruction stream and via
    the snapshots of the waits it executes.  A wait (s >= v) is dropped iff
    the engine's observed VC already dominates it.  Unknown update modes
    disable elision for that semaphore (conservative).
    """
    join = lambda a, b: {k: max(a.get(k, 0), b.get(k, 0)) for k in set(a) | set(b)}
    sem_val = {}        # sem name -> current value
    sem_snap = {}       # sem name -> list of (value, VC) snapshots
    eng_vc = {}         # engine name -> observed VC
    unsafe = set()      # sems with non-increment updates
    n_drop = 0
    for f in nc.m.functions:
        for bb in f.blocks:
            for ins in bb.instructions:
                eng = str(ins.engine)
                vc = dict(eng_vc.get(eng, {}))
                si = ins.sync_info
                waits = list(si.on_wait) if si is not None and si.on_wait else []
                kept = []
                for w in waits:
                    s, v = w.ant_name, w.wait_value
                    if w.wait_mode != "sem-ge-imm" or s in unsafe:
                        kept.append(w)
                        continue
                    if vc.get(s, 0) >= v:
                        n_drop += 1
                        continue
                    if sem_val.get(s, 0) < v:
                        # increment not yet seen in emission order; keep and
                        # learn nothing (conservative)
                        kept.append(w)
                        continue
                    kept.append(w)
                    snap = {}
                    for sv, svc in sem_snap.get(s, ()):
                        if sv <= v:
                            snap = svc
                        else:
                            break
                    vc = join(vc, snap)
                    vc[s] = max(vc.get(s, 0), v)
                if si is not None and len(kept) != len(waits):
                    si.on_wait = kept
                # apply this instruction's increments
                ups = si.on_update if si is not None and si.on_update else []
                for u in ups:
                    s = u.ant_name
                    if u.update_mode not in ("sem-inc", "sem-add-imm"):
                        unsafe.add(s)
                        continue
                    nv = sem_val.get(s, 0) + (u.update_value or 1)
                    sem_val[s] = nv
                    lst = sem_snap.setdefault(s, [])
                    prev = lst[-1][1] if lst else {}
                    lst.append((nv, join(prev, vc)))
                    # Engine-sem increments fire when the instruction
                    # completes, and the engine is sequential, so later
                    # instructions on this engine observe them.  DMA-queue
                    # increments fire asynchronously at transfer completion:
                    # the issuing engine must NOT absorb those.
                    if "DMA" not in s:
                        vc[s] = max(vc.get(s, 0), nv)
                eng_vc[eng] = vc
    return n_drop


_NC_CACHE = None


def kernel(pred: np.ndarray, target: np.ndarray) -> np.ndarray:
    global _NC_CACHE
    from concourse.bass_utils import run_bass_kernel_spmd

    pred = np.ascontiguousarray(np.asarray(pred, dtype=np.float32))
    target = np.ascontiguousarray(np.asarray(target, dtype=np.float32))
    assert pred.shape == (B, 9) and target.shape == (B, 9)

    if _NC_CACHE is None:
        _NC_CACHE = _build_nc()
        _elide_implied_waits(_NC_CACHE)
    nc = _NC_CACHE

    ps = pred.reshape(N_CORES, ROWS_PER_CORE, 9)
    ts = target.reshape(N_CORES, ROWS_PER_CORE, 9)
    in_maps = [{"pred": ps[i], "target": ts[i]} for i in range(N_CORES)]
    res = run_bass_kernel_spmd(nc, in_maps, core_ids=list(range(N_CORES)))
    globals()["_LAST_RESULT"] = res

    mse_sum = 0.0
    rot_sum = 0.0
    for r in res.results:
        part = np.asarray(r["partials"], dtype=np.float64)
        mse_sum += part[:, :NCHUNK].sum()
        rot_sum += part[:, NCHUNK:].sum()
    n = float(B * 9)
    return np.asarray(np.float32(mse_sum / n + 0.5 * (rot_sum / n)))



# revision 17
# speedup vs baseline: 1.1164x; 1.1164x over previous
"""CustomPoseLoss Trainium2 kernel.

loss = mean((pred-target)^2) + 0.5 * mean((R(pred)-R(target))^2)
where R(M) = sign(det M) * polar(M) for each 3x3 matrix (row of 9).

Implementation: closed-form polar decomposition per row, fully vectorized as
channel-plane arithmetic:
  S = M^T M, normalized by tr(S)/3; eigenvalues of S via Cardano
  (acos/cos evaluated as polynomials so only the ln/exp LUT set is needed);
  W^-1 = (S + s2 I) adj(N) / det(N) with N = ssig*S + Pg*I  (Cayley-Hamilton
  inverse-sqrt);  R = sign(det) * M W^-1 / sqrt(m).
det(N) is formed from the eigenvalue product (positive, cancellation-free)
and clamped, so near-singular rows stay bounded.

Engine plan: heavy products in f16 on DVE (2x mode); scalar chain fp32 on
DVE+ACT; mse-sub / clamp / dR-sub on GpSimd (off critical path).  Chunks are
software-pipelined: chunk k's tail (T2, R = M*T2) is emitted as filler inside
chunk k+1's scalar-chain stalls, so the Vector engine never waits on ACT legs.

Sharding: pure data parallel over 8 cores; each core reduces its shard to
[128, 2*NCHUNK] partial sums (mse, rot), host combines in float64.
"""

import numpy as np

B = 1048576
N_CORES = 8
ROWS_PER_CORE = B // N_CORES          # 131072
P = 128
ROWS_PER_PART = ROWS_PER_CORE // P    # 1024
CHUNKS = (128, 128, 256, 256, 256)    # rows per partition per chunk
NCHUNK = len(CHUNKS)
TMAX = max(CHUNKS)
EPS_D = 1e-5

ACOS_A = (1.5707288, -0.2121144, 0.0742610, -0.0187293)   # A&S 4.4.45
HALF_SQRT3 = 0.8660254037844386

LN3 = float(np.log(3.0))
LN6 = float(np.log(6.0))
LN2 = float(np.log(2.0))
EPS_W = 6e-3


class Regs:
    """[128, 2*TMAX] plane slots with explicit reuse (SBUF is capped)."""

    def __init__(self, pool, dtype, prefix="v", shape=None):
        self.pool = pool
        self.dtype = dtype
        self.prefix = prefix
        self.shape = shape or [P, 2 * TMAX]
        self.free_tags = []
        self.n = 0
        self.tag_of = {}

    def alloc(self, width=None):
        if self.free_tags:
            tag = self.free_tags.pop()
        else:
            self.n += 1
            tag = f"{self.prefix}{self.n}"
        tl = self.pool.tile(self.shape, self.dtype, tag=tag, name=tag)
        if width is not None:
            tl = tl[:, 0:width]
        self.tag_of[id(tl)] = tag
        return tl

    def free(self, *tiles):
        for tl in tiles:
            self.free_tags.append(self.tag_of.pop(id(tl)))


_CONST_STATE = {}
bass_mod = None


def _c(nc, v):
    """[P,1] fp32 constant AP, DVE-memset once (keeps ACT single-wait)."""
    key = float(np.float32(v))
    consts = _CONST_STATE.setdefault(id(nc), {})
    if key not in consts:
        pool = _CONST_STATE[(id(nc), "pool")]
        from concourse import mybir
        t = pool.tile([P, 1], mybir.dt.float32, tag=f"c{len(consts)}")
        nc.vector.memset(t, key)
        consts[key] = t
    return consts[key][:, 0:1]


def _emit_A(nc, env, k, T, praw, traw, tiles, acc_mse_col, fill, pre_cast,
            post_cast):
    """Phase A of chunk k: everything through T1m and w16.

    `fill(n)` emits up to n pending phase-B DVE ops from the previous chunk
    at known ACT-wait stall points.  `pre_cast()` emits the chunk k-2 ACT
    accumulate (whose Pool wait transitively covers the casts' X-tile WAR
    wait — walrus allows only one sync wait per Activation instruction);
    `post_cast()` emits the chunk k+2 DMA prefetch.
    """
    mybir = env["mybir"]
    regs, regs16, bias0 = env["regs"], env["regs16"], env["bias0"]
    Alu = mybir.AluOpType
    Act = mybir.ActivationFunctionType
    L = 2 * T
    X, D, Sm, QS, Am, T1m, Nm, W16 = (tiles[n] for n in
                                      ("X", "D", "Sm", "QS", "Am", "T1m",
                                       "Nm", "W16"))

    def mul(o, a, b):
        nc.vector.tensor_tensor(out=o, in0=a, in1=b, op=Alu.mult)

    def add(o, a, b):
        nc.vector.tensor_tensor(out=o, in0=a, in1=b, op=Alu.add)

    def sub(o, a, b):
        nc.vector.tensor_tensor(out=o, in0=a, in1=b, op=Alu.subtract)

    def vs(o, a, s1, op0, s2=None, op1=None):
        if s2 is None:
            nc.vector.tensor_scalar(out=o, in0=a, scalar1=float(s1),
                                    scalar2=None, op0=getattr(Alu, op0))
        else:
            nc.vector.tensor_scalar(out=o, in0=a, scalar1=float(s1),
                                    scalar2=float(s2), op0=getattr(Alu, op0),
                                    op1=getattr(Alu, op1))

    def stt(o, a, s, b, op0, op1):
        nc.vector.scalar_tensor_tensor(out=o, in0=a, scalar=float(s), in1=b,
                                       op0=getattr(Alu, op0),
                                       op1=getattr(Alu, op1))

    def act(o, a, func, scale=1.0, bias=None, accum_out=None):
        if func == "Copy":
            nc.scalar.activation(out=o, in_=a, func=Act.Copy, bias=0.0,
                                 scale=float(scale), accum_out=accum_out)
        else:
            nc.scalar.activation(out=o, in_=a, func=getattr(Act, func),
                                 bias=bias0[:, 0:1] if bias is None else bias,
                                 scale=float(scale), accum_out=accum_out)

    def bc(plane, n):
        # broadcast [P, L] plane across n sub-planes -> [P, n, L]
        return bass_mod.AP(tensor=plane.tensor, offset=plane.offset,
                           ap=[plane.ap[0], [0, n], plane.ap[1]])

    A = lambda: regs.alloc(L)       # fp32 [P, L] planes
    H = lambda: regs16.alloc(L)     # f16 [P, L] planes

    # ---- cast+deinterleave both inputs into X[P, 9, 2T] (f16) ----
    rvp = praw[:, 0:T * 9].rearrange("p (n c) -> p n c", c=9)
    rvt = traw[:, 0:T * 9].rearrange("p (n c) -> p n c", c=9)
    xin_p = bass_mod.AP(tensor=rvp.tensor, offset=rvp.offset,
                        ap=[rvp.ap[0], rvp.ap[2], rvp.ap[1]])
    xin_t = bass_mod.AP(tensor=rvt.tensor, offset=rvt.offset,
                        ap=[rvt.ap[0], rvt.ap[2], rvt.ap[1]])
    pre_cast()
    act(X[:, :, 0:T], xin_p, "Copy")
    act(X[:, :, T:L], xin_t, "Copy")
    post_cast()
    x = [X[:, c, 0:L] for c in range(9)]        # [P, L] f16 unit-stride

    # ---- mse sub on GpSimd (off critical path) ----
    Dv = D[:, :, 0:T]
    nc.gpsimd.tensor_tensor(out=Dv, in0=X[:, :, 0:T], in1=X[:, :, T:L],
                            op=Alu.subtract)

    fill(6)

    # ---- det(M) f16 from X planes (emitted first: its ACT consumers
    #      run during the S block) ----
    cA16, cB16, det16 = H(), H(), H()
    mul(cA16, x[4], x[8]); mul(cB16, x[5], x[7]); sub(cA16, cA16, cB16)
    mul(det16, x[0], cA16)
    mul(cA16, x[5], x[6]); mul(cB16, x[3], x[8]); sub(cA16, cA16, cB16)
    mul(cA16, x[1], cA16); add(det16, det16, cA16)
    mul(cA16, x[3], x[7]); mul(cB16, x[4], x[6]); sub(cA16, cA16, cB16)
    mul(cA16, x[2], cA16); add(det16, det16, cA16)
    sgd16 = H(); act(sgd16, det16, "Sign")
    lnad, ad32 = A(), A()
    act(ad32, det16, "Abs")
    act(lnad, ad32, "Ln")
    regs.free(ad32)
    regs16.free(cA16, cB16, det16)
    # mse Square-accum here: its DVE wait is covered by the det ACT ops
    # above (walrus allows only one sync wait per Activation instruction)
    act(Dv, Dv, "Square", accum_out=acc_mse_col)

    # ---- S = M^T M (f16): order [s00,s11,s22,s01,s02,s12] ----
    tmp16 = H()
    for i in range(3):
        sd = Sm[:, i, 0:L]
        mul(sd, x[i], x[i])
        mul(tmp16, x[i + 3], x[i + 3]); add(sd, sd, tmp16)
        mul(tmp16, x[i + 6], x[i + 6]); add(sd, sd, tmp16)
    for oi, (ia, ib) in enumerate((((0, 3, 6), (1, 4, 7)),
                                   ((0, 3, 6), (2, 5, 8)),
                                   ((1, 4, 7), (2, 5, 8)))):
        so = Sm[:, 3 + oi, 0:L]
        mul(so, x[ia[0]], x[ib[0]])
        mul(tmp16, x[ia[1]], x[ib[1]]); add(so, so, tmp16)
        mul(tmp16, x[ia[2]], x[ib[2]]); add(so, so, tmp16)
    # tr and normalization scale q = 3/tr (ln domain)
    tr16 = H()
    add(tr16, Sm[:, 0, 0:L], Sm[:, 1, 0:L]); add(tr16, tr16, Sm[:, 2, 0:L])
    vs(tr16, tr16, 6e-5, "max")
    lnt = A(); act(lnt, tr16, "Ln")
    q16 = H(); act(q16, lnt, "Exp", scale=-1.0, bias=_c(nc, LN3))
    regs16.free(tr16)
    fill(5)
    nc.vector.tensor_tensor(out=Sm[:, :, 0:L], in0=Sm[:, :, 0:L],
                            in1=bc(q16, 6), op=Alu.mult)
    regs16.free(q16)

    # Pg = exp(lnad + 1.5*(ln3 - lnt));  dets = Pg^2
    lnPg = A()
    stt(lnPg, lnt, -1.5, lnad, "mult", "add")
    regs.free(lnad)
    Pg32 = A(); act(Pg32, lnPg, "Exp", scale=1.0, bias=_c(nc, 1.5 * LN3))
    Pg16 = H(); act(Pg16, Pg32, "Copy")
    dets = A(); act(dets, Pg32, "Square")
    regs.free(lnPg)

    # ---- tr(S^2) fp32 from normalized f16 S ----
    act(QS[:, :, 0:L], Sm[:, :, 0:L], "Square")
    fill(3)
    u1, u2 = A(), A()
    add(u1, QS[:, 0, 0:L], QS[:, 1, 0:L]); add(u1, u1, QS[:, 2, 0:L])
    add(u2, QS[:, 3, 0:L], QS[:, 4, 0:L]); add(u2, u2, QS[:, 5, 0:L])
    trS2 = A()
    stt(trS2, u2, 2.0, u1, "mult", "add")
    # p and 1/(2 p^3) via ln/exp
    trK2, p, ip3h = u1, A(), u2            # reuse u1/u2 slots
    vs(trK2, trS2, -3.0, "add", 1e-30, "max")
    lnk = A(); act(lnk, trK2, "Ln")
    act(p, lnk, "Exp", scale=0.5, bias=_c(nc, -0.5 * LN6))
    act(ip3h, lnk, "Exp", scale=-1.5, bias=_c(nc, 1.5 * LN6 - LN2))
    regs.free(lnk)
    # arg
    detK, arg = A(), A()
    stt(detK, trS2, 0.5, dets, "mult", "add")
    vs(detK, detK, -2.5, "add")
    fill(3)
    vs(ip3h, ip3h, 1e30, "min")
    mul(arg, detK, ip3h)
    vs(arg, arg, 1.0, "min", -1.0, "max")
    regs.free(detK, trS2, dets, u2)   # u2 == ip3h
    # ---- th3 = acos(arg) ----
    y, om, h = A(), A(), A()
    act(y, arg, "Abs")
    sg_a = A(); act(sg_a, arg, "Sign")
    fill(2)
    vs(om, y, -1.0, "mult", 1.0, "add")
    lnom = A(); act(lnom, om, "Ln")
    act(om, lnom, "Exp", scale=0.5)              # sqrt(1-y)
    regs.free(lnom)
    vs(h, y, ACOS_A[3], "mult", ACOS_A[2], "add")
    mul(h, h, y); vs(h, h, ACOS_A[1], "add")
    mul(h, h, y); vs(h, h, ACOS_A[0], "add")
    fill(3)
    mul(h, h, om)
    th3 = om                                     # reuse slot
    vs(arg, sg_a, -np.pi / 2, "mult", np.pi / 2, "add")
    mul(th3, sg_a, h); add(th3, th3, arg)
    regs.free(h, arg, y, sg_a)    # th3 == om stays
    # ---- cos((th3+2pik)/3) ----
    z, c0p = A(), A()
    act(z, th3, "Square", scale=1.0 / 3.0)
    fill(2)
    vs(c0p, z, 1.0 / 40320.0, "mult", -1.0 / 720.0, "add")
    mul(c0p, c0p, z); vs(c0p, c0p, 1.0 / 24.0, "add")
    mul(c0p, c0p, z); vs(c0p, c0p, -0.5, "add")
    mul(c0p, c0p, z); vs(c0p, c0p, 1.0, "add")
    regs.free(z, om)   # om == th3
    s0, uc1, c1p, c2p = A(), A(), A(), A()
    act(s0, c0p, "Square")
    vs(uc1, c0p, -0.5, "mult")
    fill(2)
    vs(s0, s0, -1.0, "mult", 1.0, "add")
    vs(s0, s0, 0.0, "max")
    lns = A(); act(lns, s0, "Ln")
    act(s0, lns, "Exp", scale=0.5)
    regs.free(lns)
    fill(3)
    vs(s0, s0, HALF_SQRT3, "mult")
    sub(c1p, uc1, s0)
    add(c2p, uc1, s0)
    regs.free(s0, uc1)
    # ---- lambda_k, g_k = sqrt(lambda_k) ----
    tp = A()
    vs(tp, p, 2.0, "mult")
    regs.free(p)
    lam, g, lnls = [], [], []
    for ck in (c0p, c1p, c2p):
        lk, lnl = A(), A()
        mul(lk, tp, ck)
        vs(lk, lk, 1.0, "add", 1e-35, "max")
        act(lnl, lk, "Ln")
        lam.append(lk); lnls.append(lnl)
    for lnl in lnls:
        gk = A()
        act(gk, lnl, "Exp", scale=0.5)
        g.append(gk)
        regs.free(lnl)
    regs.free(tp, c0p, c1p, c2p)
    fill(5)
    g01, ssig, s2i, tmp2 = A(), A(), A(), A()
    add(g01, g[0], g[1])
    add(ssig, g01, g[2])
    mul(s2i, g[0], g[1]); mul(tmp2, g[2], g01); add(s2i, s2i, tmp2)
    regs.free(g01, tmp2, *g)
    # ---- w = sign/(sqrt(m)*detN) via ln domain, clamped ----
    # detN = prod_k (ssig*lam_k + Pg): form the product, one Ln.
    t_, nk0, nk1 = A(), A(), A()
    mul(nk0, ssig, lam[0]); add(nk0, nk0, Pg32)
    mul(nk1, ssig, lam[1]); add(nk1, nk1, Pg32)
    mul(nk0, nk0, nk1)
    mul(nk1, ssig, lam[2]); add(nk1, nk1, Pg32)
    mul(nk0, nk0, nk1)
    act(t_, nk0, "Ln")
    ssig16, s2i16 = H(), H()
    act(ssig16, ssig, "Copy")
    act(s2i16, s2i, "Copy")
    fill(3)
    stt(t_, lnt, 0.5, t_, "mult", "add")
    vs(t_, t_, float(np.log(EPS_W) + 0.5 * LN3), "max")
    wmag16 = H()
    act(wmag16, t_, "Exp", scale=-1.0, bias=_c(nc, 0.5 * LN3))
    regs.free(t_, nk0, nk1, lnt, ssig, s2i, Pg32, *lam)

    # ---- N = ssig*S + Pg*I (f16, batched) ----
    fill(3)
    nc.vector.tensor_tensor(out=Nm[:, :, 0:L], in0=Sm[:, :, 0:L],
                            in1=bc(ssig16, 6), op=Alu.mult)
    nc.vector.tensor_tensor(out=Nm[:, 0:3, 0:L], in0=Nm[:, 0:3, 0:L],
                            in1=bc(Pg16, 3), op=Alu.add)
    regs16.free(Pg16, ssig16)
    # A' diagonal (Am = S_diag + s2)
    nc.vector.tensor_tensor(out=Am[:, :, 0:L], in0=Sm[:, 0:3, 0:L],
                            in1=bc(s2i16, 3), op=Alu.add)
    regs16.free(s2i16)
    # w16 = sign(det) * wmag  (into the double-buffered W16 plane)
    mul(W16[:, 0:L], wmag16, sgd16)
    regs16.free(wmag16, sgd16)
    # ---- adj(N) (f16) ----
    n00, n11, n22 = (Nm[:, i, 0:L] for i in range(3))
    n01, n02, n12 = (Nm[:, i, 0:L] for i in range(3, 6))
    aj = [H() for _ in range(6)]
    a00, a01, a02, a11, a12, a22 = aj

    def cof(o, a, b, c, dd):
        mul(o, a, b); mul(tmp16, c, dd); sub(o, o, tmp16)

    cof(a00, n11, n22, n12, n12)
    cof(a01, n02, n12, n01, n22)
    cof(a02, n01, n12, n02, n11)
    cof(a11, n00, n22, n02, n02)
    cof(a12, n01, n02, n00, n12)
    cof(a22, n00, n11, n01, n01)
    # ---- T1 = A' adjN (f16); rows of A': (b0,s01,s02),(s01,b1,s12),(s02,s12,b2)
    b0, b1, b2 = (Am[:, i, 0:L] for i in range(3))
    s01p, s02p, s12p = Sm[:, 3, 0:L], Sm[:, 4, 0:L], Sm[:, 5, 0:L]

    def mm3(o, r0, r1, r2, k0, k1, k2):
        mul(o, r0, k0)
        mul(tmp16, r1, k1); add(o, o, tmp16)
        mul(tmp16, r2, k2); add(o, o, tmp16)

    mm3(T1m[:, 0, 0:L], b0, s01p, s02p, a00, a01, a02)
    mm3(T1m[:, 1, 0:L], b0, s01p, s02p, a01, a11, a12)
    mm3(T1m[:, 2, 0:L], b0, s01p, s02p, a02, a12, a22)
    mm3(T1m[:, 3, 0:L], s01p, b1, s12p, a01, a11, a12)
    mm3(T1m[:, 4, 0:L], s01p, b1, s12p, a02, a12, a22)
    mm3(T1m[:, 5, 0:L], s02p, s12p, b2, a02, a12, a22)
    regs16.free(*aj)
    regs16.free(tmp16)


def _make_B(nc, env, k, T, tiles, acc_rot_col):
    """Phase B of chunk k: T2 = w*T1, R = M*T2 as a list of single-op DVE
    thunks (drained as filler during chunk k+1's phase A), plus the GpSimd
    tail (clamp + dR sub) and the ACT accumulate thunk (emitted early in
    chunk k+2)."""
    mybir = env["mybir"]
    Alu = mybir.AluOpType
    Act = mybir.ActivationFunctionType
    L = 2 * T
    X, T1m, Nm, W16, RT, BT = (tiles[n] for n in
                               ("X", "T1m", "Nm", "W16", "RT", "BT"))
    x = [X[:, c, 0:L] for c in range(9)]

    def mul(o, a, b):
        nc.vector.tensor_tensor(out=o, in0=a, in1=b, op=Alu.mult)

    def add(o, a, b):
        nc.vector.tensor_tensor(out=o, in0=a, in1=b, op=Alu.add)

    def bc(plane, n):
        return bass_mod.AP(tensor=plane.tensor, offset=plane.offset,
                           ap=[plane.ap[0], [0, n], plane.ap[1]])

    dve = []
    # T2 = w*T1 in f16 (Nm tile is dead after adj; reuse it)
    T2f = Nm
    dve.append(lambda: nc.vector.tensor_tensor(
        out=T2f[:, :, 0:L], in0=T1m[:, :, 0:L], in1=bc(W16[:, 0:L], 6),
        op=Alu.mult))
    t00, t01, t02 = T2f[:, 0, 0:L], T2f[:, 1, 0:L], T2f[:, 2, 0:L]
    t11, t12, t22 = T2f[:, 3, 0:L], T2f[:, 4, 0:L], T2f[:, 5, 0:L]
    T2 = [[t00, t01, t02], [t01, t11, t12], [t02, t12, t22]]
    tmpr = BT[:, 0:L]
    for i in range(3):
        for j in range(3):
            o = RT[:, 3 * i + j, 0:L]
            dve.append(lambda o=o, i=i, j=j: mul(o, x[3 * i], T2[0][j]))
            dve.append(lambda i=i, j=j: mul(tmpr, x[3 * i + 1], T2[1][j]))
            dve.append(lambda o=o: add(o, o, tmpr))
            dve.append(lambda i=i, j=j: mul(tmpr, x[3 * i + 2], T2[2][j]))
            dve.append(lambda o=o: add(o, o, tmpr))

    def pool_tail():
        # clamp junk rows, dR = R_pred - R_target in place (off DVE)
        nc.gpsimd.tensor_scalar(out=RT[:, :, 0:L], in0=RT[:, :, 0:L],
                                scalar1=8.0, scalar2=-8.0,
                                op0=Alu.min, op1=Alu.max)
        nc.gpsimd.tensor_tensor(out=RT[:, :, 0:T], in0=RT[:, :, 0:T],
                                in1=RT[:, :, T:L], op=Alu.subtract)

    def act_accum():
        bias0 = env["bias0"]
        nc.scalar.activation(out=RT[:, :, 0:T], in_=RT[:, :, 0:T],
                             func=Act.Square, bias=bias0[:, 0:1], scale=1.0,
                             accum_out=acc_rot_col)

    return dve, pool_tail, act_accum


def _build_nc():
    global bass_mod
    import concourse.bass as bass
    import concourse.tile as tile
    from concourse import mybir
    bass_mod = bass

    f32 = mybir.dt.float32
    f16 = mybir.dt.float16
    nc = bass.Bass()
    pred = nc.dram_tensor("pred", [ROWS_PER_CORE, 9], f32, kind="ExternalInput")
    targ = nc.dram_tensor("target", [ROWS_PER_CORE, 9], f32, kind="ExternalInput")
    out = nc.dram_tensor("partials", [P, 2 * NCHUNK], f32, kind="ExternalOutput")

    predv = pred.rearrange("(p n) c -> p n c", p=P)    # [128, 1024, 9]
    targv = targ.rearrange("(p n) c -> p n c", p=P)
    row0 = np.cumsum((0,) + CHUNKS)                    # chunk row offsets

    with tile.TileContext(nc) as tc:
        with (
            tc.tile_pool(name="raw", bufs=1) as rawp,
            tc.tile_pool(name="pl", bufs=1) as pl,
            tc.tile_pool(name="acc", bufs=1) as accp,
        ):
            acc = accp.tile([P, 2 * NCHUNK], f32, tag="acc")
            bias0 = accp.tile([P, 1], f32, tag="bias0")
            nc.vector.memset(bias0, 0.0)
            _CONST_STATE[(id(nc), "pool")] = accp
            env = {
                "mybir": mybir,
                "regs": Regs(pl, f32, prefix="v"),
                "regs16": Regs(pl, f16, prefix="h"),
                "bias0": bias0,
            }

            raw_cache = {}

            def raw_tiles(k):
                if k not in raw_cache:
                    raw_cache[k] = (
                        rawp.tile([P, TMAX * 9], f32, tag=f"praw{k % 2}",
                                  name=f"praw{k % 2}"),
                        rawp.tile([P, TMAX * 9], f32, tag=f"traw{k % 2}",
                                  name=f"traw{k % 2}"))
                return raw_cache[k]

            def chunk_tiles(k):
                j = k % 2
                return {
                    "X": pl.tile([P, 9, 2 * TMAX], f16, tag=f"X{j}", name=f"X{j}"),
                    "D": pl.tile([P, 9, TMAX], f16, tag="D", name="D"),
                    "Sm": pl.tile([P, 6, 2 * TMAX], f16, tag=f"Sm{j}", name=f"Sm{j}"),
                    "QS": pl.tile([P, 6, 2 * TMAX], f32, tag="QS", name="QS"),
                    "Am": pl.tile([P, 3, 2 * TMAX], f16, tag="Am", name="Am"),
                    "T1m": pl.tile([P, 6, 2 * TMAX], f16, tag=f"T1m{j}", name=f"T1m{j}"),
                    "Nm": pl.tile([P, 6, 2 * TMAX], f16, tag=f"Nm{j}", name=f"Nm{j}"),
                    "W16": pl.tile([P, 2 * TMAX], f16, tag=f"W16{j}", name=f"W16{j}"),
                    "RT": pl.tile([P, 9, 2 * TMAX], f16, tag=f"RT{j}", name=f"RT{j}"),
                    "BT": pl.tile([P, 2 * TMAX], f16, tag=f"BT{j}", name=f"BT{j}"),
                }

            def dma_chunk(k):
                prw, trw = raw_tiles(k)
                t0, t1 = row0[k], row0[k + 1]
                n = (t1 - t0) * 9
                nc.sync.dma_start(out=prw[:, 0:n], in_=predv[:, t0:t1, :])
                nc.sync.dma_start(out=trw[:, 0:n], in_=targv[:, t0:t1, :])

            dma_chunk(0)
            dma_chunk(1)

            pending = []        # phase-B DVE thunks of chunk k-1
            tail_q = []         # (pool_tail, act_accum) of chunk k-1
            accum_slot = []     # ACT accumulates to emit at next post_cast

            def filler(n):
                for _ in range(min(n, len(pending))):
                    pending.pop(0)()

            for k, T in enumerate(CHUNKS):
                praw, traw = raw_tiles(k)
                tiles = chunk_tiles(k)

                def pre_cast():
                    while accum_slot:
                        accum_slot.pop(0)()

                def post_cast(k=k):
                    if k + 2 < NCHUNK:
                        dma_chunk(k + 2)

                _emit_A(nc, env, k, T, praw, traw, tiles,
                        acc[:, k:k + 1], filler, pre_cast, post_cast)
                # drain the rest of chunk k-1's phase B, then its GpSimd tail;
                # its ACT accumulate goes early into chunk k+1
                filler(len(pending))
                if tail_q:
                    pool_tail, act_accum = tail_q.pop(0)
                    pool_tail()
                    accum_slot.append(act_accum)
                dve, pool_tail, act_accum = _make_B(
                    nc, env, k, T, tiles, acc[:, NCHUNK + k:NCHUNK + k + 1])
                pending = dve
                tail_q.append((pool_tail, act_accum))

            # drain the pipeline: B of the last chunk, tails, accumulates
            filler(len(pending))
            while accum_slot:
                accum_slot.pop(0)()
            while tail_q:
                pool_tail, act_accum = tail_q.pop(0)
                pool_tail()
                act_accum()
            nc.sync.dma_start(out=out[:, :], in_=acc)
    return nc


def _elide_implied_waits(nc):
    """Drop semaphore waits already implied by program order or transitively
    by earlier waits (vector-clock propagation).  Tile's per-instruction wait
    emission is not transitively minimal, and walrus can encode only one sync
    wait on Activation/DMA instructions (and ~4 on control instructions), so
    the redundant waits both break codegen and waste sequencer time.

    Model: each semaphore s carries a snapshot VC at every increment value;
    an engine's observed VC advances via its own instruction stream and via
    the snapshots of the waits it executes.  A wait (s >= v) is dropped iff
    the engine's observed VC already dominates it.  Unknown update modes
    disable elision for that semaphore (conservative).
    """
    join = lambda a, b: {k: max(a.get(k, 0), b.get(k, 0)) for k in set(a) | set(b)}
    sem_val = {}        # sem name -> current value
    sem_snap = {}       # sem name -> list of (value, VC) snapshots
    eng_vc = {}         # engine name -> observed VC
    unsafe = set()      # sems with non-increment updates
    n_drop = 0
    for f in nc.m.functions:
        for bb in f.blocks:
            for ins in bb.instructions:
                eng = str(ins.engine)
                vc = dict(eng_vc.get(eng, {}))
                si = ins.sync_info
                waits = list(si.on_wait) if si is not None and si.on_wait else []
                kept = []
                for w in waits:
                    s, v = w.ant_name, w.wait_value
                    if w.wait_mode != "sem-ge-imm" or s in unsafe:
                        kept.append(w)
                        continue
                    if vc.get(s, 0) >= v:
                        n_drop += 1
                        continue
                    if sem_val.get(s, 0) < v:
                        # increment not yet seen in emission order; keep and
                        # learn nothing (conservative)
                        kept.append(w)
                        continue
                    kept.append(w)
                    snap = {}
                    for sv, svc in sem_snap.get(s, ()):
                        if sv <= v:
                            snap = svc
                        else:
                            break
                    vc = join(vc, snap)
                    vc[s] = max(vc.get(s, 0), v)
                if si is not None and len(kept) != len(waits):
                    si.on_wait = kept
                # apply this instruction's increments
                ups = si.on_update if si is not None and si.on_update else []
                for u in ups:
                    s = u.ant_name
                    if u.update_mode not in ("sem-inc", "sem-add-imm"):
                        unsafe.add(s)
                        continue
                    nv = sem_val.get(s, 0) + (u.update_value or 1)
                    sem_val[s] = nv
                    lst = sem_snap.setdefault(s, [])
                    prev = lst[-1][1] if lst else {}
                    lst.append((nv, join(prev, vc)))
                    # Engine-sem increments fire when the instruction
                    # completes, and the engine is sequential, so later
                    # instructions on this engine observe them.  DMA-queue
                    # increments fire asynchronously at transfer completion:
                    # the issuing engine must NOT absorb those.
                    if "DMA" not in s:
                        vc[s] = max(vc.get(s, 0), nv)
                eng_vc[eng] = vc
    return n_drop


def _spill_excess_waits(nc):
    """walrus encodes at most ONE sync wait on real engine instructions
    (Activation/DVE/DMA); the Tile scheduler can leave more after
    cross-engine reordering.  Keep one wait on the instruction and hoist
    the rest onto preceding InstEventSemaphore control instructions
    (which accept 2 waits each)."""
    from concourse import mybir
    n_spill = 0
    for f in nc.m.functions:
        for bb in f.blocks:
            out = []
            for ins in bb.instructions:
                si = ins.sync_info
                waits = list(si.on_wait) if si is not None and si.on_wait else []
                is_engine_op = bool(ins.ins) or bool(ins.outs)
                if len(waits) > 1 and is_engine_op and \
                        not isinstance(ins, mybir.InstEventSemaphore):
                    spill = waits[:-1]
                    si.on_wait = waits[-1:]
                    while spill:
                        grp, spill = spill[:2], spill[2:]
                        n_spill += 1
                        ev = mybir.InstEventSemaphore(
                            name=f"wspill_{n_spill}", engine=ins.engine,
                            ins=[], outs=[],
                            sync_info=mybir.SyncInfo(on_wait=grp,
                                                     on_update=[]))
                        out.append(ev)
                out.append(ins)
            bb.instructions = out
    return n_spill


_NC_CACHE = None


def kernel(pred: np.ndarray, target: np.ndarray) -> np.ndarray:
    global _NC_CACHE
    from concourse.bass_utils import run_bass_kernel_spmd

    pred = np.ascontiguousarray(np.asarray(pred, dtype=np.float32))
    target = np.ascontiguousarray(np.asarray(target, dtype=np.float32))
    assert pred.shape == (B, 9) and target.shape == (B, 9)

    if _NC_CACHE is None:
        _NC_CACHE = _build_nc()
        _elide_implied_waits(_NC_CACHE)
        _spill_excess_waits(_NC_CACHE)
    nc = _NC_CACHE

    ps = pred.reshape(N_CORES, ROWS_PER_CORE, 9)
    ts = target.reshape(N_CORES, ROWS_PER_CORE, 9)
    in_maps = [{"pred": ps[i], "target": ts[i]} for i in range(N_CORES)]
    res = run_bass_kernel_spmd(nc, in_maps, core_ids=list(range(N_CORES)))
    globals()["_LAST_RESULT"] = res

    mse_sum = 0.0
    rot_sum = 0.0
    for r in res.results:
        part = np.asarray(r["partials"], dtype=np.float64)
        mse_sum += part[:, :NCHUNK].sum()
        rot_sum += part[:, NCHUNK:].sum()
    n = float(B * 9)
    return np.asarray(np.float32(mse_sum / n + 0.5 * (rot_sum / n)))


# revision 22
# speedup vs baseline: 1.1305x; 1.0126x over previous
"""CustomPoseLoss Trainium2 kernel.

loss = mean((pred-target)^2) + 0.5 * mean((R(pred)-R(target))^2)
where R(M) = sign(det M) * polar(M) for each 3x3 matrix (row of 9).

Implementation: closed-form polar decomposition per row, fully vectorized as
channel-plane arithmetic:
  S = M^T M, normalized by tr(S)/3; eigenvalues of S via Cardano
  (acos/cos evaluated as polynomials so only the ln/exp LUT set is needed);
  W^-1 = (S + s2 I) adj(N) / det(N) with N = ssig*S + Pg*I  (Cayley-Hamilton
  inverse-sqrt);  R = sign(det) * M W^-1 / sqrt(m).
det(N) is formed from the eigenvalue product (positive, cancellation-free)
and clamped, so near-singular rows stay bounded.

Engine plan: heavy products in f16 on DVE (2x mode); scalar chain fp32 on
DVE+ACT; mse-sub / clamp / dR-sub on GpSimd (off critical path).  Chunks are
software-pipelined: chunk k's tail (T2, R = M*T2) is emitted as filler inside
chunk k+1's scalar-chain stalls, so the Vector engine never waits on ACT legs.

Sharding: pure data parallel over 8 cores; each core reduces its shard to
[128, 2*NCHUNK] partial sums (mse, rot), host combines in float64.
"""

import numpy as np

B = 1048576
N_CORES = 8
ROWS_PER_CORE = B // N_CORES          # 131072
P = 128
ROWS_PER_PART = ROWS_PER_CORE // P    # 1024
CHUNKS = (256, 256, 256, 256)         # rows per partition per chunk
NCHUNK = len(CHUNKS)
TMAX = max(CHUNKS)
EPS_D = 1e-5

ACOS_A = (1.5707288, -0.2121144, 0.0742610, -0.0187293)   # A&S 4.4.45
HALF_SQRT3 = 0.8660254037844386

LN3 = float(np.log(3.0))
LN6 = float(np.log(6.0))
LN2 = float(np.log(2.0))
EPS_W = 6e-3


class Regs:
    """[128, 2*TMAX] plane slots with explicit reuse (SBUF is capped)."""

    def __init__(self, pool, dtype, prefix="v", shape=None):
        self.pool = pool
        self.dtype = dtype
        self.prefix = prefix
        self.shape = shape or [P, 2 * TMAX]
        self.free_tags = []
        self.n = 0
        self.tag_of = {}

    def alloc(self, width=None):
        if self.free_tags:
            tag = self.free_tags.pop()
        else:
            self.n += 1
            tag = f"{self.prefix}{self.n}"
        tl = self.pool.tile(self.shape, self.dtype, tag=tag, name=tag)
        if width is not None:
            tl = tl[:, 0:width]
        self.tag_of[id(tl)] = tag
        return tl

    def free(self, *tiles):
        for tl in tiles:
            self.free_tags.append(self.tag_of.pop(id(tl)))


_CONST_STATE = {}
bass_mod = None


def _c(nc, v):
    """[P,1] fp32 constant AP, DVE-memset once (keeps ACT single-wait)."""
    key = float(np.float32(v))
    consts = _CONST_STATE.setdefault(id(nc), {})
    if key not in consts:
        pool = _CONST_STATE[(id(nc), "pool")]
        from concourse import mybir
        t = pool.tile([P, 1], mybir.dt.float32, tag=f"c{len(consts)}")
        nc.vector.memset(t, key)
        consts[key] = t
    return consts[key][:, 0:1]


def _emit_A(nc, env, k, T, praw, traw, tiles, acc_mse_col, fill, pre_cast,
            post_cast):
    """Phase A of chunk k: everything through T1m and w16.

    `fill(n)` emits up to n pending phase-B DVE ops from the previous chunk
    at known ACT-wait stall points.  `pre_cast()` emits the chunk k-2 ACT
    accumulate (whose Pool wait transitively covers the casts' X-tile WAR
    wait — walrus allows only one sync wait per Activation instruction);
    `post_cast()` emits the chunk k+2 DMA prefetch.
    """
    mybir = env["mybir"]
    regs, regs16, bias0 = env["regs"], env["regs16"], env["bias0"]
    Alu = mybir.AluOpType
    Act = mybir.ActivationFunctionType
    L = 2 * T
    X, D, Sm, QS, Am, T1m, Nm, W16 = (tiles[n] for n in
                                      ("X", "D", "Sm", "QS", "Am", "T1m",
                                       "Nm", "W16"))

    def mul(o, a, b):
        nc.vector.tensor_tensor(out=o, in0=a, in1=b, op=Alu.mult)

    def add(o, a, b):
        nc.vector.tensor_tensor(out=o, in0=a, in1=b, op=Alu.add)

    def sub(o, a, b):
        nc.vector.tensor_tensor(out=o, in0=a, in1=b, op=Alu.subtract)

    def vs(o, a, s1, op0, s2=None, op1=None):
        if s2 is None:
            nc.vector.tensor_scalar(out=o, in0=a, scalar1=float(s1),
                                    scalar2=None, op0=getattr(Alu, op0))
        else:
            nc.vector.tensor_scalar(out=o, in0=a, scalar1=float(s1),
                                    scalar2=float(s2), op0=getattr(Alu, op0),
                                    op1=getattr(Alu, op1))

    def stt(o, a, s, b, op0, op1):
        nc.vector.scalar_tensor_tensor(out=o, in0=a, scalar=float(s), in1=b,
                                       op0=getattr(Alu, op0),
                                       op1=getattr(Alu, op1))

    def act(o, a, func, scale=1.0, bias=None, accum_out=None):
        if func == "Copy":
            nc.scalar.activation(out=o, in_=a, func=Act.Copy, bias=0.0,
                                 scale=float(scale), accum_out=accum_out)
        else:
            nc.scalar.activation(out=o, in_=a, func=getattr(Act, func),
                                 bias=bias0[:, 0:1] if bias is None else bias,
                                 scale=float(scale), accum_out=accum_out)

    def bc(plane, n):
        # broadcast [P, L] plane across n sub-planes -> [P, n, L]
        return bass_mod.AP(tensor=plane.tensor, offset=plane.offset,
                           ap=[plane.ap[0], [0, n], plane.ap[1]])

    A = lambda: regs.alloc(L)       # fp32 [P, L] planes
    H = lambda: regs16.alloc(L)     # f16 [P, L] planes

    # ---- cast+deinterleave both inputs into X[P, 9, 2T] (f16) ----
    rvp = praw[:, 0:T * 9].rearrange("p (n c) -> p n c", c=9)
    rvt = traw[:, 0:T * 9].rearrange("p (n c) -> p n c", c=9)
    xin_p = bass_mod.AP(tensor=rvp.tensor, offset=rvp.offset,
                        ap=[rvp.ap[0], rvp.ap[2], rvp.ap[1]])
    xin_t = bass_mod.AP(tensor=rvt.tensor, offset=rvt.offset,
                        ap=[rvt.ap[0], rvt.ap[2], rvt.ap[1]])
    pre_cast()
    act(X[:, :, 0:T], xin_p, "Copy")
    act(X[:, :, T:L], xin_t, "Copy")
    post_cast()
    x = [X[:, c, 0:L] for c in range(9)]        # [P, L] f16 unit-stride

    # ---- channel squares for the S diagonal (ACT; overlaps DVE det/S) ----
    act(tiles["SQ"][:, :, 0:L], X[:, :, 0:L], "Square")

    # ---- mse sub on GpSimd (off critical path) ----
    Dv = D[:, :, 0:T]
    nc.gpsimd.tensor_tensor(out=Dv, in0=X[:, :, 0:T], in1=X[:, :, T:L],
                            op=Alu.subtract)

    fill(6)

    # ---- det(M) f16 from X planes (emitted first: its ACT consumers
    #      run during the S block) ----
    cA16, cB16, det16 = H(), H(), H()
    mul(cA16, x[4], x[8]); mul(cB16, x[5], x[7]); sub(cA16, cA16, cB16)
    mul(det16, x[0], cA16)
    mul(cA16, x[5], x[6]); mul(cB16, x[3], x[8]); sub(cA16, cA16, cB16)
    mul(cA16, x[1], cA16); add(det16, det16, cA16)
    mul(cA16, x[3], x[7]); mul(cB16, x[4], x[6]); sub(cA16, cA16, cB16)
    mul(cA16, x[2], cA16); add(det16, det16, cA16)
    sgd16 = H(); act(sgd16, det16, "Sign")
    lnad, ad32 = A(), A()
    act(ad32, det16, "Abs")
    act(lnad, ad32, "Ln")
    regs.free(ad32)
    regs16.free(cA16, cB16, det16)
    # mse Square-accum here: its DVE wait is covered by the det ACT ops
    # above (walrus allows only one sync wait per Activation instruction)
    act(Dv, Dv, "Square", accum_out=acc_mse_col)

    # ---- S = M^T M (f16): order [s00,s11,s22,s01,s02,s12] ----
    # diag from the ACT-computed channel squares (SQ = X*X): 2 DVE adds
    tmp16 = H()
    SQ = tiles["SQ"]
    add(Sm[:, 0:3, 0:L], SQ[:, 0:3, 0:L], SQ[:, 3:6, 0:L])
    add(Sm[:, 0:3, 0:L], Sm[:, 0:3, 0:L], SQ[:, 6:9, 0:L])
    for oi, (ia, ib) in enumerate((((0, 3, 6), (1, 4, 7)),
                                   ((0, 3, 6), (2, 5, 8)),
                                   ((1, 4, 7), (2, 5, 8)))):
        so = Sm[:, 3 + oi, 0:L]
        mul(so, x[ia[0]], x[ib[0]])
        mul(tmp16, x[ia[1]], x[ib[1]]); add(so, so, tmp16)
        mul(tmp16, x[ia[2]], x[ib[2]]); add(so, so, tmp16)
    # tr and normalization scale q = 3/tr (ln domain)
    tr16 = H()
    add(tr16, Sm[:, 0, 0:L], Sm[:, 1, 0:L]); add(tr16, tr16, Sm[:, 2, 0:L])
    vs(tr16, tr16, 6e-5, "max")
    lnt = A(); act(lnt, tr16, "Ln")
    q16 = H(); act(q16, lnt, "Exp", scale=-1.0, bias=_c(nc, LN3))
    regs16.free(tr16)
    fill(5)
    nc.vector.tensor_tensor(out=Sm[:, :, 0:L], in0=Sm[:, :, 0:L],
                            in1=bc(q16, 6), op=Alu.mult)
    regs16.free(q16)

    # Pg = exp(lnad + 1.5*(ln3 - lnt));  dets = Pg^2
    lnPg = A()
    stt(lnPg, lnt, -1.5, lnad, "mult", "add")
    regs.free(lnad)
    Pg32 = A(); act(Pg32, lnPg, "Exp", scale=1.0, bias=_c(nc, 1.5 * LN3))
    Pg16 = H(); act(Pg16, Pg32, "Copy")
    dets = A(); act(dets, Pg32, "Square")
    regs.free(lnPg)

    # ---- tr(S^2) fp32 from normalized f16 S ----
    act(QS[:, :, 0:L], Sm[:, :, 0:L], "Square")
    fill(3)
    u1, u2 = A(), A()
    add(u1, QS[:, 0, 0:L], QS[:, 1, 0:L]); add(u1, u1, QS[:, 2, 0:L])
    add(u2, QS[:, 3, 0:L], QS[:, 4, 0:L]); add(u2, u2, QS[:, 5, 0:L])
    trS2 = A()
    stt(trS2, u2, 2.0, u1, "mult", "add")
    # p and 1/(2 p^3) via ln/exp
    trK2, p, ip3h = u1, A(), u2            # reuse u1/u2 slots
    vs(trK2, trS2, -3.0, "add", 1e-30, "max")
    lnk = A(); act(lnk, trK2, "Ln")
    act(p, lnk, "Exp", scale=0.5, bias=_c(nc, -0.5 * LN6))
    act(ip3h, lnk, "Exp", scale=-1.5, bias=_c(nc, 1.5 * LN6 - LN2))
    regs.free(lnk)
    # arg
    detK, arg = A(), A()
    stt(detK, trS2, 0.5, dets, "mult", "add")
    vs(detK, detK, -2.5, "add")
    fill(3)
    vs(ip3h, ip3h, 1e30, "min")
    mul(arg, detK, ip3h)
    vs(arg, arg, 1.0, "min", -1.0, "max")
    regs.free(detK, trS2, dets, u2)   # u2 == ip3h
    # ---- th3 = acos(arg) ----
    y, om, h = A(), A(), A()
    act(y, arg, "Abs")
    sg_a = A(); act(sg_a, arg, "Sign")
    fill(2)
    vs(om, y, -1.0, "mult", 1.0, "add")
    lnom = A(); act(lnom, om, "Ln")
    act(om, lnom, "Exp", scale=0.5)              # sqrt(1-y)
    regs.free(lnom)
    vs(h, y, ACOS_A[3], "mult", ACOS_A[2], "add")
    mul(h, h, y); vs(h, h, ACOS_A[1], "add")
    mul(h, h, y); vs(h, h, ACOS_A[0], "add")
    fill(3)
    mul(h, h, om)
    th3 = om                                     # reuse slot
    vs(arg, sg_a, -np.pi / 2, "mult", np.pi / 2, "add")
    mul(th3, sg_a, h); add(th3, th3, arg)
    regs.free(h, arg, y, sg_a)    # th3 == om stays
    # ---- cos((th3+2pik)/3) ----
    z, c0p = A(), A()
    act(z, th3, "Square", scale=1.0 / 3.0)
    fill(2)
    vs(c0p, z, 1.0 / 40320.0, "mult", -1.0 / 720.0, "add")
    mul(c0p, c0p, z); vs(c0p, c0p, 1.0 / 24.0, "add")
    mul(c0p, c0p, z); vs(c0p, c0p, -0.5, "add")
    mul(c0p, c0p, z); vs(c0p, c0p, 1.0, "add")
    regs.free(z, om)   # om == th3
    s0, uc1, c1p, c2p = A(), A(), A(), A()
    act(s0, c0p, "Square")
    vs(uc1, c0p, -0.5, "mult")
    fill(2)
    vs(s0, s0, -1.0, "mult", 1.0, "add")
    vs(s0, s0, 0.0, "max")
    lns = A(); act(lns, s0, "Ln")
    act(s0, lns, "Exp", scale=0.5)
    regs.free(lns)
    fill(3)
    vs(s0, s0, HALF_SQRT3, "mult")
    sub(c1p, uc1, s0)
    add(c2p, uc1, s0)
    regs.free(s0, uc1)
    # ---- lambda_k, g_k = sqrt(lambda_k) ----
    tp = A()
    vs(tp, p, 2.0, "mult")
    regs.free(p)
    lam, g, lnls = [], [], []
    for ck in (c0p, c1p, c2p):
        lk, lnl = A(), A()
        mul(lk, tp, ck)
        vs(lk, lk, 1.0, "add", 1e-35, "max")
        act(lnl, lk, "Ln")
        lam.append(lk); lnls.append(lnl)
    for lnl in lnls:
        gk = A()
        act(gk, lnl, "Exp", scale=0.5)
        g.append(gk)
        regs.free(lnl)
    regs.free(tp, c0p, c1p, c2p)
    fill(5)
    g01, ssig, s2i, tmp2 = A(), A(), A(), A()
    add(g01, g[0], g[1])
    add(ssig, g01, g[2])
    mul(s2i, g[0], g[1]); mul(tmp2, g[2], g01); add(s2i, s2i, tmp2)
    regs.free(g01, tmp2, *g)
    # ---- w = sign/(sqrt(m)*detN) via ln domain, clamped ----
    # detN = prod_k (ssig*lam_k + Pg): form the product, one Ln.
    t_, nk0, nk1 = A(), A(), A()
    mul(nk0, ssig, lam[0]); add(nk0, nk0, Pg32)
    mul(nk1, ssig, lam[1]); add(nk1, nk1, Pg32)
    mul(nk0, nk0, nk1)
    mul(nk1, ssig, lam[2]); add(nk1, nk1, Pg32)
    mul(nk0, nk0, nk1)
    act(t_, nk0, "Ln")
    ssig16, s2i16 = H(), H()
    act(ssig16, ssig, "Copy")
    act(s2i16, s2i, "Copy")
    fill(3)
    stt(t_, lnt, 0.5, t_, "mult", "add")
    vs(t_, t_, float(np.log(EPS_W) + 0.5 * LN3), "max")
    wmag16 = H()
    act(wmag16, t_, "Exp", scale=-1.0, bias=_c(nc, 0.5 * LN3))
    regs.free(t_, nk0, nk1, lnt, ssig, s2i, Pg32, *lam)

    # ---- N = ssig*S + Pg*I (f16, batched) ----
    fill(3)
    nc.vector.tensor_tensor(out=Nm[:, :, 0:L], in0=Sm[:, :, 0:L],
                            in1=bc(ssig16, 6), op=Alu.mult)
    nc.vector.tensor_tensor(out=Nm[:, 0:3, 0:L], in0=Nm[:, 0:3, 0:L],
                            in1=bc(Pg16, 3), op=Alu.add)
    regs16.free(Pg16, ssig16)
    # A' diagonal (Am = S_diag + s2)
    nc.vector.tensor_tensor(out=Am[:, :, 0:L], in0=Sm[:, 0:3, 0:L],
                            in1=bc(s2i16, 3), op=Alu.add)
    regs16.free(s2i16)
    # w16 = sign(det) * wmag  (into the double-buffered W16 plane)
    mul(W16[:, 0:L], wmag16, sgd16)
    regs16.free(wmag16, sgd16)
    # ---- adj(N) (f16) ----
    n00, n11, n22 = (Nm[:, i, 0:L] for i in range(3))
    n01, n02, n12 = (Nm[:, i, 0:L] for i in range(3, 6))
    aj = [H() for _ in range(6)]
    a00, a01, a02, a11, a12, a22 = aj

    def cof(o, a, b, c, dd):
        mul(o, a, b); mul(tmp16, c, dd); sub(o, o, tmp16)

    cof(a00, n11, n22, n12, n12)
    cof(a01, n02, n12, n01, n22)
    cof(a02, n01, n12, n02, n11)
    cof(a11, n00, n22, n02, n02)
    cof(a12, n01, n02, n00, n12)
    cof(a22, n00, n11, n01, n01)
    # ---- T1 = A' adjN (f16); rows of A': (b0,s01,s02),(s01,b1,s12),(s02,s12,b2)
    b0, b1, b2 = (Am[:, i, 0:L] for i in range(3))
    s01p, s02p, s12p = Sm[:, 3, 0:L], Sm[:, 4, 0:L], Sm[:, 5, 0:L]

    def mm3(o, r0, r1, r2, k0, k1, k2):
        mul(o, r0, k0)
        mul(tmp16, r1, k1); add(o, o, tmp16)
        mul(tmp16, r2, k2); add(o, o, tmp16)

    mm3(T1m[:, 0, 0:L], b0, s01p, s02p, a00, a01, a02)
    mm3(T1m[:, 1, 0:L], b0, s01p, s02p, a01, a11, a12)
    mm3(T1m[:, 2, 0:L], b0, s01p, s02p, a02, a12, a22)
    mm3(T1m[:, 3, 0:L], s01p, b1, s12p, a01, a11, a12)
    mm3(T1m[:, 4, 0:L], s01p, b1, s12p, a02, a12, a22)
    mm3(T1m[:, 5, 0:L], s02p, s12p, b2, a02, a12, a22)
    regs16.free(*aj)
    regs16.free(tmp16)


def _make_B(nc, env, k, T, tiles, acc_rot_col, last=False):
    """Phase B of chunk k: T2 = w*T1, R = M*T2 as a list of single-op DVE
    thunks (drained as filler during chunk k+1's phase A).  Three of the
    nine R outputs run on GpSimd (emitted with the T2 thunk).  The tail
    (clamp + dR sub) also runs on GpSimd, and the ACT accumulate thunk is
    emitted early in chunk k+2.  For the last chunk everything stays on
    DVE (it is idle during the pipeline drain)."""
    mybir = env["mybir"]
    Alu = mybir.AluOpType
    Act = mybir.ActivationFunctionType
    L = 2 * T
    X, T1m, Nm, W16, RT, BT, PT = (tiles[n] for n in
                                   ("X", "T1m", "Nm", "W16", "RT", "BT",
                                    "PT"))
    x = [X[:, c, 0:L] for c in range(9)]
    tail_eng = nc.vector if last else nc.gpsimd

    def mul(o, a, b):
        nc.vector.tensor_tensor(out=o, in0=a, in1=b, op=Alu.mult)

    def add(o, a, b):
        nc.vector.tensor_tensor(out=o, in0=a, in1=b, op=Alu.add)

    def bc(plane, n):
        return bass_mod.AP(tensor=plane.tensor, offset=plane.offset,
                           ap=[plane.ap[0], [0, n], plane.ap[1]])

    T2f = Nm
    t00, t01, t02 = T2f[:, 0, 0:L], T2f[:, 1, 0:L], T2f[:, 2, 0:L]
    t11, t12, t22 = T2f[:, 3, 0:L], T2f[:, 4, 0:L], T2f[:, 5, 0:L]
    T2 = [[t00, t01, t02], [t01, t11, t12], [t02, t12, t22]]
    tmpr = BT[:, 0:L]
    tmpp = PT[:, 0:L]

    def emit_out(i, j, eng, tmp):
        o = RT[:, 3 * i + j, 0:L]
        eng.tensor_tensor(out=o, in0=x[3 * i], in1=T2[0][j], op=Alu.mult)
        eng.tensor_tensor(out=tmp, in0=x[3 * i + 1], in1=T2[1][j],
                          op=Alu.mult)
        eng.tensor_tensor(out=o, in0=o, in1=tmp, op=Alu.add)
        eng.tensor_tensor(out=tmp, in0=x[3 * i + 2], in1=T2[2][j],
                          op=Alu.mult)
        eng.tensor_tensor(out=o, in0=o, in1=tmp, op=Alu.add)

    def t2_and_pool_share():
        # T2 = w*T1 in f16 (Nm tile is dead after adj; reuse it)
        nc.vector.tensor_tensor(out=T2f[:, :, 0:L], in0=T1m[:, :, 0:L],
                                in1=bc(W16[:, 0:L], 6), op=Alu.mult)
        if not last:
            for i in range(3):
                emit_out(i, 2, nc.gpsimd, tmpp)

    dve = [t2_and_pool_share]
    cols = (0, 1) if not last else (0, 1, 2)
    for i in range(3):
        for j in cols:
            o = RT[:, 3 * i + j, 0:L]
            dve.append(lambda o=o, i=i, j=j: mul(o, x[3 * i], T2[0][j]))
            dve.append(lambda i=i, j=j: mul(tmpr, x[3 * i + 1], T2[1][j]))
            dve.append(lambda o=o: add(o, o, tmpr))
            dve.append(lambda i=i, j=j: mul(tmpr, x[3 * i + 2], T2[2][j]))
            dve.append(lambda o=o: add(o, o, tmpr))

    def pool_tail():
        # clamp junk rows, dR = R_pred - R_target in place (off DVE)
        tail_eng.tensor_scalar(out=RT[:, :, 0:L], in0=RT[:, :, 0:L],
                               scalar1=8.0, scalar2=-8.0,
                               op0=Alu.min, op1=Alu.max)
        tail_eng.tensor_tensor(out=RT[:, :, 0:T], in0=RT[:, :, 0:T],
                               in1=RT[:, :, T:L], op=Alu.subtract)

    def act_accum():
        bias0 = env["bias0"]
        nc.scalar.activation(out=RT[:, :, 0:T], in_=RT[:, :, 0:T],
                             func=Act.Square, bias=bias0[:, 0:1], scale=1.0,
                             accum_out=acc_rot_col)

    return dve, pool_tail, act_accum


def _build_nc():
    global bass_mod
    import concourse.bass as bass
    import concourse.tile as tile
    from concourse import mybir
    bass_mod = bass

    f32 = mybir.dt.float32
    f16 = mybir.dt.float16
    nc = bass.Bass()
    pred = nc.dram_tensor("pred", [ROWS_PER_CORE, 9], f32, kind="ExternalInput")
    targ = nc.dram_tensor("target", [ROWS_PER_CORE, 9], f32, kind="ExternalInput")
    out = nc.dram_tensor("partials", [P, 2 * NCHUNK], f32, kind="ExternalOutput")

    predv = pred.rearrange("(p n) c -> p n c", p=P)    # [128, 1024, 9]
    targv = targ.rearrange("(p n) c -> p n c", p=P)
    row0 = np.cumsum((0,) + CHUNKS)                    # chunk row offsets

    with tile.TileContext(nc) as tc:
        with (
            tc.tile_pool(name="raw", bufs=1) as rawp,
            tc.tile_pool(name="pl", bufs=1) as pl,
            tc.tile_pool(name="acc", bufs=1) as accp,
        ):
            acc = accp.tile([P, 2 * NCHUNK], f32, tag="acc")
            bias0 = accp.tile([P, 1], f32, tag="bias0")
            nc.vector.memset(bias0, 0.0)
            _CONST_STATE[(id(nc), "pool")] = accp
            env = {
                "mybir": mybir,
                "regs": Regs(pl, f32, prefix="v"),
                "regs16": Regs(pl, f16, prefix="h"),
                "bias0": bias0,
            }

            raw_cache = {}

            def raw_tiles(k):
                if k not in raw_cache:
                    raw_cache[k] = (
                        rawp.tile([P, TMAX * 9], f32, tag=f"praw{k % 2}",
                                  name=f"praw{k % 2}"),
                        rawp.tile([P, TMAX * 9], f32, tag=f"traw{k % 2}",
                                  name=f"traw{k % 2}"))
                return raw_cache[k]

            def chunk_tiles(k):
                j = k % 2
                return {
                    "X": pl.tile([P, 9, 2 * TMAX], f16, tag=f"X{j}", name=f"X{j}"),
                    "D": pl.tile([P, 9, TMAX], f16, tag="D", name="D"),
                    "Sm": pl.tile([P, 6, 2 * TMAX], f16, tag=f"Sm{j}", name=f"Sm{j}"),
                    "QS": pl.tile([P, 6, 2 * TMAX], f32, tag="QS", name="QS"),
                    "Am": pl.tile([P, 3, 2 * TMAX], f16, tag="Am", name="Am"),
                    "T1m": pl.tile([P, 6, 2 * TMAX], f16, tag=f"T1m{j}", name=f"T1m{j}"),
                    "Nm": pl.tile([P, 6, 2 * TMAX], f16, tag=f"Nm{j}", name=f"Nm{j}"),
                    "W16": pl.tile([P, 2 * TMAX], f16, tag=f"W16{j}", name=f"W16{j}"),
                    "RT": pl.tile([P, 9, 2 * TMAX], f16, tag=f"RT{j}", name=f"RT{j}"),
                    "BT": pl.tile([P, 2 * TMAX], f16, tag=f"BT{j}", name=f"BT{j}"),
                    "PT": pl.tile([P, 2 * TMAX], f16, tag=f"PT{j}", name=f"PT{j}"),
                    "SQ": pl.tile([P, 9, 2 * TMAX], f16, tag="SQ", name="SQ"),
                }

            def dma_chunk(k):
                prw, trw = raw_tiles(k)
                t0, t1 = row0[k], row0[k + 1]
                n = (t1 - t0) * 9
                nc.sync.dma_start(out=prw[:, 0:n], in_=predv[:, t0:t1, :])
                nc.sync.dma_start(out=trw[:, 0:n], in_=targv[:, t0:t1, :])

            dma_chunk(0)
            dma_chunk(1)

            pending = []        # phase-B DVE thunks of chunk k-1
            tail_q = []         # (pool_tail, act_accum) of chunk k-1
            accum_slot = []     # ACT accumulates to emit at next post_cast

            def filler(n):
                for _ in range(min(n, len(pending))):
                    pending.pop(0)()

            for k, T in enumerate(CHUNKS):
                praw, traw = raw_tiles(k)
                tiles = chunk_tiles(k)

                def pre_cast():
                    while accum_slot:
                        accum_slot.pop(0)()

                def post_cast(k=k):
                    if k + 2 < NCHUNK:
                        dma_chunk(k + 2)

                _emit_A(nc, env, k, T, praw, traw, tiles,
                        acc[:, k:k + 1], filler, pre_cast, post_cast)
                # drain the rest of chunk k-1's phase B, then its GpSimd tail;
                # its ACT accumulate goes early into chunk k+1
                filler(len(pending))
                if tail_q:
                    pool_tail, act_accum = tail_q.pop(0)
                    pool_tail()
                    accum_slot.append(act_accum)
                dve, pool_tail, act_accum = _make_B(
                    nc, env, k, T, tiles, acc[:, NCHUNK + k:NCHUNK + k + 1],
                    last=(k == NCHUNK - 1))
                pending = dve
                tail_q.append((pool_tail, act_accum))

            # drain the pipeline: B of the last chunk, tails, accumulates
            filler(len(pending))
            while accum_slot:
                accum_slot.pop(0)()
            while tail_q:
                pool_tail, act_accum = tail_q.pop(0)
                pool_tail()
                act_accum()
            nc.sync.dma_start(out=out[:, :], in_=acc)
    return nc


def _elide_implied_waits(nc):
    """Drop semaphore waits already implied by program order or transitively
    by earlier waits (vector-clock propagation).  Tile's per-instruction wait
    emission is not transitively minimal, and walrus can encode only one sync
    wait on Activation/DMA instructions (and ~4 on control instructions), so
    the redundant waits both break codegen and waste sequencer time.

    Model: each semaphore s carries a snapshot VC at every increment value;
    an engine's observed VC advances via its own instruction stream and via
    the snapshots of the waits it executes.  A wait (s >= v) is dropped iff
    the engine's observed VC already dominates it.  Unknown update modes
    disable elision for that semaphore (conservative).
    """
    join = lambda a, b: {k: max(a.get(k, 0), b.get(k, 0)) for k in set(a) | set(b)}
    sem_val = {}        # sem name -> current value
    sem_snap = {}       # sem name -> list of (value, VC) snapshots
    eng_vc = {}         # engine name -> observed VC
    unsafe = set()      # sems with non-increment updates
    n_drop = 0
    for f in nc.m.functions:
        for bb in f.blocks:
            for ins in bb.instructions:
                eng = str(ins.engine)
                vc = dict(eng_vc.get(eng, {}))
                si = ins.sync_info
                waits = list(si.on_wait) if si is not None and si.on_wait else []
                kept = []
                for w in waits:
                    s, v = w.ant_name, w.wait_value
                    if w.wait_mode != "sem-ge-imm" or s in unsafe:
                        kept.append(w)
                        continue
                    if vc.get(s, 0) >= v:
                        n_drop += 1
                        continue
                    if sem_val.get(s, 0) < v:
                        # increment not yet seen in emission order; keep and
                        # learn nothing (conservative)
                        kept.append(w)
                        continue
                    kept.append(w)
                    snap = {}
                    for sv, svc in sem_snap.get(s, ()):
                        if sv <= v:
                            snap = svc
                        else:
                            break
                    vc = join(vc, snap)
                    vc[s] = max(vc.get(s, 0), v)
                if si is not None and len(kept) != len(waits):
                    si.on_wait = kept
                # apply this instruction's increments
                ups = si.on_update if si is not None and si.on_update else []
                for u in ups:
                    s = u.ant_name
                    if u.update_mode not in ("sem-inc", "sem-add-imm"):
                        unsafe.add(s)
                        continue
                    nv = sem_val.get(s, 0) + (u.update_value or 1)
                    sem_val[s] = nv
                    lst = sem_snap.setdefault(s, [])
                    prev = lst[-1][1] if lst else {}
                    lst.append((nv, join(prev, vc)))
                    # Engine-sem increments fire when the instruction
                    # completes, and the engine is sequential, so later
                    # instructions on this engine observe them.  DMA-queue
                    # increments fire asynchronously at transfer completion:
                    # the issuing engine must NOT absorb those.
                    if "DMA" not in s:
                        vc[s] = max(vc.get(s, 0), nv)
                eng_vc[eng] = vc
    return n_drop


def _spill_excess_waits(nc):
    """walrus encodes at most ONE sync wait on real engine instructions
    (Activation/DVE/DMA); the Tile scheduler can leave more after
    cross-engine reordering.  Keep one wait on the instruction and hoist
    the rest onto preceding InstEventSemaphore control instructions
    (which accept 2 waits each)."""
    from concourse import mybir
    n_spill = 0
    for f in nc.m.functions:
        for bb in f.blocks:
            out = []
            for ins in bb.instructions:
                si = ins.sync_info
                waits = list(si.on_wait) if si is not None and si.on_wait else []
                is_engine_op = bool(ins.ins) or bool(ins.outs)
                if len(waits) > 1 and is_engine_op and \
                        not isinstance(ins, mybir.InstEventSemaphore):
                    spill = waits[:-1]
                    si.on_wait = waits[-1:]
                    while spill:
                        grp, spill = spill[:2], spill[2:]
                        n_spill += 1
                        ev = mybir.InstEventSemaphore(
                            name=f"wspill_{n_spill}", engine=ins.engine,
                            ins=[], outs=[],
                            sync_info=mybir.SyncInfo(on_wait=grp,
                                                     on_update=[]))
                        out.append(ev)
                out.append(ins)
            bb.instructions = out
    return n_spill


_NC_CACHE = None


def kernel(pred: np.ndarray, target: np.ndarray) -> np.ndarray:
    global _NC_CACHE
    from concourse.bass_utils import run_bass_kernel_spmd

    pred = np.ascontiguousarray(np.asarray(pred, dtype=np.float32))
    target = np.ascontiguousarray(np.asarray(target, dtype=np.float32))
    assert pred.shape == (B, 9) and target.shape == (B, 9)

    if _NC_CACHE is None:
        _NC_CACHE = _build_nc()
        _elide_implied_waits(_NC_CACHE)
        _spill_excess_waits(_NC_CACHE)
    nc = _NC_CACHE

    ps = pred.reshape(N_CORES, ROWS_PER_CORE, 9)
    ts = target.reshape(N_CORES, ROWS_PER_CORE, 9)
    in_maps = [{"pred": ps[i], "target": ts[i]} for i in range(N_CORES)]
    res = run_bass_kernel_spmd(nc, in_maps, core_ids=list(range(N_CORES)))
    globals()["_LAST_RESULT"] = res

    mse_sum = 0.0
    rot_sum = 0.0
    for r in res.results:
        part = np.asarray(r["partials"], dtype=np.float64)
        mse_sum += part[:, :NCHUNK].sum()
        rot_sum += part[:, NCHUNK:].sum()
    n = float(B * 9)
    return np.asarray(np.float32(mse_sum / n + 0.5 * (rot_sum / n)))


# revision 23
# speedup vs baseline: 1.2337x; 1.0912x over previous
"""CustomPoseLoss Trainium2 kernel.

loss = mean((pred-target)^2) + 0.5 * mean((R(pred)-R(target))^2)
where R(M) = sign(det M) * polar(M) for each 3x3 matrix (row of 9).

Implementation: closed-form polar decomposition per row, fully vectorized as
channel-plane arithmetic:
  S = M^T M, normalized by tr(S)/3; eigenvalues of S via Cardano
  (acos/cos evaluated as polynomials so only the ln/exp LUT set is needed);
  W^-1 = (S + s2 I) adj(N) / det(N) with N = ssig*S + Pg*I  (Cayley-Hamilton
  inverse-sqrt);  R = sign(det) * M W^-1 / sqrt(m).
det(N) is formed from the eigenvalue product (positive, cancellation-free)
and clamped, so near-singular rows stay bounded.

Engine plan: heavy products in f16 on DVE (2x mode); scalar chain fp32 on
DVE+ACT; mse-sub / clamp / dR-sub on GpSimd (off critical path).  Chunks are
software-pipelined: chunk k's tail (T2, R = M*T2) is emitted as filler inside
chunk k+1's scalar-chain stalls, so the Vector engine never waits on ACT legs.

Sharding: pure data parallel over 8 cores; each core reduces its shard to
[128, 2*NCHUNK] partial sums (mse, rot), host combines in float64.
"""

import numpy as np

B = 1048576
N_CORES = 8
ROWS_PER_CORE = B // N_CORES          # 131072
P = 128
ROWS_PER_PART = ROWS_PER_CORE // P    # 1024
CHUNKS = (256, 256, 256, 256)         # rows per partition per chunk
NCHUNK = len(CHUNKS)
TMAX = max(CHUNKS)
EPS_D = 1e-5

ACOS_A = (1.5707288, -0.2121144, 0.0742610, -0.0187293)   # A&S 4.4.45
HALF_SQRT3 = 0.8660254037844386

LN3 = float(np.log(3.0))
LN6 = float(np.log(6.0))
LN2 = float(np.log(2.0))
EPS_W = 6e-3


class Regs:
    """[128, 2*TMAX] plane slots with explicit reuse (SBUF is capped)."""

    def __init__(self, pool, dtype, prefix="v", shape=None):
        self.pool = pool
        self.dtype = dtype
        self.prefix = prefix
        self.shape = shape or [P, 2 * TMAX]
        self.free_tags = []
        self.n = 0
        self.tag_of = {}

    def alloc(self, width=None):
        if self.free_tags:
            tag = self.free_tags.pop()
        else:
            self.n += 1
            tag = f"{self.prefix}{self.n}"
        tl = self.pool.tile(self.shape, self.dtype, tag=tag, name=tag)
        if width is not None:
            tl = tl[:, 0:width]
        self.tag_of[id(tl)] = tag
        return tl

    def free(self, *tiles):
        for tl in tiles:
            self.free_tags.append(self.tag_of.pop(id(tl)))


_CONST_STATE = {}
bass_mod = None


def _c(nc, v):
    """[P,1] fp32 constant AP, DVE-memset once (keeps ACT single-wait)."""
    key = float(np.float32(v))
    consts = _CONST_STATE.setdefault(id(nc), {})
    if key not in consts:
        pool = _CONST_STATE[(id(nc), "pool")]
        from concourse import mybir
        t = pool.tile([P, 1], mybir.dt.float32, tag=f"c{len(consts)}")
        nc.vector.memset(t, key)
        consts[key] = t
    return consts[key][:, 0:1]


def _emit_A(nc, env, k, T, praw, traw, tiles, acc_mse_col, fill, pre_cast,
            post_cast):
    """Phase A of chunk k: everything through T1m and w16.

    `fill(n)` emits up to n pending phase-B DVE ops from the previous chunk
    at known ACT-wait stall points.  `pre_cast()` emits the chunk k-2 ACT
    accumulate (whose Pool wait transitively covers the casts' X-tile WAR
    wait — walrus allows only one sync wait per Activation instruction);
    `post_cast()` emits the chunk k+2 DMA prefetch.
    """
    mybir = env["mybir"]
    regs, regs16, bias0 = env["regs"], env["regs16"], env["bias0"]
    Alu = mybir.AluOpType
    Act = mybir.ActivationFunctionType
    L = 2 * T
    X, D, Sm, QS, Am, T1m, Nm, W16 = (tiles[n] for n in
                                      ("X", "D", "Sm", "QS", "Am", "T1m",
                                       "Nm", "W16"))

    def mul(o, a, b):
        nc.vector.tensor_tensor(out=o, in0=a, in1=b, op=Alu.mult)

    def add(o, a, b):
        nc.vector.tensor_tensor(out=o, in0=a, in1=b, op=Alu.add)

    def sub(o, a, b):
        nc.vector.tensor_tensor(out=o, in0=a, in1=b, op=Alu.subtract)

    def vs(o, a, s1, op0, s2=None, op1=None):
        if s2 is None:
            nc.vector.tensor_scalar(out=o, in0=a, scalar1=float(s1),
                                    scalar2=None, op0=getattr(Alu, op0))
        else:
            nc.vector.tensor_scalar(out=o, in0=a, scalar1=float(s1),
                                    scalar2=float(s2), op0=getattr(Alu, op0),
                                    op1=getattr(Alu, op1))

    def stt(o, a, s, b, op0, op1):
        nc.vector.scalar_tensor_tensor(out=o, in0=a, scalar=float(s), in1=b,
                                       op0=getattr(Alu, op0),
                                       op1=getattr(Alu, op1))

    def act(o, a, func, scale=1.0, bias=None, accum_out=None):
        if func == "Copy":
            nc.scalar.activation(out=o, in_=a, func=Act.Copy, bias=0.0,
                                 scale=float(scale), accum_out=accum_out)
        else:
            nc.scalar.activation(out=o, in_=a, func=getattr(Act, func),
                                 bias=bias0[:, 0:1] if bias is None else bias,
                                 scale=float(scale), accum_out=accum_out)

    def bc(plane, n):
        # broadcast [P, L] plane across n sub-planes -> [P, n, L]
        return bass_mod.AP(tensor=plane.tensor, offset=plane.offset,
                           ap=[plane.ap[0], [0, n], plane.ap[1]])

    A = lambda: regs.alloc(L)       # fp32 [P, L] planes
    H = lambda: regs16.alloc(L)     # f16 [P, L] planes

    # ---- cast+deinterleave both inputs into X[P, 9, 2T] (f16) ----
    rvp = praw[:, 0:T * 9].rearrange("p (n c) -> p n c", c=9)
    rvt = traw[:, 0:T * 9].rearrange("p (n c) -> p n c", c=9)
    xin_p = bass_mod.AP(tensor=rvp.tensor, offset=rvp.offset,
                        ap=[rvp.ap[0], rvp.ap[2], rvp.ap[1]])
    xin_t = bass_mod.AP(tensor=rvt.tensor, offset=rvt.offset,
                        ap=[rvt.ap[0], rvt.ap[2], rvt.ap[1]])
    act(X[:, :, 0:T], xin_p, "Copy")
    act(X[:, :, T:L], xin_t, "Copy")
    post_cast()
    x = [X[:, c, 0:L] for c in range(9)]        # [P, L] f16 unit-stride

    # ---- channel squares for the S diagonal (ACT; overlaps DVE det/S) ----
    act(tiles["SQ"][:, :, 0:L], X[:, :, 0:L], "Square")

    # ---- mse sub on GpSimd (off critical path) ----
    Dv = D[:, :, 0:T]
    nc.gpsimd.tensor_tensor(out=Dv, in0=X[:, :, 0:T], in1=X[:, :, T:L],
                            op=Alu.subtract)

    fill(6)

    # ---- det(M) f16 from X planes (emitted first: its ACT consumers
    #      run during the S block) ----
    cA16, cB16, det16 = H(), H(), H()
    mul(cA16, x[4], x[8]); mul(cB16, x[5], x[7]); sub(cA16, cA16, cB16)
    mul(det16, x[0], cA16)
    mul(cA16, x[5], x[6]); mul(cB16, x[3], x[8]); sub(cA16, cA16, cB16)
    mul(cA16, x[1], cA16); add(det16, det16, cA16)
    mul(cA16, x[3], x[7]); mul(cB16, x[4], x[6]); sub(cA16, cA16, cB16)
    mul(cA16, x[2], cA16); add(det16, det16, cA16)
    sgd16 = H(); act(sgd16, det16, "Sign")
    lnad, ad32 = A(), A()
    act(ad32, det16, "Abs")
    act(lnad, ad32, "Ln")
    regs.free(ad32)
    regs16.free(cA16, cB16, det16)
    # mse Square-accum here: its DVE wait is covered by the det ACT ops
    # above (walrus allows only one sync wait per Activation instruction)
    act(Dv, Dv, "Square", accum_out=acc_mse_col)
    # previous chunk's rot accumulate lands in the same ACT lull
    pre_cast()

    # ---- S = M^T M (f16): order [s00,s11,s22,s01,s02,s12] ----
    # diag from the ACT-computed channel squares (SQ = X*X): 2 DVE adds
    tmp16 = H()
    SQ = tiles["SQ"]
    add(Sm[:, 0:3, 0:L], SQ[:, 0:3, 0:L], SQ[:, 3:6, 0:L])
    add(Sm[:, 0:3, 0:L], Sm[:, 0:3, 0:L], SQ[:, 6:9, 0:L])
    for oi, (ia, ib) in enumerate((((0, 3, 6), (1, 4, 7)),
                                   ((0, 3, 6), (2, 5, 8)),
                                   ((1, 4, 7), (2, 5, 8)))):
        so = Sm[:, 3 + oi, 0:L]
        mul(so, x[ia[0]], x[ib[0]])
        mul(tmp16, x[ia[1]], x[ib[1]]); add(so, so, tmp16)
        mul(tmp16, x[ia[2]], x[ib[2]]); add(so, so, tmp16)
    # tr and normalization scale q = 3/tr (ln domain)
    tr16 = H()
    add(tr16, Sm[:, 0, 0:L], Sm[:, 1, 0:L]); add(tr16, tr16, Sm[:, 2, 0:L])
    vs(tr16, tr16, 6e-5, "max")
    lnt = A(); act(lnt, tr16, "Ln")
    q16 = H(); act(q16, lnt, "Exp", scale=-1.0, bias=_c(nc, LN3))
    regs16.free(tr16)
    fill(5)
    nc.vector.tensor_tensor(out=Sm[:, :, 0:L], in0=Sm[:, :, 0:L],
                            in1=bc(q16, 6), op=Alu.mult)
    regs16.free(q16)

    # Pg = exp(lnad + 1.5*(ln3 - lnt));  dets = Pg^2
    lnPg = A()
    stt(lnPg, lnt, -1.5, lnad, "mult", "add")
    regs.free(lnad)
    Pg32 = A(); act(Pg32, lnPg, "Exp", scale=1.0, bias=_c(nc, 1.5 * LN3))
    Pg16 = H(); act(Pg16, Pg32, "Copy")
    dets = A(); act(dets, Pg32, "Square")
    regs.free(lnPg)

    # ---- tr(S^2) fp32 from normalized f16 S ----
    act(QS[:, :, 0:L], Sm[:, :, 0:L], "Square")
    fill(3)
    u1, u2 = A(), A()
    add(u1, QS[:, 0, 0:L], QS[:, 1, 0:L]); add(u1, u1, QS[:, 2, 0:L])
    add(u2, QS[:, 3, 0:L], QS[:, 4, 0:L]); add(u2, u2, QS[:, 5, 0:L])
    trS2 = A()
    stt(trS2, u2, 2.0, u1, "mult", "add")
    # p and 1/(2 p^3) via ln/exp
    trK2, p, ip3h = u1, A(), u2            # reuse u1/u2 slots
    vs(trK2, trS2, -3.0, "add", 1e-30, "max")
    lnk = A(); act(lnk, trK2, "Ln")
    act(p, lnk, "Exp", scale=0.5, bias=_c(nc, -0.5 * LN6))
    act(ip3h, lnk, "Exp", scale=-1.5, bias=_c(nc, 1.5 * LN6 - LN2))
    regs.free(lnk)
    # arg
    detK, arg = A(), A()
    stt(detK, trS2, 0.5, dets, "mult", "add")
    vs(detK, detK, -2.5, "add")
    fill(3)
    vs(ip3h, ip3h, 1e30, "min")
    mul(arg, detK, ip3h)
    vs(arg, arg, 1.0, "min", -1.0, "max")
    regs.free(detK, trS2, dets, u2)   # u2 == ip3h
    # ---- th3 = acos(arg) ----
    y, om, h = A(), A(), A()
    act(y, arg, "Abs")
    sg_a = A(); act(sg_a, arg, "Sign")
    fill(2)
    vs(om, y, -1.0, "mult", 1.0, "add")
    lnom = A(); act(lnom, om, "Ln")
    act(om, lnom, "Exp", scale=0.5)              # sqrt(1-y)
    regs.free(lnom)
    vs(h, y, ACOS_A[3], "mult", ACOS_A[2], "add")
    mul(h, h, y); vs(h, h, ACOS_A[1], "add")
    mul(h, h, y); vs(h, h, ACOS_A[0], "add")
    fill(3)
    mul(h, h, om)
    th3 = om                                     # reuse slot
    vs(arg, sg_a, -np.pi / 2, "mult", np.pi / 2, "add")
    mul(th3, sg_a, h); add(th3, th3, arg)
    regs.free(h, arg, y, sg_a)    # th3 == om stays
    # ---- cos((th3+2pik)/3) ----
    z, c0p = A(), A()
    act(z, th3, "Square", scale=1.0 / 3.0)
    fill(2)
    vs(c0p, z, 1.0 / 40320.0, "mult", -1.0 / 720.0, "add")
    mul(c0p, c0p, z); vs(c0p, c0p, 1.0 / 24.0, "add")
    mul(c0p, c0p, z); vs(c0p, c0p, -0.5, "add")
    mul(c0p, c0p, z); vs(c0p, c0p, 1.0, "add")
    regs.free(z, om)   # om == th3
    s0, uc1, c1p, c2p = A(), A(), A(), A()
    act(s0, c0p, "Square")
    vs(uc1, c0p, -0.5, "mult")
    fill(2)
    vs(s0, s0, -1.0, "mult", 1.0, "add")
    vs(s0, s0, 0.0, "max")
    lns = A(); act(lns, s0, "Ln")
    act(s0, lns, "Exp", scale=0.5)
    regs.free(lns)
    fill(3)
    vs(s0, s0, HALF_SQRT3, "mult")
    sub(c1p, uc1, s0)
    add(c2p, uc1, s0)
    regs.free(s0, uc1)
    # ---- lambda_k, g_k = sqrt(lambda_k) ----
    tp = A()
    vs(tp, p, 2.0, "mult")
    regs.free(p)
    lam, g, lnls = [], [], []
    for ck in (c0p, c1p, c2p):
        lk, lnl = A(), A()
        mul(lk, tp, ck)
        vs(lk, lk, 1.0, "add", 1e-35, "max")
        act(lnl, lk, "Ln")
        lam.append(lk); lnls.append(lnl)
    for lnl in lnls:
        gk = A()
        act(gk, lnl, "Exp", scale=0.5)
        g.append(gk)
        regs.free(lnl)
    regs.free(tp, c0p, c1p, c2p)
    fill(5)
    g01, ssig, s2i, tmp2 = A(), A(), A(), A()
    add(g01, g[0], g[1])
    add(ssig, g01, g[2])
    mul(s2i, g[0], g[1]); mul(tmp2, g[2], g01); add(s2i, s2i, tmp2)
    regs.free(g01, tmp2, *g)
    # ---- w = sign/(sqrt(m)*detN) via ln domain, clamped ----
    # detN = prod_k (ssig*lam_k + Pg): form the product, one Ln.
    t_, nk0, nk1 = A(), A(), A()
    mul(nk0, ssig, lam[0]); add(nk0, nk0, Pg32)
    mul(nk1, ssig, lam[1]); add(nk1, nk1, Pg32)
    mul(nk0, nk0, nk1)
    mul(nk1, ssig, lam[2]); add(nk1, nk1, Pg32)
    mul(nk0, nk0, nk1)
    act(t_, nk0, "Ln")
    ssig16, s2i16 = H(), H()
    act(ssig16, ssig, "Copy")
    act(s2i16, s2i, "Copy")
    fill(3)
    stt(t_, lnt, 0.5, t_, "mult", "add")
    vs(t_, t_, float(np.log(EPS_W) + 0.5 * LN3), "max")
    wmag16 = H()
    act(wmag16, t_, "Exp", scale=-1.0, bias=_c(nc, 0.5 * LN3))
    regs.free(t_, nk0, nk1, lnt, ssig, s2i, Pg32, *lam)

    # ---- N = ssig*S + Pg*I (f16, batched) ----
    fill(3)
    nc.vector.tensor_tensor(out=Nm[:, :, 0:L], in0=Sm[:, :, 0:L],
                            in1=bc(ssig16, 6), op=Alu.mult)
    nc.vector.tensor_tensor(out=Nm[:, 0:3, 0:L], in0=Nm[:, 0:3, 0:L],
                            in1=bc(Pg16, 3), op=Alu.add)
    regs16.free(Pg16, ssig16)
    # A' diagonal (Am = S_diag + s2)
    nc.vector.tensor_tensor(out=Am[:, :, 0:L], in0=Sm[:, 0:3, 0:L],
                            in1=bc(s2i16, 3), op=Alu.add)
    regs16.free(s2i16)
    # w16 = sign(det) * wmag  (into the double-buffered W16 plane)
    mul(W16[:, 0:L], wmag16, sgd16)
    regs16.free(wmag16, sgd16)
    # ---- adj(N) (f16) ----
    n00, n11, n22 = (Nm[:, i, 0:L] for i in range(3))
    n01, n02, n12 = (Nm[:, i, 0:L] for i in range(3, 6))
    aj = [H() for _ in range(6)]
    a00, a01, a02, a11, a12, a22 = aj

    def cof(o, a, b, c, dd):
        mul(o, a, b); mul(tmp16, c, dd); sub(o, o, tmp16)

    cof(a00, n11, n22, n12, n12)
    cof(a01, n02, n12, n01, n22)
    cof(a02, n01, n12, n02, n11)
    cof(a11, n00, n22, n02, n02)
    cof(a12, n01, n02, n00, n12)
    cof(a22, n00, n11, n01, n01)
    # ---- T1 = A' adjN (f16); rows of A': (b0,s01,s02),(s01,b1,s12),(s02,s12,b2)
    b0, b1, b2 = (Am[:, i, 0:L] for i in range(3))
    s01p, s02p, s12p = Sm[:, 3, 0:L], Sm[:, 4, 0:L], Sm[:, 5, 0:L]

    def mm3(o, r0, r1, r2, k0, k1, k2):
        mul(o, r0, k0)
        mul(tmp16, r1, k1); add(o, o, tmp16)
        mul(tmp16, r2, k2); add(o, o, tmp16)

    mm3(T1m[:, 0, 0:L], b0, s01p, s02p, a00, a01, a02)
    mm3(T1m[:, 1, 0:L], b0, s01p, s02p, a01, a11, a12)
    mm3(T1m[:, 2, 0:L], b0, s01p, s02p, a02, a12, a22)
    mm3(T1m[:, 3, 0:L], s01p, b1, s12p, a01, a11, a12)
    mm3(T1m[:, 4, 0:L], s01p, b1, s12p, a02, a12, a22)
    mm3(T1m[:, 5, 0:L], s02p, s12p, b2, a02, a12, a22)
    regs16.free(*aj)
    regs16.free(tmp16)


def _make_B(nc, env, k, T, tiles, acc_rot_col, last=False):
    """Phase B of chunk k: T2 = w*T1, R = M*T2 as a list of single-op DVE
    thunks (drained as filler during chunk k+1's phase A).  Three of the
    nine R outputs run on GpSimd (emitted with the T2 thunk).  The tail
    (clamp + dR sub) also runs on GpSimd, and the ACT accumulate thunk is
    emitted early in chunk k+2.  For the last chunk everything stays on
    DVE (it is idle during the pipeline drain)."""
    mybir = env["mybir"]
    Alu = mybir.AluOpType
    Act = mybir.ActivationFunctionType
    L = 2 * T
    X, T1m, Nm, W16, RT, BT, PT = (tiles[n] for n in
                                   ("X", "T1m", "Nm", "W16", "RT", "BT",
                                    "PT"))
    x = [X[:, c, 0:L] for c in range(9)]
    tail_eng = nc.vector if last else nc.gpsimd

    def mul(o, a, b):
        nc.vector.tensor_tensor(out=o, in0=a, in1=b, op=Alu.mult)

    def add(o, a, b):
        nc.vector.tensor_tensor(out=o, in0=a, in1=b, op=Alu.add)

    def bc(plane, n):
        return bass_mod.AP(tensor=plane.tensor, offset=plane.offset,
                           ap=[plane.ap[0], [0, n], plane.ap[1]])

    T2f = Nm
    t00, t01, t02 = T2f[:, 0, 0:L], T2f[:, 1, 0:L], T2f[:, 2, 0:L]
    t11, t12, t22 = T2f[:, 3, 0:L], T2f[:, 4, 0:L], T2f[:, 5, 0:L]
    T2 = [[t00, t01, t02], [t01, t11, t12], [t02, t12, t22]]
    tmpr = BT[:, 0:L]
    tmpp = PT[:, 0:L]

    def emit_out(i, j, eng, tmp):
        o = RT[:, 3 * i + j, 0:L]
        eng.tensor_tensor(out=o, in0=x[3 * i], in1=T2[0][j], op=Alu.mult)
        eng.tensor_tensor(out=tmp, in0=x[3 * i + 1], in1=T2[1][j],
                          op=Alu.mult)
        eng.tensor_tensor(out=o, in0=o, in1=tmp, op=Alu.add)
        eng.tensor_tensor(out=tmp, in0=x[3 * i + 2], in1=T2[2][j],
                          op=Alu.mult)
        eng.tensor_tensor(out=o, in0=o, in1=tmp, op=Alu.add)

    def t2_op():
        # T2 = w*T1 in f16 (Nm tile is dead after adj; reuse it)
        nc.vector.tensor_tensor(out=T2f[:, :, 0:L], in0=T1m[:, :, 0:L],
                                in1=bc(W16[:, 0:L], 6), op=Alu.mult)

    dve = [t2_op]
    cols = (0, 1, 2)
    for i in range(3):
        for j in cols:
            o = RT[:, 3 * i + j, 0:L]
            dve.append(lambda o=o, i=i, j=j: mul(o, x[3 * i], T2[0][j]))
            dve.append(lambda i=i, j=j: mul(tmpr, x[3 * i + 1], T2[1][j]))
            dve.append(lambda o=o: add(o, o, tmpr))
            dve.append(lambda i=i, j=j: mul(tmpr, x[3 * i + 2], T2[2][j]))
            dve.append(lambda o=o: add(o, o, tmpr))

    def pool_tail():
        # clamp junk rows, dR = R_pred - R_target in place (off DVE)
        tail_eng.tensor_scalar(out=RT[:, :, 0:L], in0=RT[:, :, 0:L],
                               scalar1=8.0, scalar2=-8.0,
                               op0=Alu.min, op1=Alu.max)
        tail_eng.tensor_tensor(out=RT[:, :, 0:T], in0=RT[:, :, 0:T],
                               in1=RT[:, :, T:L], op=Alu.subtract)

    def act_accum():
        bias0 = env["bias0"]
        nc.scalar.activation(out=RT[:, :, 0:T], in_=RT[:, :, 0:T],
                             func=Act.Square, bias=bias0[:, 0:1], scale=1.0,
                             accum_out=acc_rot_col)

    return dve, pool_tail, act_accum


def _build_nc():
    global bass_mod
    import concourse.bass as bass
    import concourse.tile as tile
    from concourse import mybir
    bass_mod = bass

    f32 = mybir.dt.float32
    f16 = mybir.dt.float16
    nc = bass.Bass()
    pred = nc.dram_tensor("pred", [ROWS_PER_CORE, 9], f32, kind="ExternalInput")
    targ = nc.dram_tensor("target", [ROWS_PER_CORE, 9], f32, kind="ExternalInput")
    out = nc.dram_tensor("partials", [P, 2 * NCHUNK], f32, kind="ExternalOutput")

    predv = pred.rearrange("(p n) c -> p n c", p=P)    # [128, 1024, 9]
    targv = targ.rearrange("(p n) c -> p n c", p=P)
    row0 = np.cumsum((0,) + CHUNKS)                    # chunk row offsets

    with tile.TileContext(nc) as tc:
        with (
            tc.tile_pool(name="raw", bufs=1) as rawp,
            tc.tile_pool(name="pl", bufs=1) as pl,
            tc.tile_pool(name="acc", bufs=1) as accp,
        ):
            acc = accp.tile([P, 2 * NCHUNK], f32, tag="acc")
            bias0 = accp.tile([P, 1], f32, tag="bias0")
            nc.vector.memset(bias0, 0.0)
            _CONST_STATE[(id(nc), "pool")] = accp
            env = {
                "mybir": mybir,
                "regs": Regs(pl, f32, prefix="v"),
                "regs16": Regs(pl, f16, prefix="h"),
                "bias0": bias0,
            }

            raw_cache = {}

            def raw_tiles(k):
                if k not in raw_cache:
                    raw_cache[k] = (
                        rawp.tile([P, TMAX * 9], f32, tag=f"praw{k % 2}",
                                  name=f"praw{k % 2}"),
                        rawp.tile([P, TMAX * 9], f32, tag=f"traw{k % 2}",
                                  name=f"traw{k % 2}"))
                return raw_cache[k]

            def chunk_tiles(k):
                j = k % 2
                return {
                    "X": pl.tile([P, 9, 2 * TMAX], f16, tag=f"X{j}", name=f"X{j}"),
                    "D": pl.tile([P, 9, TMAX], f16, tag="D", name="D"),
                    "Sm": pl.tile([P, 6, 2 * TMAX], f16, tag=f"Sm{j}", name=f"Sm{j}"),
                    "QS": pl.tile([P, 6, 2 * TMAX], f32, tag="QS", name="QS"),
                    "Am": pl.tile([P, 3, 2 * TMAX], f16, tag="Am", name="Am"),
                    "T1m": pl.tile([P, 6, 2 * TMAX], f16, tag=f"T1m{j}", name=f"T1m{j}"),
                    "Nm": pl.tile([P, 6, 2 * TMAX], f16, tag=f"Nm{j}", name=f"Nm{j}"),
                    "W16": pl.tile([P, 2 * TMAX], f16, tag=f"W16{j}", name=f"W16{j}"),
                    "RT": pl.tile([P, 9, 2 * TMAX], f16, tag=f"RT{j}", name=f"RT{j}"),
                    "BT": pl.tile([P, 2 * TMAX], f16, tag=f"BT{j}", name=f"BT{j}"),
                    "PT": pl.tile([P, 2 * TMAX], f16, tag=f"PT{j}", name=f"PT{j}"),
                    "SQ": pl.tile([P, 9, 2 * TMAX], f16, tag="SQ", name="SQ"),
                }

            def dma_chunk(k):
                prw, trw = raw_tiles(k)
                t0, t1 = row0[k], row0[k + 1]
                n = (t1 - t0) * 9
                nc.sync.dma_start(out=prw[:, 0:n], in_=predv[:, t0:t1, :])
                nc.sync.dma_start(out=trw[:, 0:n], in_=targv[:, t0:t1, :])

            dma_chunk(0)
            dma_chunk(1)

            pending = []        # phase-B DVE thunks of chunk k-1
            tail_q = []         # (pool_tail, act_accum) of chunk k-1
            accum_slot = []     # ACT accumulates to emit at next post_cast

            def filler(n):
                for _ in range(min(n, len(pending))):
                    pending.pop(0)()

            for k, T in enumerate(CHUNKS):
                praw, traw = raw_tiles(k)
                tiles = chunk_tiles(k)

                def pre_cast():
                    while accum_slot:
                        accum_slot.pop(0)()

                def post_cast(k=k):
                    if k + 2 < NCHUNK:
                        dma_chunk(k + 2)

                _emit_A(nc, env, k, T, praw, traw, tiles,
                        acc[:, k:k + 1], filler, pre_cast, post_cast)
                # drain the rest of chunk k-1's phase B, then its GpSimd tail;
                # its ACT accumulate goes early into chunk k+1
                filler(len(pending))
                if tail_q:
                    pool_tail, act_accum = tail_q.pop(0)
                    pool_tail()
                    accum_slot.append(act_accum)
                dve, pool_tail, act_accum = _make_B(
                    nc, env, k, T, tiles, acc[:, NCHUNK + k:NCHUNK + k + 1],
                    last=(k == NCHUNK - 1))
                pending = dve
                tail_q.append((pool_tail, act_accum))

            # drain the pipeline: B of the last chunk, tails, accumulates
            filler(len(pending))
            while accum_slot:
                accum_slot.pop(0)()
            while tail_q:
                pool_tail, act_accum = tail_q.pop(0)
                pool_tail()
                act_accum()
            nc.sync.dma_start(out=out[:, :], in_=acc)
    return nc


def _elide_implied_waits(nc):
    """Drop semaphore waits already implied by program order or transitively
    by earlier waits (vector-clock propagation).  Tile's per-instruction wait
    emission is not transitively minimal, and walrus can encode only one sync
    wait on Activation/DMA instructions (and ~4 on control instructions), so
    the redundant waits both break codegen and waste sequencer time.

    Model: each semaphore s carries a snapshot VC at every increment value;
    an engine's observed VC advances via its own instruction stream and via
    the snapshots of the waits it executes.  A wait (s >= v) is dropped iff
    the engine's observed VC already dominates it.  Unknown update modes
    disable elision for that semaphore (conservative).
    """
    join = lambda a, b: {k: max(a.get(k, 0), b.get(k, 0)) for k in set(a) | set(b)}
    sem_val = {}        # sem name -> current value
    sem_snap = {}       # sem name -> list of (value, VC) snapshots
    eng_vc = {}         # engine name -> observed VC
    unsafe = set()      # sems with non-increment updates
    n_drop = 0
    for f in nc.m.functions:
        for bb in f.blocks:
            for ins in bb.instructions:
                eng = str(ins.engine)
                vc = dict(eng_vc.get(eng, {}))
                si = ins.sync_info
                waits = list(si.on_wait) if si is not None and si.on_wait else []
                kept = []
                for w in waits:
                    s, v = w.ant_name, w.wait_value
                    if w.wait_mode != "sem-ge-imm" or s in unsafe:
                        kept.append(w)
                        continue
                    if vc.get(s, 0) >= v:
                        n_drop += 1
                        continue
                    if sem_val.get(s, 0) < v:
                        # increment not yet seen in emission order; keep and
                        # learn nothing (conservative)
                        kept.append(w)
                        continue
                    kept.append(w)
                    snap = {}
                    for sv, svc in sem_snap.get(s, ()):
                        if sv <= v:
                            snap = svc
                        else:
                            break
                    vc = join(vc, snap)
                    vc[s] = max(vc.get(s, 0), v)
                if si is not None and len(kept) != len(waits):
                    si.on_wait = kept
                # apply this instruction's increments
                ups = si.on_update if si is not None and si.on_update else []
                for u in ups:
                    s = u.ant_name
                    if u.update_mode not in ("sem-inc", "sem-add-imm"):
                        unsafe.add(s)
                        continue
                    nv = sem_val.get(s, 0) + (u.update_value or 1)
                    sem_val[s] = nv
                    lst = sem_snap.setdefault(s, [])
                    prev = lst[-1][1] if lst else {}
                    lst.append((nv, join(prev, vc)))
                    # Engine-sem increments fire when the instruction
                    # completes, and the engine is sequential, so later
                    # instructions on this engine observe them.  DMA-queue
                    # increments fire asynchronously at transfer completion:
                    # the issuing engine must NOT absorb those.
                    if "DMA" not in s:
                        vc[s] = max(vc.get(s, 0), nv)
                eng_vc[eng] = vc
    return n_drop


def _spill_excess_waits(nc):
    """walrus encodes at most ONE sync wait on real engine instructions
    (Activation/DVE/DMA); the Tile scheduler can leave more after
    cross-engine reordering.  Keep one wait on the instruction and hoist
    the rest onto preceding InstEventSemaphore control instructions
    (which accept 2 waits each)."""
    from concourse import mybir
    n_spill = 0
    for f in nc.m.functions:
        for bb in f.blocks:
            out = []
            for ins in bb.instructions:
                si = ins.sync_info
                waits = list(si.on_wait) if si is not None and si.on_wait else []
                is_engine_op = bool(ins.ins) or bool(ins.outs)
                if len(waits) > 1 and is_engine_op and \
                        not isinstance(ins, mybir.InstEventSemaphore):
                    spill = waits[:-1]
                    si.on_wait = waits[-1:]
                    while spill:
                        grp, spill = spill[:2], spill[2:]
                        n_spill += 1
                        ev = mybir.InstEventSemaphore(
                            name=f"wspill_{n_spill}", engine=ins.engine,
                            ins=[], outs=[],
                            sync_info=mybir.SyncInfo(on_wait=grp,
                                                     on_update=[]))
                        out.append(ev)
                out.append(ins)
            bb.instructions = out
    return n_spill


_NC_CACHE = None


def kernel(pred: np.ndarray, target: np.ndarray) -> np.ndarray:
    global _NC_CACHE
    from concourse.bass_utils import run_bass_kernel_spmd

    pred = np.ascontiguousarray(np.asarray(pred, dtype=np.float32))
    target = np.ascontiguousarray(np.asarray(target, dtype=np.float32))
    assert pred.shape == (B, 9) and target.shape == (B, 9)

    if _NC_CACHE is None:
        _NC_CACHE = _build_nc()
        _elide_implied_waits(_NC_CACHE)
        _spill_excess_waits(_NC_CACHE)
    nc = _NC_CACHE

    ps = pred.reshape(N_CORES, ROWS_PER_CORE, 9)
    ts = target.reshape(N_CORES, ROWS_PER_CORE, 9)
    in_maps = [{"pred": ps[i], "target": ts[i]} for i in range(N_CORES)]
    res = run_bass_kernel_spmd(nc, in_maps, core_ids=list(range(N_CORES)))
    globals()["_LAST_RESULT"] = res

    mse_sum = 0.0
    rot_sum = 0.0
    for r in res.results:
        part = np.asarray(r["partials"], dtype=np.float64)
        mse_sum += part[:, :NCHUNK].sum()
        rot_sum += part[:, NCHUNK:].sum()
    n = float(B * 9)
    return np.asarray(np.float32(mse_sum / n + 0.5 * (rot_sum / n)))


# revision 27
# speedup vs baseline: 1.3268x; 1.0755x over previous
"""CustomPoseLoss Trainium2 kernel.

loss = mean((pred-target)^2) + 0.5 * mean((R(pred)-R(target))^2)
where R(M) = sign(det M) * polar(M) for each 3x3 matrix (row of 9).

Implementation: closed-form polar decomposition per row, fully vectorized as
channel-plane arithmetic:
  S = M^T M, normalized by tr(S)/3; eigenvalues of S via Cardano
  (acos/cos evaluated as polynomials so only the ln/exp LUT set is needed);
  W^-1 = (S + s2 I) adj(N) / det(N) with N = ssig*S + Pg*I  (Cayley-Hamilton
  inverse-sqrt);  R = sign(det) * M W^-1 / sqrt(m).
det(N) is formed from the eigenvalue product (positive, cancellation-free)
and clamped, so near-singular rows stay bounded.

Engine plan: heavy products in f16 on DVE (2x mode); scalar chain fp32 on
DVE+ACT; mse-sub / clamp / dR-sub on GpSimd (off critical path).  Chunks are
software-pipelined: chunk k's tail (T2, R = M*T2) is emitted as filler inside
chunk k+1's scalar-chain stalls, so the Vector engine never waits on ACT legs.

Sharding: pure data parallel over 8 cores; each core reduces its shard to
[128, 2*NCHUNK] partial sums (mse, rot), host combines in float64.
"""

import numpy as np

B = 1048576
N_CORES = 8
ROWS_PER_CORE = B // N_CORES          # 131072
P = 128
ROWS_PER_PART = ROWS_PER_CORE // P    # 1024
CHUNKS = (256, 256, 256, 256)         # rows per partition per chunk
NCHUNK = len(CHUNKS)
TMAX = max(CHUNKS)
EPS_D = 1e-5

ACOS_A = (1.5707288, -0.2121144, 0.0742610, -0.0187293)   # A&S 4.4.45
HALF_SQRT3 = 0.8660254037844386

LN3 = float(np.log(3.0))
LN6 = float(np.log(6.0))
LN2 = float(np.log(2.0))
EPS_W = 6e-3


class Regs:
    """[128, 2*TMAX] plane slots with explicit reuse (SBUF is capped)."""

    def __init__(self, pool, dtype, prefix="v", shape=None):
        self.pool = pool
        self.dtype = dtype
        self.prefix = prefix
        self.shape = shape or [P, 2 * TMAX]
        self.free_tags = []
        self.n = 0
        self.tag_of = {}

    def alloc(self, width=None):
        if self.free_tags:
            tag = self.free_tags.pop()
        else:
            self.n += 1
            tag = f"{self.prefix}{self.n}"
        tl = self.pool.tile(self.shape, self.dtype, tag=tag, name=tag)
        if width is not None:
            tl = tl[:, 0:width]
        self.tag_of[id(tl)] = tag
        return tl

    def free(self, *tiles):
        for tl in tiles:
            self.free_tags.append(self.tag_of.pop(id(tl)))


_CONST_STATE = {}
bass_mod = None


def _c(nc, v):
    """[P,1] fp32 constant AP, DVE-memset once (keeps ACT single-wait)."""
    key = float(np.float32(v))
    consts = _CONST_STATE.setdefault(id(nc), {})
    if key not in consts:
        pool = _CONST_STATE[(id(nc), "pool")]
        from concourse import mybir
        t = pool.tile([P, 1], mybir.dt.float32, tag=f"c{len(consts)}")
        nc.vector.memset(t, key)
        consts[key] = t
    return consts[key][:, 0:1]


def _emit_A(nc, env, k, T, praw, traw, tiles, acc_mse_col, fill, pre_cast,
            post_cast):
    """Phase A of chunk k: everything through T1m and w16.

    `fill(n)` emits up to n pending phase-B DVE ops from the previous chunk
    at known ACT-wait stall points.  `pre_cast()` emits the chunk k-2 ACT
    accumulate (whose Pool wait transitively covers the casts' X-tile WAR
    wait — walrus allows only one sync wait per Activation instruction);
    `post_cast()` emits the chunk k+2 DMA prefetch.
    """
    mybir = env["mybir"]
    regs, regs16, bias0 = env["regs"], env["regs16"], env["bias0"]
    Alu = mybir.AluOpType
    Act = mybir.ActivationFunctionType
    L = 2 * T
    X, D, Sm, QS, Am, T1m, Nm, W16 = (tiles[n] for n in
                                      ("X", "D", "Sm", "QS", "Am", "T1m",
                                       "Nm", "W16"))

    def mul(o, a, b):
        nc.vector.tensor_tensor(out=o, in0=a, in1=b, op=Alu.mult)

    def add(o, a, b):
        nc.vector.tensor_tensor(out=o, in0=a, in1=b, op=Alu.add)

    def sub(o, a, b):
        nc.vector.tensor_tensor(out=o, in0=a, in1=b, op=Alu.subtract)

    def vs(o, a, s1, op0, s2=None, op1=None):
        if s2 is None:
            nc.vector.tensor_scalar(out=o, in0=a, scalar1=float(s1),
                                    scalar2=None, op0=getattr(Alu, op0))
        else:
            nc.vector.tensor_scalar(out=o, in0=a, scalar1=float(s1),
                                    scalar2=float(s2), op0=getattr(Alu, op0),
                                    op1=getattr(Alu, op1))

    def stt(o, a, s, b, op0, op1):
        nc.vector.scalar_tensor_tensor(out=o, in0=a, scalar=float(s), in1=b,
                                       op0=getattr(Alu, op0),
                                       op1=getattr(Alu, op1))

    def act(o, a, func, scale=1.0, bias=None, accum_out=None):
        if func == "Copy":
            nc.scalar.activation(out=o, in_=a, func=Act.Copy, bias=0.0,
                                 scale=float(scale), accum_out=accum_out)
        else:
            nc.scalar.activation(out=o, in_=a, func=getattr(Act, func),
                                 bias=bias0[:, 0:1] if bias is None else bias,
                                 scale=float(scale), accum_out=accum_out)

    def bc(plane, n):
        # broadcast [P, L] plane across n sub-planes -> [P, n, L]
        return bass_mod.AP(tensor=plane.tensor, offset=plane.offset,
                           ap=[plane.ap[0], [0, n], plane.ap[1]])

    A = lambda: regs.alloc(L)       # fp32 [P, L] planes
    H = lambda: regs16.alloc(L)     # f16 [P, L] planes

    # ---- cast+deinterleave both inputs into X[P, 9, 2T] (f16) ----
    rvp = praw[:, 0:T * 9].rearrange("p (n c) -> p n c", c=9)
    rvt = traw[:, 0:T * 9].rearrange("p (n c) -> p n c", c=9)
    xin_p = bass_mod.AP(tensor=rvp.tensor, offset=rvp.offset,
                        ap=[rvp.ap[0], rvp.ap[2], rvp.ap[1]])
    xin_t = bass_mod.AP(tensor=rvt.tensor, offset=rvt.offset,
                        ap=[rvt.ap[0], rvt.ap[2], rvt.ap[1]])
    act(X[:, :, 0:T], xin_p, "Copy")
    act(X[:, :, T:L], xin_t, "Copy")
    post_cast()
    x = [X[:, c, 0:L] for c in range(9)]        # [P, L] f16 unit-stride

    # ---- channel squares for the S diagonal (ACT; overlaps DVE det/S) ----
    act(tiles["SQ"][:, :, 0:L], X[:, :, 0:L], "Square")

    # ---- mse sub on GpSimd (off critical path) ----
    Dv = D[:, :, 0:T]
    nc.gpsimd.tensor_tensor(out=Dv, in0=X[:, :, 0:T], in1=X[:, :, T:L],
                            op=Alu.subtract)

    fill(6)

    # ---- det(M) f16 from X planes (emitted first: its ACT consumers
    #      run during the S block) ----
    cA16, cB16, det16 = H(), H(), H()
    mul(cA16, x[4], x[8]); mul(cB16, x[5], x[7]); sub(cA16, cA16, cB16)
    mul(det16, x[0], cA16)
    mul(cA16, x[5], x[6]); mul(cB16, x[3], x[8]); sub(cA16, cA16, cB16)
    mul(cA16, x[1], cA16); add(det16, det16, cA16)
    mul(cA16, x[3], x[7]); mul(cB16, x[4], x[6]); sub(cA16, cA16, cB16)
    mul(cA16, x[2], cA16); add(det16, det16, cA16)
    sgd16 = H(); act(sgd16, det16, "Sign")
    lnad, ad32 = A(), A()
    act(ad32, det16, "Abs")
    act(lnad, ad32, "Ln")
    regs.free(ad32)
    regs16.free(cA16, cB16, det16)
    # mse Square-accum here: its DVE wait is covered by the det ACT ops
    # above (walrus allows only one sync wait per Activation instruction)
    act(Dv, Dv, "Square", accum_out=acc_mse_col)
    # previous chunk's rot accumulate lands in the same ACT lull
    pre_cast()

    # ---- S = M^T M (f16): order [s00,s11,s22,s01,s02,s12] ----
    # diag from the ACT-computed channel squares (SQ = X*X): 2 DVE adds
    tmp16 = H()
    SQ = tiles["SQ"]
    add(Sm[:, 0:3, 0:L], SQ[:, 0:3, 0:L], SQ[:, 3:6, 0:L])
    add(Sm[:, 0:3, 0:L], Sm[:, 0:3, 0:L], SQ[:, 6:9, 0:L])
    for oi, (ia, ib) in enumerate((((0, 3, 6), (1, 4, 7)),
                                   ((0, 3, 6), (2, 5, 8)),
                                   ((1, 4, 7), (2, 5, 8)))):
        so = Sm[:, 3 + oi, 0:L]
        mul(so, x[ia[0]], x[ib[0]])
        mul(tmp16, x[ia[1]], x[ib[1]]); add(so, so, tmp16)
        mul(tmp16, x[ia[2]], x[ib[2]]); add(so, so, tmp16)
    # tr and normalization scale q = 3/tr (ln domain)
    tr16 = H()
    add(tr16, Sm[:, 0, 0:L], Sm[:, 1, 0:L]); add(tr16, tr16, Sm[:, 2, 0:L])
    vs(tr16, tr16, 6e-5, "max")
    lnt = A(); act(lnt, tr16, "Ln")
    q16 = H(); act(q16, lnt, "Exp", scale=-1.0, bias=_c(nc, LN3))
    regs16.free(tr16)
    fill(5)
    nc.vector.tensor_tensor(out=Sm[:, :, 0:L], in0=Sm[:, :, 0:L],
                            in1=bc(q16, 6), op=Alu.mult)
    regs16.free(q16)

    # Pg = exp(lnad + 1.5*(ln3 - lnt));  dets = Pg^2
    lnPg = A()
    stt(lnPg, lnt, -1.5, lnad, "mult", "add")
    regs.free(lnad)
    Pg16 = H(); act(Pg16, lnPg, "Exp", scale=1.0, bias=_c(nc, 1.5 * LN3))
    dets = A(); act(dets, Pg16, "Square")
    regs.free(lnPg)

    # ---- tr(S^2) fp32 from normalized f16 S ----
    act(QS[:, :, 0:L], Sm[:, :, 0:L], "Square")
    fill(3)
    u1, u2 = A(), A()
    add(u1, QS[:, 0, 0:L], QS[:, 1, 0:L]); add(u1, u1, QS[:, 2, 0:L])
    add(u2, QS[:, 3, 0:L], QS[:, 4, 0:L]); add(u2, u2, QS[:, 5, 0:L])
    trS2 = A()
    stt(trS2, u2, 2.0, u1, "mult", "add")
    # p and 1/(2 p^3) via ln/exp; p itself is f16 (feeds the f16 tail)
    trK2, ip3h = u1, u2                    # reuse u1/u2 slots
    p = H()
    vs(trK2, trS2, -3.0, "add", 1e-30, "max")
    lnk = A(); act(lnk, trK2, "Ln")
    act(p, lnk, "Exp", scale=0.5, bias=_c(nc, -0.5 * LN6))
    act(ip3h, lnk, "Exp", scale=-1.5, bias=_c(nc, 1.5 * LN6 - LN2))
    regs.free(lnk)
    # arg
    detK, arg = A(), A()
    stt(detK, trS2, 0.5, dets, "mult", "add")
    vs(detK, detK, -2.5, "add")
    fill(3)
    vs(ip3h, ip3h, 1e30, "min")
    mul(arg, detK, ip3h)
    vs(arg, arg, 1.0, "min", -1.0, "max")
    regs.free(detK, trS2, dets, u2)   # u2 == ip3h
    # ---- th3 = acos(arg); poly arithmetic in f16 (values are O(1) and
    #      smooth), fp32 kept only for the cancelling 1-y subtraction ----
    y, om32 = A(), A()
    act(y, arg, "Abs")
    y16, sg16, h = H(), H(), H()
    act(y16, arg, "Abs")
    act(sg16, arg, "Sign")
    fill(2)
    vs(om32, y, -1.0, "mult", 1.0, "add")
    lnom = arg                                   # reuse slot
    act(lnom, om32, "Ln")
    om16 = H()
    act(om16, lnom, "Exp", scale=0.5)            # sqrt(1-y), f16
    vs(h, y16, ACOS_A[3], "mult", ACOS_A[2], "add")
    mul(h, h, y16); vs(h, h, ACOS_A[1], "add")
    mul(h, h, y16); vs(h, h, ACOS_A[0], "add")
    fill(3)
    mul(h, h, om16)
    th3, lin = om16, y16                         # reuse slots
    vs(lin, sg16, -np.pi / 2, "mult", np.pi / 2, "add")
    mul(th3, sg16, h); add(th3, th3, lin)
    regs.free(y, om32, arg)    # arg == lnom
    regs16.free(h, sg16)
    # ---- cos((th3+2pik)/3) ----
    z, c0p = H(), H()
    act(z, th3, "Square", scale=1.0 / 3.0)
    fill(2)
    vs(c0p, z, 1.0 / 40320.0, "mult", -1.0 / 720.0, "add")
    mul(c0p, c0p, z); vs(c0p, c0p, 1.0 / 24.0, "add")
    mul(c0p, c0p, z); vs(c0p, c0p, -0.5, "add")
    mul(c0p, c0p, z); vs(c0p, c0p, 1.0, "add")
    regs16.free(z, om16, y16)   # om16 == th3, y16 == lin
    s032 = A()
    act(s032, c0p, "Square")
    uc1, c1p, c2p, s016 = H(), H(), H(), H()
    vs(uc1, c0p, -0.5, "mult")
    fill(2)
    vs(s032, s032, -1.0, "mult", 1.0, "add")
    vs(s032, s032, 0.0, "max")
    lns = A(); act(lns, s032, "Ln")
    act(s016, lns, "Exp", scale=0.5)
    regs.free(lns, s032)
    fill(3)
    vs(s016, s016, HALF_SQRT3, "mult")
    sub(c1p, uc1, s016)
    add(c2p, uc1, s016)
    regs16.free(s016, uc1)
    # ---- lambda_k, g_k = sqrt(lambda_k) (f16) ----
    tp = H()
    vs(tp, p, 2.0, "mult")
    regs16.free(p)
    lam, g, lnls = [], [], []
    for ck in (c0p, c1p, c2p):
        lk, lnl = H(), A()
        mul(lk, tp, ck)
        vs(lk, lk, 1.0, "add", 6.5e-5, "max")
        act(lnl, lk, "Ln")
        lam.append(lk); lnls.append(lnl)
    for lnl in lnls:
        gk = H()
        act(gk, lnl, "Exp", scale=0.5)
        g.append(gk)
        regs.free(lnl)
    regs16.free(tp, c0p, c1p, c2p)
    fill(5)
    g01, ssig16, s2i16, tmp2 = H(), H(), H(), H()
    add(g01, g[0], g[1])
    add(ssig16, g01, g[2])
    mul(s2i16, g[0], g[1]); mul(tmp2, g[2], g01); add(s2i16, s2i16, tmp2)
    regs16.free(g01, tmp2, *g)
    # ---- w = sign/(sqrt(m)*detN) via ln domain, clamped ----
    # detN = prod_k (ssig*lam_k + Pg): f16 product, one Ln.  Underflow of
    # the product flushes to 0 -> Ln = -inf -> caught by the max clamp.
    t_ = A()
    nk0, nk1 = H(), H()
    mul(nk0, ssig16, lam[0]); add(nk0, nk0, Pg16)
    mul(nk1, ssig16, lam[1]); add(nk1, nk1, Pg16)
    mul(nk0, nk0, nk1)
    mul(nk1, ssig16, lam[2]); add(nk1, nk1, Pg16)
    mul(nk0, nk0, nk1)
    act(t_, nk0, "Ln")
    regs16.free(nk0, nk1)
    fill(3)
    stt(t_, lnt, 0.5, t_, "mult", "add")
    vs(t_, t_, float(np.log(EPS_W) + 0.5 * LN3), "max")
    wmag16 = H()
    act(wmag16, t_, "Exp", scale=-1.0, bias=_c(nc, 0.5 * LN3))
    regs.free(t_, lnt)
    for lk in lam:
        regs16.free(lk)

    # ---- N = ssig*S + Pg*I (f16, batched) ----
    fill(3)
    nc.vector.tensor_tensor(out=Nm[:, :, 0:L], in0=Sm[:, :, 0:L],
                            in1=bc(ssig16, 6), op=Alu.mult)
    nc.vector.tensor_tensor(out=Nm[:, 0:3, 0:L], in0=Nm[:, 0:3, 0:L],
                            in1=bc(Pg16, 3), op=Alu.add)
    regs16.free(Pg16, ssig16)
    # A' diagonal (Am = S_diag + s2)
    nc.vector.tensor_tensor(out=Am[:, :, 0:L], in0=Sm[:, 0:3, 0:L],
                            in1=bc(s2i16, 3), op=Alu.add)
    regs16.free(s2i16)
    # w16 = sign(det) * wmag  (into the double-buffered W16 plane)
    mul(W16[:, 0:L], wmag16, sgd16)
    regs16.free(wmag16, sgd16)
    # ---- adj(N) (f16) ----
    n00, n11, n22 = (Nm[:, i, 0:L] for i in range(3))
    n01, n02, n12 = (Nm[:, i, 0:L] for i in range(3, 6))
    aj = [H() for _ in range(6)]
    a00, a01, a02, a11, a12, a22 = aj

    def cof(o, a, b, c, dd):
        mul(o, a, b); mul(tmp16, c, dd); sub(o, o, tmp16)

    cof(a00, n11, n22, n12, n12)
    cof(a01, n02, n12, n01, n22)
    cof(a02, n01, n12, n02, n11)
    cof(a11, n00, n22, n02, n02)
    cof(a12, n01, n02, n00, n12)
    cof(a22, n00, n11, n01, n01)
    # ---- T1 = A' adjN (f16); rows of A': (b0,s01,s02),(s01,b1,s12),(s02,s12,b2)
    b0, b1, b2 = (Am[:, i, 0:L] for i in range(3))
    s01p, s02p, s12p = Sm[:, 3, 0:L], Sm[:, 4, 0:L], Sm[:, 5, 0:L]

    def mm3(o, r0, r1, r2, k0, k1, k2):
        mul(o, r0, k0)
        mul(tmp16, r1, k1); add(o, o, tmp16)
        mul(tmp16, r2, k2); add(o, o, tmp16)

    mm3(T1m[:, 0, 0:L], b0, s01p, s02p, a00, a01, a02)
    mm3(T1m[:, 1, 0:L], b0, s01p, s02p, a01, a11, a12)
    mm3(T1m[:, 2, 0:L], b0, s01p, s02p, a02, a12, a22)
    mm3(T1m[:, 3, 0:L], s01p, b1, s12p, a01, a11, a12)
    mm3(T1m[:, 4, 0:L], s01p, b1, s12p, a02, a12, a22)
    mm3(T1m[:, 5, 0:L], s02p, s12p, b2, a02, a12, a22)
    regs16.free(*aj)
    regs16.free(tmp16)


def _make_B(nc, env, k, T, tiles, acc_rot_col, last=False):
    """Phase B of chunk k: T2 = w*T1, R = M*T2 as a list of single-op DVE
    thunks (drained as filler during chunk k+1's phase A).  Three of the
    nine R outputs run on GpSimd (emitted with the T2 thunk).  The tail
    (clamp + dR sub) also runs on GpSimd, and the ACT accumulate thunk is
    emitted early in chunk k+2.  For the last chunk everything stays on
    DVE (it is idle during the pipeline drain)."""
    mybir = env["mybir"]
    Alu = mybir.AluOpType
    Act = mybir.ActivationFunctionType
    L = 2 * T
    X, T1m, Nm, W16, RT, BT, PT = (tiles[n] for n in
                                   ("X", "T1m", "Nm", "W16", "RT", "BT",
                                    "PT"))
    x = [X[:, c, 0:L] for c in range(9)]
    tail_eng = nc.vector if last else nc.gpsimd

    def mul(o, a, b):
        nc.vector.tensor_tensor(out=o, in0=a, in1=b, op=Alu.mult)

    def add(o, a, b):
        nc.vector.tensor_tensor(out=o, in0=a, in1=b, op=Alu.add)

    def bc(plane, n):
        return bass_mod.AP(tensor=plane.tensor, offset=plane.offset,
                           ap=[plane.ap[0], [0, n], plane.ap[1]])

    T2f = Nm
    t00, t01, t02 = T2f[:, 0, 0:L], T2f[:, 1, 0:L], T2f[:, 2, 0:L]
    t11, t12, t22 = T2f[:, 3, 0:L], T2f[:, 4, 0:L], T2f[:, 5, 0:L]
    T2 = [[t00, t01, t02], [t01, t11, t12], [t02, t12, t22]]
    tmpr = BT[:, 0:L]
    tmpp = PT[:, 0:L]

    def emit_out(i, j, eng, tmp):
        o = RT[:, 3 * i + j, 0:L]
        eng.tensor_tensor(out=o, in0=x[3 * i], in1=T2[0][j], op=Alu.mult)
        eng.tensor_tensor(out=tmp, in0=x[3 * i + 1], in1=T2[1][j],
                          op=Alu.mult)
        eng.tensor_tensor(out=o, in0=o, in1=tmp, op=Alu.add)
        eng.tensor_tensor(out=tmp, in0=x[3 * i + 2], in1=T2[2][j],
                          op=Alu.mult)
        eng.tensor_tensor(out=o, in0=o, in1=tmp, op=Alu.add)

    def t2_op():
        # T2 = w*T1 in f16 (Nm tile is dead after adj; reuse it)
        nc.vector.tensor_tensor(out=T2f[:, :, 0:L], in0=T1m[:, :, 0:L],
                                in1=bc(W16[:, 0:L], 6), op=Alu.mult)

    dve = [t2_op]
    cols = (0, 1, 2)
    for i in range(3):
        for j in cols:
            o = RT[:, 3 * i + j, 0:L]
            dve.append(lambda o=o, i=i, j=j: mul(o, x[3 * i], T2[0][j]))
            dve.append(lambda i=i, j=j: mul(tmpr, x[3 * i + 1], T2[1][j]))
            dve.append(lambda o=o: add(o, o, tmpr))
            dve.append(lambda i=i, j=j: mul(tmpr, x[3 * i + 2], T2[2][j]))
            dve.append(lambda o=o: add(o, o, tmpr))

    def pool_tail():
        # clamp junk rows, dR = R_pred - R_target in place (off DVE)
        tail_eng.tensor_scalar(out=RT[:, :, 0:L], in0=RT[:, :, 0:L],
                               scalar1=8.0, scalar2=-8.0,
                               op0=Alu.min, op1=Alu.max)
        tail_eng.tensor_tensor(out=RT[:, :, 0:T], in0=RT[:, :, 0:T],
                               in1=RT[:, :, T:L], op=Alu.subtract)

    def act_accum():
        bias0 = env["bias0"]
        nc.scalar.activation(out=RT[:, :, 0:T], in_=RT[:, :, 0:T],
                             func=Act.Square, bias=bias0[:, 0:1], scale=1.0,
                             accum_out=acc_rot_col)

    return dve, pool_tail, act_accum


def _build_nc():
    global bass_mod
    import concourse.bass as bass
    import concourse.tile as tile
    from concourse import mybir
    bass_mod = bass

    f32 = mybir.dt.float32
    f16 = mybir.dt.float16
    nc = bass.Bass()
    pred = nc.dram_tensor("pred", [ROWS_PER_CORE, 9], f32, kind="ExternalInput")
    targ = nc.dram_tensor("target", [ROWS_PER_CORE, 9], f32, kind="ExternalInput")
    out = nc.dram_tensor("partials", [P, 2 * NCHUNK], f32, kind="ExternalOutput")

    predv = pred.rearrange("(p n) c -> p n c", p=P)    # [128, 1024, 9]
    targv = targ.rearrange("(p n) c -> p n c", p=P)
    row0 = np.cumsum((0,) + CHUNKS)                    # chunk row offsets

    with tile.TileContext(nc) as tc:
        with (
            tc.tile_pool(name="raw", bufs=1) as rawp,
            tc.tile_pool(name="pl", bufs=1) as pl,
            tc.tile_pool(name="acc", bufs=1) as accp,
        ):
            acc = accp.tile([P, 2 * NCHUNK], f32, tag="acc")
            bias0 = accp.tile([P, 1], f32, tag="bias0")
            nc.vector.memset(bias0, 0.0)
            _CONST_STATE[(id(nc), "pool")] = accp
            env = {
                "mybir": mybir,
                "regs": Regs(pl, f32, prefix="v"),
                "regs16": Regs(pl, f16, prefix="h"),
                "bias0": bias0,
            }

            raw_cache = {}

            def raw_tiles(k):
                if k not in raw_cache:
                    raw_cache[k] = (
                        rawp.tile([P, TMAX * 9], f32, tag=f"praw{k % 2}",
                                  name=f"praw{k % 2}"),
                        rawp.tile([P, TMAX * 9], f32, tag=f"traw{k % 2}",
                                  name=f"traw{k % 2}"))
                return raw_cache[k]

            def chunk_tiles(k):
                j = k % 2
                return {
                    "X": pl.tile([P, 9, 2 * TMAX], f16, tag=f"X{j}", name=f"X{j}"),
                    "D": pl.tile([P, 9, TMAX], f16, tag="D", name="D"),
                    "Sm": pl.tile([P, 6, 2 * TMAX], f16, tag=f"Sm{j}", name=f"Sm{j}"),
                    "QS": pl.tile([P, 6, 2 * TMAX], f32, tag="QS", name="QS"),
                    "Am": pl.tile([P, 3, 2 * TMAX], f16, tag="Am", name="Am"),
                    "T1m": pl.tile([P, 6, 2 * TMAX], f16, tag=f"T1m{j}", name=f"T1m{j}"),
                    "Nm": pl.tile([P, 6, 2 * TMAX], f16, tag=f"Nm{j}", name=f"Nm{j}"),
                    "W16": pl.tile([P, 2 * TMAX], f16, tag=f"W16{j}", name=f"W16{j}"),
                    "RT": pl.tile([P, 9, 2 * TMAX], f16, tag=f"RT{j}", name=f"RT{j}"),
                    "BT": pl.tile([P, 2 * TMAX], f16, tag=f"BT{j}", name=f"BT{j}"),
                    "PT": pl.tile([P, 2 * TMAX], f16, tag=f"PT{j}", name=f"PT{j}"),
                    "SQ": pl.tile([P, 9, 2 * TMAX], f16, tag="SQ", name="SQ"),
                }

            def dma_chunk(k):
                prw, trw = raw_tiles(k)
                t0, t1 = row0[k], row0[k + 1]
                n = (t1 - t0) * 9
                nc.sync.dma_start(out=prw[:, 0:n], in_=predv[:, t0:t1, :])
                nc.sync.dma_start(out=trw[:, 0:n], in_=targv[:, t0:t1, :])

            dma_chunk(0)
            dma_chunk(1)

            pending = []        # phase-B DVE thunks of chunk k-1
            tail_q = []         # (pool_tail, act_accum) of chunk k-1
            accum_slot = []     # ACT accumulates to emit at next post_cast

            def filler(n):
                for _ in range(min(n, len(pending))):
                    pending.pop(0)()

            for k, T in enumerate(CHUNKS):
                praw, traw = raw_tiles(k)
                tiles = chunk_tiles(k)

                def pre_cast():
                    while accum_slot:
                        accum_slot.pop(0)()

                def post_cast(k=k):
                    if k + 2 < NCHUNK:
                        dma_chunk(k + 2)

                _emit_A(nc, env, k, T, praw, traw, tiles,
                        acc[:, k:k + 1], filler, pre_cast, post_cast)
                # drain the rest of chunk k-1's phase B, then its GpSimd tail;
                # its ACT accumulate goes early into chunk k+1
                filler(len(pending))
                if tail_q:
                    pool_tail, act_accum = tail_q.pop(0)
                    pool_tail()
                    accum_slot.append(act_accum)
                dve, pool_tail, act_accum = _make_B(
                    nc, env, k, T, tiles, acc[:, NCHUNK + k:NCHUNK + k + 1],
                    last=(k == NCHUNK - 1))
                pending = dve
                tail_q.append((pool_tail, act_accum))

            # drain the pipeline: B of the last chunk, tails, accumulates
            filler(len(pending))
            while accum_slot:
                accum_slot.pop(0)()
            while tail_q:
                pool_tail, act_accum = tail_q.pop(0)
                pool_tail()
                act_accum()
            nc.sync.dma_start(out=out[:, :], in_=acc)
    return nc


def _elide_implied_waits(nc):
    """Drop semaphore waits already implied by program order or transitively
    by earlier waits (vector-clock propagation).  Tile's per-instruction wait
    emission is not transitively minimal, and walrus can encode only one sync
    wait on Activation/DMA instructions (and ~4 on control instructions), so
    the redundant waits both break codegen and waste sequencer time.

    Model: each semaphore s carries a snapshot VC at every increment value;
    an engine's observed VC advances via its own instruction stream and via
    the snapshots of the waits it executes.  A wait (s >= v) is dropped iff
    the engine's observed VC already dominates it.  Unknown update modes
    disable elision for that semaphore (conservative).
    """
    join = lambda a, b: {k: max(a.get(k, 0), b.get(k, 0)) for k in set(a) | set(b)}
    sem_val = {}        # sem name -> current value
    sem_snap = {}       # sem name -> list of (value, VC) snapshots
    eng_vc = {}         # engine name -> observed VC
    unsafe = set()      # sems with non-increment updates
    n_drop = 0
    for f in nc.m.functions:
        for bb in f.blocks:
            for ins in bb.instructions:
                eng = str(ins.engine)
                vc = dict(eng_vc.get(eng, {}))
                si = ins.sync_info
                waits = list(si.on_wait) if si is not None and si.on_wait else []
                kept = []
                for w in waits:
                    s, v = w.ant_name, w.wait_value
                    if w.wait_mode != "sem-ge-imm" or s in unsafe:
                        kept.append(w)
                        continue
                    if vc.get(s, 0) >= v:
                        n_drop += 1
                        continue
                    if sem_val.get(s, 0) < v:
                        # increment not yet seen in emission order; keep and
                        # learn nothing (conservative)
                        kept.append(w)
                        continue
                    kept.append(w)
                    snap = {}
                    for sv, svc in sem_snap.get(s, ()):
                        if sv <= v:
                            snap = svc
                        else:
                            break
                    vc = join(vc, snap)
                    vc[s] = max(vc.get(s, 0), v)
                if si is not None and len(kept) != len(waits):
                    si.on_wait = kept
                # apply this instruction's increments
                ups = si.on_update if si is not None and si.on_update else []
                for u in ups:
                    s = u.ant_name
                    if u.update_mode not in ("sem-inc", "sem-add-imm"):
                        unsafe.add(s)
                        continue
                    nv = sem_val.get(s, 0) + (u.update_value or 1)
                    sem_val[s] = nv
                    lst = sem_snap.setdefault(s, [])
                    prev = lst[-1][1] if lst else {}
                    lst.append((nv, join(prev, vc)))
                    # Engine-sem increments fire when the instruction
                    # completes, and the engine is sequential, so later
                    # instructions on this engine observe them.  DMA-queue
                    # increments fire asynchronously at transfer completion:
                    # the issuing engine must NOT absorb those.
                    if "DMA" not in s:
                        vc[s] = max(vc.get(s, 0), nv)
                eng_vc[eng] = vc
    return n_drop


def _spill_excess_waits(nc):
    """walrus encodes at most ONE sync wait on real engine instructions
    (Activation/DVE/DMA); the Tile scheduler can leave more after
    cross-engine reordering.  Keep one wait on the instruction and hoist
    the rest onto preceding InstEventSemaphore control instructions
    (which accept 2 waits each)."""
    from concourse import mybir
    n_spill = 0
    for f in nc.m.functions:
        for bb in f.blocks:
            out = []
            for ins in bb.instructions:
                si = ins.sync_info
                waits = list(si.on_wait) if si is not None and si.on_wait else []
                is_engine_op = bool(ins.ins) or bool(ins.outs)
                if len(waits) > 1 and is_engine_op and \
                        not isinstance(ins, mybir.InstEventSemaphore):
                    spill = waits[:-1]
                    si.on_wait = waits[-1:]
                    while spill:
                        grp, spill = spill[:2], spill[2:]
                        n_spill += 1
                        ev = mybir.InstEventSemaphore(
                            name=f"wspill_{n_spill}", engine=ins.engine,
                            ins=[], outs=[],
                            sync_info=mybir.SyncInfo(on_wait=grp,
                                                     on_update=[]))
                        out.append(ev)
                out.append(ins)
            bb.instructions = out
    return n_spill


_NC_CACHE = None


def kernel(pred: np.ndarray, target: np.ndarray) -> np.ndarray:
    global _NC_CACHE
    from concourse.bass_utils import run_bass_kernel_spmd

    pred = np.ascontiguousarray(np.asarray(pred, dtype=np.float32))
    target = np.ascontiguousarray(np.asarray(target, dtype=np.float32))
    assert pred.shape == (B, 9) and target.shape == (B, 9)

    if _NC_CACHE is None:
        _NC_CACHE = _build_nc()
        _elide_implied_waits(_NC_CACHE)
        _spill_excess_waits(_NC_CACHE)
    nc = _NC_CACHE

    ps = pred.reshape(N_CORES, ROWS_PER_CORE, 9)
    ts = target.reshape(N_CORES, ROWS_PER_CORE, 9)
    in_maps = [{"pred": ps[i], "target": ts[i]} for i in range(N_CORES)]
    res = run_bass_kernel_spmd(nc, in_maps, core_ids=list(range(N_CORES)))
    globals()["_LAST_RESULT"] = res

    mse_sum = 0.0
    rot_sum = 0.0
    for r in res.results:
        part = np.asarray(r["partials"], dtype=np.float64)
        mse_sum += part[:, :NCHUNK].sum()
        rot_sum += part[:, NCHUNK:].sum()
    n = float(B * 9)
    return np.asarray(np.float32(mse_sum / n + 0.5 * (rot_sum / n)))


# revision 32
# speedup vs baseline: 1.3354x; 1.0065x over previous
"""CustomPoseLoss Trainium2 kernel.

loss = mean((pred-target)^2) + 0.5 * mean((R(pred)-R(target))^2)
where R(M) = sign(det M) * polar(M) for each 3x3 matrix (row of 9).

Implementation: closed-form polar decomposition per row, fully vectorized as
channel-plane arithmetic:
  S = M^T M, normalized by tr(S)/3; eigenvalues of S via Cardano
  (acos/cos evaluated as polynomials so only the ln/exp LUT set is needed);
  W^-1 = (S + s2 I) adj(N) / det(N) with N = ssig*S + Pg*I  (Cayley-Hamilton
  inverse-sqrt);  R = sign(det) * M W^-1 / sqrt(m).
det(N) is formed from the eigenvalue product (positive, cancellation-free)
and clamped, so near-singular rows stay bounded.

Engine plan: heavy products in f16 on DVE (2x mode); scalar chain fp32 on
DVE+ACT; mse-sub / clamp / dR-sub on GpSimd (off critical path).  Chunks are
software-pipelined: chunk k's tail (T2, R = M*T2) is emitted as filler inside
chunk k+1's scalar-chain stalls, so the Vector engine never waits on ACT legs.

Sharding: pure data parallel over 8 cores; each core reduces its shard to
[128, 2*NCHUNK] partial sums (mse, rot), host combines in float64.
"""

import numpy as np

B = 1048576
N_CORES = 8
ROWS_PER_CORE = B // N_CORES          # 131072
P = 128
ROWS_PER_PART = ROWS_PER_CORE // P    # 1024
CHUNKS = (256, 256, 256, 256)         # rows per partition per chunk
NCHUNK = len(CHUNKS)
TMAX = max(CHUNKS)
EPS_D = 1e-5

ACOS_A = (1.5707288, -0.2121144, 0.0742610, -0.0187293)   # A&S 4.4.45
HALF_SQRT3 = 0.8660254037844386

LN3 = float(np.log(3.0))
LN6 = float(np.log(6.0))
LN2 = float(np.log(2.0))
EPS_W = 6e-3


class Regs:
    """[128, 2*TMAX] plane slots with explicit reuse (SBUF is capped)."""

    def __init__(self, pool, dtype, prefix="v", shape=None):
        self.pool = pool
        self.dtype = dtype
        self.prefix = prefix
        self.shape = shape or [P, 2 * TMAX]
        self.free_tags = []
        self.n = 0
        self.tag_of = {}

    def alloc(self, width=None):
        if self.free_tags:
            tag = self.free_tags.pop()
        else:
            self.n += 1
            tag = f"{self.prefix}{self.n}"
        tl = self.pool.tile(self.shape, self.dtype, tag=tag, name=tag)
        if width is not None:
            tl = tl[:, 0:width]
        self.tag_of[id(tl)] = tag
        return tl

    def free(self, *tiles):
        for tl in tiles:
            self.free_tags.append(self.tag_of.pop(id(tl)))


_CONST_STATE = {}
bass_mod = None


def _c(nc, v):
    """[P,1] fp32 constant AP, DVE-memset once (keeps ACT single-wait)."""
    key = float(np.float32(v))
    consts = _CONST_STATE.setdefault(id(nc), {})
    if key not in consts:
        pool = _CONST_STATE[(id(nc), "pool")]
        from concourse import mybir
        t = pool.tile([P, 1], mybir.dt.float32, tag=f"c{len(consts)}")
        nc.vector.memset(t, key)
        consts[key] = t
    return consts[key][:, 0:1]


def _emit_A(nc, env, k, T, praw, traw, tiles, acc_mse_col, fill, pre_cast,
            post_cast):
    """Phase A of chunk k: everything through T1m and w16.

    `fill(n)` emits up to n pending phase-B DVE ops from the previous chunk
    at known ACT-wait stall points.  `pre_cast()` emits the chunk k-2 ACT
    accumulate (whose Pool wait transitively covers the casts' X-tile WAR
    wait — walrus allows only one sync wait per Activation instruction);
    `post_cast()` emits the chunk k+2 DMA prefetch.
    """
    mybir = env["mybir"]
    regs, regs16, bias0 = env["regs"], env["regs16"], env["bias0"]
    Alu = mybir.AluOpType
    Act = mybir.ActivationFunctionType
    L = 2 * T
    X, D, Sm, QS, Am, T1m, Nm, W16 = (tiles[n] for n in
                                      ("X", "D", "Sm", "QS", "Am", "T1m",
                                       "Nm", "W16"))

    def mul(o, a, b):
        nc.vector.tensor_tensor(out=o, in0=a, in1=b, op=Alu.mult)

    def add(o, a, b):
        nc.vector.tensor_tensor(out=o, in0=a, in1=b, op=Alu.add)

    def sub(o, a, b):
        nc.vector.tensor_tensor(out=o, in0=a, in1=b, op=Alu.subtract)

    def vs(o, a, s1, op0, s2=None, op1=None):
        if s2 is None:
            nc.vector.tensor_scalar(out=o, in0=a, scalar1=float(s1),
                                    scalar2=None, op0=getattr(Alu, op0))
        else:
            nc.vector.tensor_scalar(out=o, in0=a, scalar1=float(s1),
                                    scalar2=float(s2), op0=getattr(Alu, op0),
                                    op1=getattr(Alu, op1))

    def stt(o, a, s, b, op0, op1):
        nc.vector.scalar_tensor_tensor(out=o, in0=a, scalar=float(s), in1=b,
                                       op0=getattr(Alu, op0),
                                       op1=getattr(Alu, op1))

    def act(o, a, func, scale=1.0, bias=None, accum_out=None):
        if func == "Copy":
            nc.scalar.activation(out=o, in_=a, func=Act.Copy, bias=0.0,
                                 scale=float(scale), accum_out=accum_out)
        else:
            nc.scalar.activation(out=o, in_=a, func=getattr(Act, func),
                                 bias=bias0[:, 0:1] if bias is None else bias,
                                 scale=float(scale), accum_out=accum_out)

    def bc(plane, n):
        # broadcast [P, L] plane across n sub-planes -> [P, n, L]
        return bass_mod.AP(tensor=plane.tensor, offset=plane.offset,
                           ap=[plane.ap[0], [0, n], plane.ap[1]])

    A = lambda: regs.alloc(L)       # fp32 [P, L] planes
    H = lambda: regs16.alloc(L)     # f16 [P, L] planes

    # ---- cast+deinterleave both inputs into X[P, 9, 2T] (f16) ----
    rvp = praw[:, 0:T * 9].rearrange("p (n c) -> p n c", c=9)
    rvt = traw[:, 0:T * 9].rearrange("p (n c) -> p n c", c=9)
    xin_p = bass_mod.AP(tensor=rvp.tensor, offset=rvp.offset,
                        ap=[rvp.ap[0], rvp.ap[2], rvp.ap[1]])
    xin_t = bass_mod.AP(tensor=rvt.tensor, offset=rvt.offset,
                        ap=[rvt.ap[0], rvt.ap[2], rvt.ap[1]])
    act(X[:, :, 0:T], xin_p, "Copy")
    act(X[:, :, T:L], xin_t, "Copy")
    post_cast()
    x = [X[:, c, 0:L] for c in range(9)]        # [P, L] f16 unit-stride

    # ---- channel squares for the S diagonal (ACT; overlaps DVE det/S) ----
    act(tiles["SQ"][:, :, 0:L], X[:, :, 0:L], "Square")

    # ---- mse sub on GpSimd (off critical path) ----
    Dv = D[:, :, 0:T]
    nc.gpsimd.tensor_tensor(out=Dv, in0=X[:, :, 0:T], in1=X[:, :, T:L],
                            op=Alu.subtract)

    fill(6)

    # ---- det(M) f16 from X planes (emitted first: its ACT consumers
    #      run during the S block) ----
    cA16, cB16, det16 = H(), H(), H()
    mul(cA16, x[4], x[8]); mul(cB16, x[5], x[7]); sub(cA16, cA16, cB16)
    mul(det16, x[0], cA16)
    mul(cA16, x[5], x[6]); mul(cB16, x[3], x[8]); sub(cA16, cA16, cB16)
    mul(cA16, x[1], cA16); add(det16, det16, cA16)
    mul(cA16, x[3], x[7]); mul(cB16, x[4], x[6]); sub(cA16, cA16, cB16)
    mul(cA16, x[2], cA16); add(det16, det16, cA16)
    sgd16 = H(); act(sgd16, det16, "Sign")
    lnad, ad32 = A(), A()
    act(ad32, det16, "Abs")
    act(lnad, ad32, "Ln")
    regs.free(ad32)
    regs16.free(cA16, cB16, det16)
    # mse Square-accum here: its DVE wait is covered by the det ACT ops
    # above (walrus allows only one sync wait per Activation instruction)
    act(Dv, Dv, "Square", accum_out=acc_mse_col)
    # previous chunk's rot accumulate lands in the same ACT lull
    pre_cast()

    # ---- S = M^T M (f16): order [s00,s11,s22,s01,s02,s12] ----
    # diag from the ACT-computed channel squares (SQ = X*X): 2 DVE adds
    tmp16 = H()
    SQ = tiles["SQ"]
    add(Sm[:, 0:3, 0:L], SQ[:, 0:3, 0:L], SQ[:, 3:6, 0:L])
    add(Sm[:, 0:3, 0:L], Sm[:, 0:3, 0:L], SQ[:, 6:9, 0:L])
    for oi, (ia, ib) in enumerate((((0, 3, 6), (1, 4, 7)),
                                   ((0, 3, 6), (2, 5, 8)),
                                   ((1, 4, 7), (2, 5, 8)))):
        so = Sm[:, 3 + oi, 0:L]
        mul(so, x[ia[0]], x[ib[0]])
        mul(tmp16, x[ia[1]], x[ib[1]]); add(so, so, tmp16)
        mul(tmp16, x[ia[2]], x[ib[2]]); add(so, so, tmp16)
    # tr and normalization scale q = 3/tr (ln domain)
    tr16 = H()
    add(tr16, Sm[:, 0, 0:L], Sm[:, 1, 0:L]); add(tr16, tr16, Sm[:, 2, 0:L])
    vs(tr16, tr16, 6e-5, "max")
    lnt = A(); act(lnt, tr16, "Ln")
    q16 = H(); act(q16, lnt, "Exp", scale=-1.0, bias=_c(nc, LN3))
    regs16.free(tr16)
    fill(5)
    nc.vector.tensor_tensor(out=Sm[:, :, 0:L], in0=Sm[:, :, 0:L],
                            in1=bc(q16, 6), op=Alu.mult)
    regs16.free(q16)

    # Pg = exp(lnad + 1.5*(ln3 - lnt));  dets = Pg^2
    lnPg = A()
    stt(lnPg, lnt, -1.5, lnad, "mult", "add")
    regs.free(lnad)
    Pg16 = H(); act(Pg16, lnPg, "Exp", scale=1.0, bias=_c(nc, 1.5 * LN3))
    # dets = Pg^2 in full fp32 via a scale-2 Exp: it feeds the cancelling
    # detK/arg computation, where f16 noise is amplified by 1/(2 p^3)
    dets = A(); act(dets, lnPg, "Exp", scale=2.0, bias=_c(nc, 3.0 * LN3))
    regs.free(lnPg)

    # ---- tr(S^2) fp32 from normalized f16 S ----
    act(QS[:, :, 0:L], Sm[:, :, 0:L], "Square")
    fill(3)
    u1, u2 = A(), A()
    add(u1, QS[:, 0, 0:L], QS[:, 1, 0:L]); add(u1, u1, QS[:, 2, 0:L])
    add(u2, QS[:, 3, 0:L], QS[:, 4, 0:L]); add(u2, u2, QS[:, 5, 0:L])
    trS2 = A()
    stt(trS2, u2, 2.0, u1, "mult", "add")
    # p and 1/(2 p^3) via ln/exp; p itself is f16 (feeds the f16 tail)
    trK2, ip3h = u1, u2                    # reuse u1/u2 slots
    p = H()
    vs(trK2, trS2, -3.0, "add", 1e-30, "max")
    lnk = A(); act(lnk, trK2, "Ln")
    act(p, lnk, "Exp", scale=0.5, bias=_c(nc, -0.5 * LN6))
    act(ip3h, lnk, "Exp", scale=-1.5, bias=_c(nc, 1.5 * LN6 - LN2))
    regs.free(lnk)
    # arg
    detK, arg = A(), A()
    stt(detK, trS2, 0.5, dets, "mult", "add")
    vs(detK, detK, -2.5, "add")
    fill(3)
    vs(ip3h, ip3h, 1e30, "min")
    mul(arg, detK, ip3h)
    vs(arg, arg, 1.0, "min", -1.0, "max")
    regs.free(detK, trS2, dets, u2)   # u2 == ip3h
    # ---- th3 = acos(arg); poly arithmetic in f16 (values are O(1) and
    #      smooth), fp32 kept only for the cancelling 1-y subtraction ----
    y, om32 = A(), A()
    act(y, arg, "Abs")
    y16, sg16, h = H(), H(), H()
    act(y16, arg, "Abs")
    act(sg16, arg, "Sign")
    fill(2)
    vs(om32, y, -1.0, "mult", 1.0, "add")
    lnom = arg                                   # reuse slot
    act(lnom, om32, "Ln")
    om16 = H()
    act(om16, lnom, "Exp", scale=0.5)            # sqrt(1-y), f16
    vs(h, y16, ACOS_A[3], "mult", ACOS_A[2], "add")
    mul(h, h, y16); vs(h, h, ACOS_A[1], "add")
    mul(h, h, y16); vs(h, h, ACOS_A[0], "add")
    fill(3)
    mul(h, h, om16)
    th3, lin = om16, y16                         # reuse slots
    vs(lin, sg16, -np.pi / 2, "mult", np.pi / 2, "add")
    mul(th3, sg16, h); add(th3, th3, lin)
    regs.free(y, om32, arg)    # arg == lnom
    regs16.free(h, sg16)
    # ---- cos((th3+2pik)/3) ----
    z, c0p = H(), H()
    act(z, th3, "Square", scale=1.0 / 3.0)
    fill(2)
    vs(c0p, z, 1.0 / 40320.0, "mult", -1.0 / 720.0, "add")
    mul(c0p, c0p, z); vs(c0p, c0p, 1.0 / 24.0, "add")
    mul(c0p, c0p, z); vs(c0p, c0p, -0.5, "add")
    mul(c0p, c0p, z); vs(c0p, c0p, 1.0, "add")
    regs16.free(z, om16, y16)   # om16 == th3, y16 == lin
    s032 = A()
    act(s032, c0p, "Square")
    uc1, c1p, c2p, s016 = H(), H(), H(), H()
    vs(uc1, c0p, -0.5, "mult")
    fill(2)
    vs(s032, s032, -1.0, "mult", 1.0, "add")
    vs(s032, s032, 0.0, "max")
    lns = A(); act(lns, s032, "Ln")
    act(s016, lns, "Exp", scale=0.5)
    regs.free(lns, s032)
    fill(3)
    vs(s016, s016, HALF_SQRT3, "mult")
    sub(c1p, uc1, s016)
    add(c2p, uc1, s016)
    regs16.free(s016, uc1)
    # ---- lambda_k, g_k = sqrt(lambda_k) (f16) ----
    tp = H()
    vs(tp, p, 2.0, "mult")
    regs16.free(p)
    lam, g, lnls = [], [], []
    for ck in (c0p, c1p, c2p):
        lk, lnl = H(), A()
        mul(lk, tp, ck)
        vs(lk, lk, 1.0, "add", 6.5e-5, "max")
        act(lnl, lk, "Ln")
        lam.append(lk); lnls.append(lnl)
    for lnl in lnls:
        gk = H()
        act(gk, lnl, "Exp", scale=0.5)
        g.append(gk)
        regs.free(lnl)
    regs16.free(tp, c0p, c1p, c2p)
    fill(5)
    g01, ssig16, s2i16, tmp2 = H(), H(), H(), H()
    add(g01, g[0], g[1])
    add(ssig16, g01, g[2])
    mul(s2i16, g[0], g[1]); mul(tmp2, g[2], g01); add(s2i16, s2i16, tmp2)
    regs16.free(g01, tmp2, *g)
    for lk in lam:
        regs16.free(lk)
    fill(3)

    # ---- N = ssig*S + Pg*I (f16, batched) ----
    fill(3)
    nc.vector.tensor_tensor(out=Nm[:, :, 0:L], in0=Sm[:, :, 0:L],
                            in1=bc(ssig16, 6), op=Alu.mult)
    nc.vector.tensor_tensor(out=Nm[:, 0:3, 0:L], in0=Nm[:, 0:3, 0:L],
                            in1=bc(Pg16, 3), op=Alu.add)
    regs16.free(Pg16, ssig16)
    # A' diagonal (Am = S_diag + s2)
    nc.vector.tensor_tensor(out=Am[:, :, 0:L], in0=Sm[:, 0:3, 0:L],
                            in1=bc(s2i16, 3), op=Alu.add)
    regs16.free(s2i16)
    # ---- adj(N) (f16) ----
    n00, n11, n22 = (Nm[:, i, 0:L] for i in range(3))
    n01, n02, n12 = (Nm[:, i, 0:L] for i in range(3, 6))
    aj = [H() for _ in range(6)]
    a00, a01, a02, a11, a12, a22 = aj

    def cof(o, a, b, c, dd):
        mul(o, a, b); mul(tmp16, c, dd); sub(o, o, tmp16)

    cof(a00, n11, n22, n12, n12)
    cof(a01, n02, n12, n01, n22)
    cof(a02, n01, n12, n02, n11)
    cof(a11, n00, n22, n02, n02)
    cof(a12, n01, n02, n00, n12)
    cof(a22, n00, n11, n01, n01)
    # ---- w = sign/(sqrt(m)*detN), with detN = row0(N).adj_row0 so it is
    #      exactly consistent with the f16 N/adj used for T1 (an eigenvalue-
    #      based detN amplifies f16 lambda noise by ssig/Pg on near-singular
    #      rows).  The ACT Ln/Exp latency hides under the T1 block below. ----
    detn, t_ = H(), A()
    mul(detn, n00, a00)
    mul(tmp16, n01, a01); add(detn, detn, tmp16)
    mul(tmp16, n02, a02); add(detn, detn, tmp16)
    act(t_, detn, "Ln")
    regs16.free(detn)
    # ---- T1 = A' adjN (f16); rows of A': (b0,s01,s02),(s01,b1,s12),(s02,s12,b2)
    b0, b1, b2 = (Am[:, i, 0:L] for i in range(3))
    s01p, s02p, s12p = Sm[:, 3, 0:L], Sm[:, 4, 0:L], Sm[:, 5, 0:L]

    def mm3(o, r0, r1, r2, k0, k1, k2):
        mul(o, r0, k0)
        mul(tmp16, r1, k1); add(o, o, tmp16)
        mul(tmp16, r2, k2); add(o, o, tmp16)

    mm3(T1m[:, 0, 0:L], b0, s01p, s02p, a00, a01, a02)
    mm3(T1m[:, 1, 0:L], b0, s01p, s02p, a01, a11, a12)
    mm3(T1m[:, 2, 0:L], b0, s01p, s02p, a02, a12, a22)
    mm3(T1m[:, 3, 0:L], s01p, b1, s12p, a01, a11, a12)
    mm3(T1m[:, 4, 0:L], s01p, b1, s12p, a02, a12, a22)
    mm3(T1m[:, 5, 0:L], s02p, s12p, b2, a02, a12, a22)
    regs16.free(*aj)
    regs16.free(tmp16)
    # finish w while T1 wraps up: wmag = exp(-(ln detN + 0.5 ln m) ...)
    stt(t_, lnt, 0.5, t_, "mult", "add")
    vs(t_, t_, float(np.log(EPS_W) + 0.5 * LN3), "max")
    wmag16 = H()
    act(wmag16, t_, "Exp", scale=-1.0, bias=_c(nc, 0.5 * LN3))
    regs.free(t_, lnt)
    # w16 = sign(det) * wmag  (into the double-buffered W16 plane)
    mul(W16[:, 0:L], wmag16, sgd16)
    regs16.free(wmag16, sgd16)


def _make_B(nc, env, k, T, tiles, acc_rot_col, last=False):
    """Phase B of chunk k: T2 = w*T1, R = M*T2 as a list of single-op DVE
    thunks (drained as filler during chunk k+1's phase A).  Three of the
    nine R outputs run on GpSimd (emitted with the T2 thunk).  The tail
    (clamp + dR sub) also runs on GpSimd, and the ACT accumulate thunk is
    emitted early in chunk k+2.  For the last chunk everything stays on
    DVE (it is idle during the pipeline drain)."""
    mybir = env["mybir"]
    Alu = mybir.AluOpType
    Act = mybir.ActivationFunctionType
    L = 2 * T
    X, T1m, Nm, W16, RT, BT, PT = (tiles[n] for n in
                                   ("X", "T1m", "Nm", "W16", "RT", "BT",
                                    "PT"))
    x = [X[:, c, 0:L] for c in range(9)]
    tail_eng = nc.vector if last else nc.gpsimd

    def mul(o, a, b):
        nc.vector.tensor_tensor(out=o, in0=a, in1=b, op=Alu.mult)

    def add(o, a, b):
        nc.vector.tensor_tensor(out=o, in0=a, in1=b, op=Alu.add)

    def bc(plane, n):
        return bass_mod.AP(tensor=plane.tensor, offset=plane.offset,
                           ap=[plane.ap[0], [0, n], plane.ap[1]])

    T2f = Nm
    t00, t01, t02 = T2f[:, 0, 0:L], T2f[:, 1, 0:L], T2f[:, 2, 0:L]
    t11, t12, t22 = T2f[:, 3, 0:L], T2f[:, 4, 0:L], T2f[:, 5, 0:L]
    T2 = [[t00, t01, t02], [t01, t11, t12], [t02, t12, t22]]
    tmpr = BT[:, 0:L]
    tmpp = PT[:, 0:L]

    def emit_out(i, j, eng, tmp):
        o = RT[:, 3 * i + j, 0:L]
        eng.tensor_tensor(out=o, in0=x[3 * i], in1=T2[0][j], op=Alu.mult)
        eng.tensor_tensor(out=tmp, in0=x[3 * i + 1], in1=T2[1][j],
                          op=Alu.mult)
        eng.tensor_tensor(out=o, in0=o, in1=tmp, op=Alu.add)
        eng.tensor_tensor(out=tmp, in0=x[3 * i + 2], in1=T2[2][j],
                          op=Alu.mult)
        eng.tensor_tensor(out=o, in0=o, in1=tmp, op=Alu.add)

    def t2_op():
        # T2 = w*T1 in f16 (Nm tile is dead after adj; reuse it)
        nc.vector.tensor_tensor(out=T2f[:, :, 0:L], in0=T1m[:, :, 0:L],
                                in1=bc(W16[:, 0:L], 6), op=Alu.mult)

    dve = [t2_op]
    cols = (0, 1, 2)
    for i in range(3):
        for j in cols:
            o = RT[:, 3 * i + j, 0:L]
            dve.append(lambda o=o, i=i, j=j: mul(o, x[3 * i], T2[0][j]))
            dve.append(lambda i=i, j=j: mul(tmpr, x[3 * i + 1], T2[1][j]))
            dve.append(lambda o=o: add(o, o, tmpr))
            dve.append(lambda i=i, j=j: mul(tmpr, x[3 * i + 2], T2[2][j]))
            dve.append(lambda o=o: add(o, o, tmpr))

    def pool_tail():
        # clamp junk rows, dR = R_pred - R_target in place (off DVE)
        tail_eng.tensor_scalar(out=RT[:, :, 0:L], in0=RT[:, :, 0:L],
                               scalar1=8.0, scalar2=-8.0,
                               op0=Alu.min, op1=Alu.max)
        tail_eng.tensor_tensor(out=RT[:, :, 0:T], in0=RT[:, :, 0:T],
                               in1=RT[:, :, T:L], op=Alu.subtract)

    def act_accum():
        bias0 = env["bias0"]
        nc.scalar.activation(out=RT[:, :, 0:T], in_=RT[:, :, 0:T],
                             func=Act.Square, bias=bias0[:, 0:1], scale=1.0,
                             accum_out=acc_rot_col)

    return dve, pool_tail, act_accum


def _build_nc():
    global bass_mod
    import concourse.bass as bass
    import concourse.tile as tile
    from concourse import mybir
    bass_mod = bass

    f32 = mybir.dt.float32
    f16 = mybir.dt.float16
    nc = bass.Bass()
    pred = nc.dram_tensor("pred", [ROWS_PER_CORE, 9], f32, kind="ExternalInput")
    targ = nc.dram_tensor("target", [ROWS_PER_CORE, 9], f32, kind="ExternalInput")
    out = nc.dram_tensor("partials", [P, 2 * NCHUNK], f32, kind="ExternalOutput")

    predv = pred.rearrange("(p n) c -> p n c", p=P)    # [128, 1024, 9]
    targv = targ.rearrange("(p n) c -> p n c", p=P)
    row0 = np.cumsum((0,) + CHUNKS)                    # chunk row offsets

    with tile.TileContext(nc) as tc:
        with (
            tc.tile_pool(name="raw", bufs=1) as rawp,
            tc.tile_pool(name="pl", bufs=1) as pl,
            tc.tile_pool(name="acc", bufs=1) as accp,
        ):
            acc = accp.tile([P, 2 * NCHUNK], f32, tag="acc")
            bias0 = accp.tile([P, 1], f32, tag="bias0")
            nc.vector.memset(bias0, 0.0)
            _CONST_STATE[(id(nc), "pool")] = accp
            env = {
                "mybir": mybir,
                "regs": Regs(pl, f32, prefix="v"),
                "regs16": Regs(pl, f16, prefix="h"),
                "bias0": bias0,
            }

            raw_cache = {}

            def raw_tiles(k):
                if k not in raw_cache:
                    raw_cache[k] = (
                        rawp.tile([P, TMAX * 9], f32, tag=f"praw{k % 2}",
                                  name=f"praw{k % 2}"),
                        rawp.tile([P, TMAX * 9], f32, tag=f"traw{k % 2}",
                                  name=f"traw{k % 2}"))
                return raw_cache[k]

            def chunk_tiles(k):
                j = k % 2
                return {
                    "X": pl.tile([P, 9, 2 * TMAX], f16, tag=f"X{j}", name=f"X{j}"),
                    "D": pl.tile([P, 9, TMAX], f16, tag="D", name="D"),
                    "Sm": pl.tile([P, 6, 2 * TMAX], f16, tag=f"Sm{j}", name=f"Sm{j}"),
                    "QS": pl.tile([P, 6, 2 * TMAX], f32, tag="QS", name="QS"),
                    "Am": pl.tile([P, 3, 2 * TMAX], f16, tag="Am", name="Am"),
                    "T1m": pl.tile([P, 6, 2 * TMAX], f16, tag=f"T1m{j}", name=f"T1m{j}"),
                    "Nm": pl.tile([P, 6, 2 * TMAX], f16, tag=f"Nm{j}", name=f"Nm{j}"),
                    "W16": pl.tile([P, 2 * TMAX], f16, tag=f"W16{j}", name=f"W16{j}"),
                    "RT": pl.tile([P, 9, 2 * TMAX], f16, tag=f"RT{j}", name=f"RT{j}"),
                    "BT": pl.tile([P, 2 * TMAX], f16, tag=f"BT{j}", name=f"BT{j}"),
                    "PT": pl.tile([P, 2 * TMAX], f16, tag=f"PT{j}", name=f"PT{j}"),
                    "SQ": pl.tile([P, 9, 2 * TMAX], f16, tag="SQ", name="SQ"),
                }

            def dma_chunk(k):
                prw, trw = raw_tiles(k)
                t0, t1 = row0[k], row0[k + 1]
                n = (t1 - t0) * 9
                nc.sync.dma_start(out=prw[:, 0:n], in_=predv[:, t0:t1, :])
                nc.sync.dma_start(out=trw[:, 0:n], in_=targv[:, t0:t1, :])

            dma_chunk(0)
            dma_chunk(1)

            pending = []        # phase-B DVE thunks of chunk k-1
            tail_q = []         # (pool_tail, act_accum) of chunk k-1
            accum_slot = []     # ACT accumulates to emit at next post_cast

            def filler(n):
                for _ in range(min(n, len(pending))):
                    pending.pop(0)()

            for k, T in enumerate(CHUNKS):
                praw, traw = raw_tiles(k)
                tiles = chunk_tiles(k)

                def pre_cast():
                    while accum_slot:
                        accum_slot.pop(0)()

                def post_cast(k=k):
                    if k + 2 < NCHUNK:
                        dma_chunk(k + 2)

                _emit_A(nc, env, k, T, praw, traw, tiles,
                        acc[:, k:k + 1], filler, pre_cast, post_cast)
                # drain the rest of chunk k-1's phase B, then its GpSimd tail;
                # its ACT accumulate goes early into chunk k+1
                filler(len(pending))
                if tail_q:
                    pool_tail, act_accum = tail_q.pop(0)
                    pool_tail()
                    accum_slot.append(act_accum)
                dve, pool_tail, act_accum = _make_B(
                    nc, env, k, T, tiles, acc[:, NCHUNK + k:NCHUNK + k + 1],
                    last=(k == NCHUNK - 1))
                pending = dve
                tail_q.append((pool_tail, act_accum))

            # drain the pipeline: B of the last chunk, tails, accumulates
            filler(len(pending))
            while accum_slot:
                accum_slot.pop(0)()
            while tail_q:
                pool_tail, act_accum = tail_q.pop(0)
                pool_tail()
                act_accum()
            nc.sync.dma_start(out=out[:, :], in_=acc)
    return nc


def _elide_implied_waits(nc):
    """Drop semaphore waits already implied by program order or transitively
    by earlier waits (vector-clock propagation).  Tile's per-instruction wait
    emission is not transitively minimal, and walrus can encode only one sync
    wait on Activation/DMA instructions (and ~4 on control instructions), so
    the redundant waits both break codegen and waste sequencer time.

    Model: each semaphore s carries a snapshot VC at every increment value;
    an engine's observed VC advances via its own instruction stream and via
    the snapshots of the waits it executes.  A wait (s >= v) is dropped iff
    the engine's observed VC already dominates it.  Unknown update modes
    disable elision for that semaphore (conservative).
    """
    join = lambda a, b: {k: max(a.get(k, 0), b.get(k, 0)) for k in set(a) | set(b)}
    sem_val = {}        # sem name -> current value
    sem_snap = {}       # sem name -> list of (value, VC) snapshots
    eng_vc = {}         # engine name -> observed VC
    unsafe = set()      # sems with non-increment updates
    n_drop = 0
    for f in nc.m.functions:
        for bb in f.blocks:
            for ins in bb.instructions:
                eng = str(ins.engine)
                vc = dict(eng_vc.get(eng, {}))
                si = ins.sync_info
                waits = list(si.on_wait) if si is not None and si.on_wait else []
                kept = []
                for w in waits:
                    s, v = w.ant_name, w.wait_value
                    if w.wait_mode != "sem-ge-imm" or s in unsafe:
                        kept.append(w)
                        continue
                    if vc.get(s, 0) >= v:
                        n_drop += 1
                        continue
                    if sem_val.get(s, 0) < v:
                        # increment not yet seen in emission order; keep and
                        # learn nothing (conservative)
                        kept.append(w)
                        continue
                    kept.append(w)
                    snap = {}
                    for sv, svc in sem_snap.get(s, ()):
                        if sv <= v:
                            snap = svc
                        else:
                            break
                    vc = join(vc, snap)
                    vc[s] = max(vc.get(s, 0), v)
                if si is not None and len(kept) != len(waits):
                    si.on_wait = kept
                # apply this instruction's increments
                ups = si.on_update if si is not None and si.on_update else []
                for u in ups:
                    s = u.ant_name
                    if u.update_mode not in ("sem-inc", "sem-add-imm"):
                        unsafe.add(s)
                        continue
                    nv = sem_val.get(s, 0) + (u.update_value or 1)
                    sem_val[s] = nv
                    lst = sem_snap.setdefault(s, [])
                    prev = lst[-1][1] if lst else {}
                    lst.append((nv, join(prev, vc)))
                    # Engine-sem increments fire when the instruction
                    # completes, and the engine is sequential, so later
                    # instructions on this engine observe them.  DMA-queue
                    # increments fire asynchronously at transfer completion:
                    # the issuing engine must NOT absorb those.
                    if "DMA" not in s:
                        vc[s] = max(vc.get(s, 0), nv)
                eng_vc[eng] = vc
    return n_drop


def _spill_excess_waits(nc):
    """walrus encodes at most ONE sync wait on real engine instructions
    (Activation/DVE/DMA); the Tile scheduler can leave more after
    cross-engine reordering.  Keep one wait on the instruction and hoist
    the rest onto preceding InstEventSemaphore control instructions
    (which accept 2 waits each)."""
    from concourse import mybir
    n_spill = 0
    for f in nc.m.functions:
        for bb in f.blocks:
            out = []
            for ins in bb.instructions:
                si = ins.sync_info
                waits = list(si.on_wait) if si is not None and si.on_wait else []
                is_engine_op = bool(ins.ins) or bool(ins.outs)
                if len(waits) > 1 and is_engine_op and \
                        not isinstance(ins, mybir.InstEventSemaphore):
                    spill = waits[:-1]
                    si.on_wait = waits[-1:]
                    while spill:
                        grp, spill = spill[:2], spill[2:]
                        n_spill += 1
                        ev = mybir.InstEventSemaphore(
                            name=f"wspill_{n_spill}", engine=ins.engine,
                            ins=[], outs=[],
                            sync_info=mybir.SyncInfo(on_wait=grp,
                                                     on_update=[]))
                        out.append(ev)
                out.append(ins)
            bb.instructions = out
    return n_spill


_NC_CACHE = None


def kernel(pred: np.ndarray, target: np.ndarray) -> np.ndarray:
    global _NC_CACHE
    from concourse.bass_utils import run_bass_kernel_spmd

    pred = np.ascontiguousarray(np.asarray(pred, dtype=np.float32))
    target = np.ascontiguousarray(np.asarray(target, dtype=np.float32))
    assert pred.shape == (B, 9) and target.shape == (B, 9)

    if _NC_CACHE is None:
        _NC_CACHE = _build_nc()
        _elide_implied_waits(_NC_CACHE)
        _spill_excess_waits(_NC_CACHE)
    nc = _NC_CACHE

    ps = pred.reshape(N_CORES, ROWS_PER_CORE, 9)
    ts = target.reshape(N_CORES, ROWS_PER_CORE, 9)
    in_maps = [{"pred": ps[i], "target": ts[i]} for i in range(N_CORES)]
    res = run_bass_kernel_spmd(nc, in_maps, core_ids=list(range(N_CORES)))
    globals()["_LAST_RESULT"] = res

    mse_sum = 0.0
    rot_sum = 0.0
    for r in res.results:
        part = np.asarray(r["partials"], dtype=np.float64)
        mse_sum += part[:, :NCHUNK].sum()
        rot_sum += part[:, NCHUNK:].sum()
    n = float(B * 9)
    return np.asarray(np.float32(mse_sum / n + 0.5 * (rot_sum / n)))


# revision 37
# speedup vs baseline: 1.3378x; 1.0018x over previous
"""CustomPoseLoss Trainium2 kernel.

loss = mean((pred-target)^2) + 0.5 * mean((R(pred)-R(target))^2)
where R(M) = sign(det M) * polar(M) for each 3x3 matrix (row of 9).

Implementation: closed-form polar decomposition per row, fully vectorized as
channel-plane arithmetic:
  S = M^T M, normalized by tr(S)/3; eigenvalues of S via Cardano
  (acos/cos evaluated as polynomials so only the ln/exp LUT set is needed);
  W^-1 = (S + s2 I) adj(N) / det(N) with N = ssig*S + Pg*I  (Cayley-Hamilton
  inverse-sqrt);  R = sign(det) * M W^-1 / sqrt(m).
det(N) is formed from the eigenvalue product (positive, cancellation-free)
and clamped, so near-singular rows stay bounded.

Engine plan: heavy products in f16 on DVE (2x mode); scalar chain fp32 on
DVE+ACT; mse-sub / clamp / dR-sub on GpSimd (off critical path).  Chunks are
software-pipelined: chunk k's tail (T2, R = M*T2) is emitted as filler inside
chunk k+1's scalar-chain stalls, so the Vector engine never waits on ACT legs.

Sharding: pure data parallel over 8 cores; each core reduces its shard to
[128, 2*NCHUNK] partial sums (mse, rot), host combines in float64.
"""

import numpy as np

B = 1048576
N_CORES = 8
ROWS_PER_CORE = B // N_CORES          # 131072
P = 128
ROWS_PER_PART = ROWS_PER_CORE // P    # 1024
CHUNKS = (256, 256, 256, 256)         # rows per partition per chunk
NCHUNK = len(CHUNKS)
TMAX = max(CHUNKS)
EPS_D = 1e-5

ACOS_A = (1.5707288, -0.2121144, 0.0742610, -0.0187293)   # A&S 4.4.45
HALF_SQRT3 = 0.8660254037844386

LN3 = float(np.log(3.0))
LN6 = float(np.log(6.0))
LN2 = float(np.log(2.0))
EPS_W = 6e-3


class Regs:
    """[128, 2*TMAX] plane slots with explicit reuse (SBUF is capped)."""

    def __init__(self, pool, dtype, prefix="v", shape=None):
        self.pool = pool
        self.dtype = dtype
        self.prefix = prefix
        self.shape = shape or [P, 2 * TMAX]
        self.free_tags = []
        self.n = 0
        self.tag_of = {}

    def alloc(self, width=None):
        if self.free_tags:
            tag = self.free_tags.pop()
        else:
            self.n += 1
            tag = f"{self.prefix}{self.n}"
        tl = self.pool.tile(self.shape, self.dtype, tag=tag, name=tag)
        if width is not None:
            tl = tl[:, 0:width]
        self.tag_of[id(tl)] = tag
        return tl

    def free(self, *tiles):
        for tl in tiles:
            self.free_tags.append(self.tag_of.pop(id(tl)))


_CONST_STATE = {}
bass_mod = None


def _c(nc, v):
    """[P,1] fp32 constant AP, DVE-memset once (keeps ACT single-wait)."""
    key = float(np.float32(v))
    consts = _CONST_STATE.setdefault(id(nc), {})
    if key not in consts:
        pool = _CONST_STATE[(id(nc), "pool")]
        from concourse import mybir
        t = pool.tile([P, 1], mybir.dt.float32, tag=f"c{len(consts)}")
        nc.vector.memset(t, key)
        consts[key] = t
    return consts[key][:, 0:1]


def _emit_A(nc, env, k, T, praw, traw, tiles, acc_mse_col, fill, pre_cast,
            post_cast):
    """Phase A of chunk k: everything through T1m and w16.

    `fill(n)` emits up to n pending phase-B DVE ops from the previous chunk
    at known ACT-wait stall points.  `pre_cast()` emits the chunk k-2 ACT
    accumulate (whose Pool wait transitively covers the casts' X-tile WAR
    wait — walrus allows only one sync wait per Activation instruction);
    `post_cast()` emits the chunk k+2 DMA prefetch.
    """
    mybir = env["mybir"]
    regs, regs16, bias0 = env["regs"], env["regs16"], env["bias0"]
    Alu = mybir.AluOpType
    Act = mybir.ActivationFunctionType
    L = 2 * T
    X, D, Sm, QS, Am, T1m, Nm, W16 = (tiles[n] for n in
                                      ("X", "D", "Sm", "QS", "Am", "T1m",
                                       "Nm", "W16"))

    def mul(o, a, b):
        nc.vector.tensor_tensor(out=o, in0=a, in1=b, op=Alu.mult)

    def add(o, a, b):
        nc.vector.tensor_tensor(out=o, in0=a, in1=b, op=Alu.add)

    def sub(o, a, b):
        nc.vector.tensor_tensor(out=o, in0=a, in1=b, op=Alu.subtract)

    def vs(o, a, s1, op0, s2=None, op1=None):
        if s2 is None:
            nc.vector.tensor_scalar(out=o, in0=a, scalar1=float(s1),
                                    scalar2=None, op0=getattr(Alu, op0))
        else:
            nc.vector.tensor_scalar(out=o, in0=a, scalar1=float(s1),
                                    scalar2=float(s2), op0=getattr(Alu, op0),
                                    op1=getattr(Alu, op1))

    def stt(o, a, s, b, op0, op1):
        nc.vector.scalar_tensor_tensor(out=o, in0=a, scalar=float(s), in1=b,
                                       op0=getattr(Alu, op0),
                                       op1=getattr(Alu, op1))

    def act(o, a, func, scale=1.0, bias=None, accum_out=None):
        if func == "Copy":
            nc.scalar.activation(out=o, in_=a, func=Act.Copy, bias=0.0,
                                 scale=float(scale), accum_out=accum_out)
        else:
            nc.scalar.activation(out=o, in_=a, func=getattr(Act, func),
                                 bias=bias0[:, 0:1] if bias is None else bias,
                                 scale=float(scale), accum_out=accum_out)

    def bc(plane, n):
        # broadcast [P, L] plane across n sub-planes -> [P, n, L]
        return bass_mod.AP(tensor=plane.tensor, offset=plane.offset,
                           ap=[plane.ap[0], [0, n], plane.ap[1]])

    A = lambda: regs.alloc(L)       # fp32 [P, L] planes
    H = lambda: regs16.alloc(L)     # f16 [P, L] planes

    # ---- cast+deinterleave both inputs into X[P, 9, 2T] (f16) ----
    rvp = praw[:, 0:T * 9].rearrange("p (n c) -> p n c", c=9)
    rvt = traw[:, 0:T * 9].rearrange("p (n c) -> p n c", c=9)
    xin_p = bass_mod.AP(tensor=rvp.tensor, offset=rvp.offset,
                        ap=[rvp.ap[0], rvp.ap[2], rvp.ap[1]])
    xin_t = bass_mod.AP(tensor=rvt.tensor, offset=rvt.offset,
                        ap=[rvt.ap[0], rvt.ap[2], rvt.ap[1]])
    act(X[:, :, 0:T], xin_p, "Copy")
    act(X[:, :, T:L], xin_t, "Copy")
    post_cast()
    x = [X[:, c, 0:L] for c in range(9)]        # [P, L] f16 unit-stride

    # ---- channel squares for the S diagonal (ACT; overlaps DVE det/S) ----
    act(tiles["SQ"][:, :, 0:L], X[:, :, 0:L], "Square")

    # ---- mse sub on GpSimd (off critical path) ----
    Dv = D[:, :, 0:T]
    nc.gpsimd.tensor_tensor(out=Dv, in0=X[:, :, 0:T], in1=X[:, :, T:L],
                            op=Alu.subtract)

    fill(6)

    # ---- det(M) f16 from X planes (emitted first: its ACT consumers
    #      run during the S block) ----
    cA16, cB16, det16 = H(), H(), H()
    mul(cA16, x[4], x[8]); mul(cB16, x[5], x[7]); sub(cA16, cA16, cB16)
    mul(det16, x[0], cA16)
    mul(cA16, x[5], x[6]); mul(cB16, x[3], x[8]); sub(cA16, cA16, cB16)
    mul(cA16, x[1], cA16); add(det16, det16, cA16)
    mul(cA16, x[3], x[7]); mul(cB16, x[4], x[6]); sub(cA16, cA16, cB16)
    mul(cA16, x[2], cA16); add(det16, det16, cA16)
    sgd16 = H(); act(sgd16, det16, "Sign")
    lnad, ad32 = A(), A()
    act(ad32, det16, "Abs")
    act(lnad, ad32, "Ln")
    regs.free(ad32)
    regs16.free(cA16, cB16, det16)
    # mse Square-accum here: its DVE wait is covered by the det ACT ops
    # above (walrus allows only one sync wait per Activation instruction)
    act(Dv, Dv, "Square", accum_out=acc_mse_col)
    # previous chunk's rot accumulate lands in the same ACT lull
    pre_cast()

    # ---- S = M^T M (f16): order [s00,s11,s22,s01,s02,s12] ----
    # off-diag first (pure DVE); the diag adds wait on the ACT SQ Square,
    # which finishes while the off-diag products run
    tmp16 = H()
    SQ = tiles["SQ"]
    for oi, (ia, ib) in enumerate((((0, 3, 6), (1, 4, 7)),
                                   ((0, 3, 6), (2, 5, 8)),
                                   ((1, 4, 7), (2, 5, 8)))):
        so = Sm[:, 3 + oi, 0:L]
        mul(so, x[ia[0]], x[ib[0]])
        mul(tmp16, x[ia[1]], x[ib[1]]); add(so, so, tmp16)
        mul(tmp16, x[ia[2]], x[ib[2]]); add(so, so, tmp16)
    # diag from the ACT-computed channel squares (SQ = X*X): 2 DVE adds
    add(Sm[:, 0:3, 0:L], SQ[:, 0:3, 0:L], SQ[:, 3:6, 0:L])
    add(Sm[:, 0:3, 0:L], Sm[:, 0:3, 0:L], SQ[:, 6:9, 0:L])
    # tr and normalization scale q = 3/tr (ln domain)
    tr16 = H()
    add(tr16, Sm[:, 0, 0:L], Sm[:, 1, 0:L]); add(tr16, tr16, Sm[:, 2, 0:L])
    vs(tr16, tr16, 6e-5, "max")
    lnt = A(); act(lnt, tr16, "Ln")
    q16 = H(); act(q16, lnt, "Exp", scale=-1.0, bias=_c(nc, LN3))
    regs16.free(tr16)
    fill(5)
    nc.vector.tensor_tensor(out=Sm[:, :, 0:L], in0=Sm[:, :, 0:L],
                            in1=bc(q16, 6), op=Alu.mult)
    regs16.free(q16)

    # Pg = exp(lnad + 1.5*(ln3 - lnt));  dets = Pg^2
    lnPg = A()
    stt(lnPg, lnt, -1.5, lnad, "mult", "add")
    regs.free(lnad)
    Pg16 = H(); act(Pg16, lnPg, "Exp", scale=1.0, bias=_c(nc, 1.5 * LN3))
    # dets = Pg^2 in full fp32 via a scale-2 Exp: it feeds the cancelling
    # detK/arg computation, where f16 noise is amplified by 1/(2 p^3)
    dets = A(); act(dets, lnPg, "Exp", scale=2.0, bias=_c(nc, 3.0 * LN3))
    regs.free(lnPg)

    # ---- tr(S^2) fp32 from normalized f16 S ----
    act(QS[:, :, 0:L], Sm[:, :, 0:L], "Square")
    fill(3)
    u1, u2 = A(), A()
    add(u1, QS[:, 0, 0:L], QS[:, 1, 0:L]); add(u1, u1, QS[:, 2, 0:L])
    add(u2, QS[:, 3, 0:L], QS[:, 4, 0:L]); add(u2, u2, QS[:, 5, 0:L])
    trS2 = A()
    stt(trS2, u2, 2.0, u1, "mult", "add")
    # p and 1/(2 p^3) via ln/exp; p itself is f16 (feeds the f16 tail)
    trK2, ip3h = u1, u2                    # reuse u1/u2 slots
    p = H()
    vs(trK2, trS2, -3.0, "add", 1e-30, "max")
    lnk = A(); act(lnk, trK2, "Ln")
    act(p, lnk, "Exp", scale=0.5, bias=_c(nc, -0.5 * LN6))
    act(ip3h, lnk, "Exp", scale=-1.5, bias=_c(nc, 1.5 * LN6 - LN2))
    regs.free(lnk)
    # arg
    detK, arg = A(), A()
    stt(detK, trS2, 0.5, dets, "mult", "add")
    vs(detK, detK, -2.5, "add")
    fill(3)
    vs(ip3h, ip3h, 1e30, "min")
    mul(arg, detK, ip3h)
    vs(arg, arg, 1.0, "min", -1.0, "max")
    regs.free(detK, trS2, dets, u2)   # u2 == ip3h
    # ---- th3 = acos(arg); poly arithmetic in f16 (values are O(1) and
    #      smooth), fp32 kept only for the cancelling 1-y subtraction ----
    y, om32 = A(), A()
    act(y, arg, "Abs")
    y16, sg16, h = H(), H(), H()
    act(y16, arg, "Abs")
    act(sg16, arg, "Sign")
    fill(2)
    vs(om32, y, -1.0, "mult", 1.0, "add")
    lnom = arg                                   # reuse slot
    act(lnom, om32, "Ln")
    om16 = H()
    act(om16, lnom, "Exp", scale=0.5)            # sqrt(1-y), f16
    vs(h, y16, ACOS_A[3], "mult", ACOS_A[2], "add")
    mul(h, h, y16); vs(h, h, ACOS_A[1], "add")
    mul(h, h, y16); vs(h, h, ACOS_A[0], "add")
    fill(3)
    mul(h, h, om16)
    th3, lin = om16, y16                         # reuse slots
    vs(lin, sg16, -np.pi / 2, "mult", np.pi / 2, "add")
    mul(th3, sg16, h); add(th3, th3, lin)
    regs.free(y, om32, arg)    # arg == lnom
    regs16.free(h, sg16)
    # ---- cos((th3+2pik)/3) ----
    z, c0p = H(), H()
    act(z, th3, "Square", scale=1.0 / 3.0)
    fill(2)
    vs(c0p, z, 1.0 / 40320.0, "mult", -1.0 / 720.0, "add")
    mul(c0p, c0p, z); vs(c0p, c0p, 1.0 / 24.0, "add")
    mul(c0p, c0p, z); vs(c0p, c0p, -0.5, "add")
    mul(c0p, c0p, z); vs(c0p, c0p, 1.0, "add")
    regs16.free(z, om16, y16)   # om16 == th3, y16 == lin
    s032 = A()
    act(s032, c0p, "Square")
    uc1, c1p, c2p, s016 = H(), H(), H(), H()
    vs(uc1, c0p, -0.5, "mult")
    fill(2)
    vs(s032, s032, -1.0, "mult", 1.0, "add")
    vs(s032, s032, 0.0, "max")
    lns = A(); act(lns, s032, "Ln")
    act(s016, lns, "Exp", scale=0.5)
    regs.free(lns, s032)
    fill(3)
    vs(s016, s016, HALF_SQRT3, "mult")
    sub(c1p, uc1, s016)
    add(c2p, uc1, s016)
    regs16.free(s016, uc1)
    # ---- lambda_k, g_k = sqrt(lambda_k) (f16) ----
    tp = H()
    vs(tp, p, 2.0, "mult")
    regs16.free(p)
    lam, g, lnls = [], [], []
    for ck in (c0p, c1p, c2p):
        lk, lnl = H(), A()
        mul(lk, tp, ck)
        vs(lk, lk, 1.0, "add", 6.5e-5, "max")
        act(lnl, lk, "Ln")
        lam.append(lk); lnls.append(lnl)
    for lnl in lnls:
        gk = H()
        act(gk, lnl, "Exp", scale=0.5)
        g.append(gk)
        regs.free(lnl)
    regs16.free(tp, c0p, c1p, c2p)
    fill(5)
    g01, ssig16, s2i16, tmp2 = H(), H(), H(), H()
    add(g01, g[0], g[1])
    add(ssig16, g01, g[2])
    mul(s2i16, g[0], g[1]); mul(tmp2, g[2], g01); add(s2i16, s2i16, tmp2)
    regs16.free(g01, tmp2, *g)
    for lk in lam:
        regs16.free(lk)
    fill(3)

    # ---- N = ssig*S + Pg*I (f16, batched) ----
    fill(3)
    nc.vector.tensor_tensor(out=Nm[:, :, 0:L], in0=Sm[:, :, 0:L],
                            in1=bc(ssig16, 6), op=Alu.mult)
    nc.vector.tensor_tensor(out=Nm[:, 0:3, 0:L], in0=Nm[:, 0:3, 0:L],
                            in1=bc(Pg16, 3), op=Alu.add)
    regs16.free(Pg16, ssig16)
    # A' diagonal (Am = S_diag + s2)
    nc.vector.tensor_tensor(out=Am[:, :, 0:L], in0=Sm[:, 0:3, 0:L],
                            in1=bc(s2i16, 3), op=Alu.add)
    regs16.free(s2i16)
    # ---- adj(N) (f16, into the double-buffered AJ tile: the T1 = A'*adjN
    #      block is phase B, drained during chunk k+1) ----
    AJ = tiles["AJ"]
    n00, n11, n22 = (Nm[:, i, 0:L] for i in range(3))
    n01, n02, n12 = (Nm[:, i, 0:L] for i in range(3, 6))
    a00, a01, a02, a11, a12, a22 = (AJ[:, i, 0:L] for i in range(6))

    def cof(o, a, b, c, dd):
        mul(o, a, b); mul(tmp16, c, dd); sub(o, o, tmp16)

    cof(a00, n11, n22, n12, n12)
    cof(a01, n02, n12, n01, n22)
    cof(a02, n01, n12, n02, n11)
    cof(a11, n00, n22, n02, n02)
    cof(a12, n01, n02, n00, n12)
    cof(a22, n00, n11, n01, n01)
    # ---- w = sign/(sqrt(m)*detN), with detN = row0(N).adj_row0 so it is
    #      exactly consistent with the f16 N/adj used for T1 (an eigenvalue-
    #      based detN amplifies f16 lambda noise by ssig/Pg on near-singular
    #      rows). ----
    detn, t_ = H(), A()
    mul(detn, n00, a00)
    mul(tmp16, n01, a01); add(detn, detn, tmp16)
    mul(tmp16, n02, a02); add(detn, detn, tmp16)
    act(t_, detn, "Ln")
    regs16.free(detn)
    regs16.free(tmp16)
    fill(4)
    stt(t_, lnt, 0.5, t_, "mult", "add")
    vs(t_, t_, float(np.log(EPS_W) + 0.5 * LN3), "max")
    wmag16 = H()
    act(wmag16, t_, "Exp", scale=-1.0, bias=_c(nc, 0.5 * LN3))
    regs.free(t_, lnt)
    fill(3)
    # w16 = sign(det) * wmag  (into the double-buffered W16 plane)
    mul(W16[:, 0:L], wmag16, sgd16)
    regs16.free(wmag16, sgd16)


def _make_B(nc, env, k, T, tiles, acc_rot_col, last=False):
    """Phase B of chunk k: T2 = w*T1, R = M*T2 as a list of single-op DVE
    thunks (drained as filler during chunk k+1's phase A).  Three of the
    nine R outputs run on GpSimd (emitted with the T2 thunk).  The tail
    (clamp + dR sub) also runs on GpSimd, and the ACT accumulate thunk is
    emitted early in chunk k+2.  For the last chunk everything stays on
    DVE (it is idle during the pipeline drain)."""
    mybir = env["mybir"]
    Alu = mybir.AluOpType
    Act = mybir.ActivationFunctionType
    L = 2 * T
    X, T1m, Nm, W16, RT, BT, Sm, Am, AJ = (
        tiles[n] for n in ("X", "T1m", "Nm", "W16", "RT", "BT", "Sm", "Am",
                           "AJ"))
    x = [X[:, c, 0:L] for c in range(9)]
    tail_eng = nc.vector if last else nc.gpsimd

    def mul(o, a, b):
        nc.vector.tensor_tensor(out=o, in0=a, in1=b, op=Alu.mult)

    def add(o, a, b):
        nc.vector.tensor_tensor(out=o, in0=a, in1=b, op=Alu.add)

    def bc(plane, n):
        return bass_mod.AP(tensor=plane.tensor, offset=plane.offset,
                           ap=[plane.ap[0], [0, n], plane.ap[1]])

    T2f = Nm
    t00, t01, t02 = T2f[:, 0, 0:L], T2f[:, 1, 0:L], T2f[:, 2, 0:L]
    t11, t12, t22 = T2f[:, 3, 0:L], T2f[:, 4, 0:L], T2f[:, 5, 0:L]
    T2 = [[t00, t01, t02], [t01, t11, t12], [t02, t12, t22]]
    tmpr = BT[:, 0:L]

    def t2_op():
        # T2 = w*T1 in f16 (Nm tile is dead after adj; reuse it)
        nc.vector.tensor_tensor(out=T2f[:, :, 0:L], in0=T1m[:, :, 0:L],
                                in1=bc(W16[:, 0:L], 6), op=Alu.mult)

    # T1 = A' adjN (f16); rows of A': (b0,s01,s02),(s01,b1,s12),(s02,s12,b2)
    b0, b1, b2 = (Am[:, i, 0:L] for i in range(3))
    s01p, s02p, s12p = Sm[:, 3, 0:L], Sm[:, 4, 0:L], Sm[:, 5, 0:L]
    a00, a01, a02, a11, a12, a22 = (AJ[:, i, 0:L] for i in range(6))
    T1_args = (
        (T1m[:, 0, 0:L], b0, s01p, s02p, a00, a01, a02),
        (T1m[:, 1, 0:L], b0, s01p, s02p, a01, a11, a12),
        (T1m[:, 2, 0:L], b0, s01p, s02p, a02, a12, a22),
        (T1m[:, 3, 0:L], s01p, b1, s12p, a01, a11, a12),
        (T1m[:, 4, 0:L], s01p, b1, s12p, a02, a12, a22),
        (T1m[:, 5, 0:L], s02p, s12p, b2, a02, a12, a22),
    )
    dve = []
    for (o, r0, r1, r2, k0, k1, k2) in T1_args:
        dve.append(lambda o=o, r0=r0, k0=k0: mul(o, r0, k0))
        dve.append(lambda r1=r1, k1=k1: mul(tmpr, r1, k1))
        dve.append(lambda o=o: add(o, o, tmpr))
        dve.append(lambda r2=r2, k2=k2: mul(tmpr, r2, k2))
        dve.append(lambda o=o: add(o, o, tmpr))
    dve.append(t2_op)
    cols = (0, 1, 2)
    for i in range(3):
        for j in cols:
            o = RT[:, 3 * i + j, 0:L]
            dve.append(lambda o=o, i=i, j=j: mul(o, x[3 * i], T2[0][j]))
            dve.append(lambda i=i, j=j: mul(tmpr, x[3 * i + 1], T2[1][j]))
            dve.append(lambda o=o: add(o, o, tmpr))
            dve.append(lambda i=i, j=j: mul(tmpr, x[3 * i + 2], T2[2][j]))
            dve.append(lambda o=o: add(o, o, tmpr))

    def pool_tail():
        # clamp junk rows, dR = R_pred - R_target in place (off DVE)
        tail_eng.tensor_scalar(out=RT[:, :, 0:L], in0=RT[:, :, 0:L],
                               scalar1=8.0, scalar2=-8.0,
                               op0=Alu.min, op1=Alu.max)
        tail_eng.tensor_tensor(out=RT[:, :, 0:T], in0=RT[:, :, 0:T],
                               in1=RT[:, :, T:L], op=Alu.subtract)

    def act_accum():
        bias0 = env["bias0"]
        nc.scalar.activation(out=RT[:, :, 0:T], in_=RT[:, :, 0:T],
                             func=Act.Square, bias=bias0[:, 0:1], scale=1.0,
                             accum_out=acc_rot_col)

    return dve, pool_tail, act_accum


def _build_nc():
    global bass_mod
    import concourse.bass as bass
    import concourse.tile as tile
    from concourse import mybir
    bass_mod = bass

    f32 = mybir.dt.float32
    f16 = mybir.dt.float16
    nc = bass.Bass()
    pred = nc.dram_tensor("pred", [ROWS_PER_CORE, 9], f32, kind="ExternalInput")
    targ = nc.dram_tensor("target", [ROWS_PER_CORE, 9], f32, kind="ExternalInput")
    out = nc.dram_tensor("partials", [P, 2 * NCHUNK], f32, kind="ExternalOutput")

    predv = pred.rearrange("(p n) c -> p n c", p=P)    # [128, 1024, 9]
    targv = targ.rearrange("(p n) c -> p n c", p=P)
    row0 = np.cumsum((0,) + CHUNKS)                    # chunk row offsets

    with tile.TileContext(nc) as tc:
        with (
            tc.tile_pool(name="raw", bufs=1) as rawp,
            tc.tile_pool(name="pl", bufs=1) as pl,
            tc.tile_pool(name="acc", bufs=1) as accp,
        ):
            acc = accp.tile([P, 2 * NCHUNK], f32, tag="acc")
            bias0 = accp.tile([P, 1], f32, tag="bias0")
            nc.vector.memset(bias0, 0.0)
            _CONST_STATE[(id(nc), "pool")] = accp
            env = {
                "mybir": mybir,
                "regs": Regs(pl, f32, prefix="v"),
                "regs16": Regs(pl, f16, prefix="h"),
                "bias0": bias0,
            }

            raw_cache = {}

            def raw_tiles(k):
                if k not in raw_cache:
                    raw_cache[k] = (
                        rawp.tile([P, TMAX * 9], f32, tag=f"praw{k % 2}",
                                  name=f"praw{k % 2}"),
                        rawp.tile([P, TMAX * 9], f32, tag=f"traw{k % 2}",
                                  name=f"traw{k % 2}"))
                return raw_cache[k]

            def chunk_tiles(k):
                j = k % 2
                return {
                    "X": pl.tile([P, 9, 2 * TMAX], f16, tag=f"X{j}", name=f"X{j}"),
                    "D": pl.tile([P, 9, TMAX], f16, tag="D", name="D"),
                    "Sm": pl.tile([P, 6, 2 * TMAX], f16, tag=f"Sm{j}", name=f"Sm{j}"),
                    "QS": pl.tile([P, 6, 2 * TMAX], f32, tag="QS", name="QS"),
                    "Am": pl.tile([P, 3, 2 * TMAX], f16, tag=f"Am{j}", name=f"Am{j}"),
                    "AJ": pl.tile([P, 6, 2 * TMAX], f16, tag=f"AJ{j}", name=f"AJ{j}"),
                    "T1m": pl.tile([P, 6, 2 * TMAX], f16, tag=f"T1m{j}", name=f"T1m{j}"),
                    "Nm": pl.tile([P, 6, 2 * TMAX], f16, tag=f"Nm{j}", name=f"Nm{j}"),
                    "W16": pl.tile([P, 2 * TMAX], f16, tag=f"W16{j}", name=f"W16{j}"),
                    "RT": pl.tile([P, 9, 2 * TMAX], f16, tag=f"RT{j}", name=f"RT{j}"),
                    "BT": pl.tile([P, 2 * TMAX], f16, tag=f"BT{j}", name=f"BT{j}"),
                    "SQ": pl.tile([P, 9, 2 * TMAX], f16, tag="SQ", name="SQ"),
                }

            def dma_chunk(k):
                prw, trw = raw_tiles(k)
                t0, t1 = row0[k], row0[k + 1]
                n = (t1 - t0) * 9
                nc.sync.dma_start(out=prw[:, 0:n], in_=predv[:, t0:t1, :])
                nc.sync.dma_start(out=trw[:, 0:n], in_=targv[:, t0:t1, :])

            dma_chunk(0)
            dma_chunk(1)

            pending = []        # phase-B DVE thunks of chunk k-1
            tail_q = []         # (pool_tail, act_accum) of chunk k-1
            accum_slot = []     # ACT accumulates to emit at next post_cast

            def filler(n):
                for _ in range(min(n, len(pending))):
                    pending.pop(0)()

            for k, T in enumerate(CHUNKS):
                praw, traw = raw_tiles(k)
                tiles = chunk_tiles(k)

                def pre_cast():
                    while accum_slot:
                        accum_slot.pop(0)()

                def post_cast(k=k):
                    if k + 2 < NCHUNK:
                        dma_chunk(k + 2)

                _emit_A(nc, env, k, T, praw, traw, tiles,
                        acc[:, k:k + 1], filler, pre_cast, post_cast)
                # drain the rest of chunk k-1's phase B, then its GpSimd tail;
                # its ACT accumulate goes early into chunk k+1
                filler(len(pending))
                if tail_q:
                    pool_tail, act_accum = tail_q.pop(0)
                    pool_tail()
                    accum_slot.append(act_accum)
                dve, pool_tail, act_accum = _make_B(
                    nc, env, k, T, tiles, acc[:, NCHUNK + k:NCHUNK + k + 1],
                    last=(k == NCHUNK - 1))
                pending = dve
                tail_q.append((pool_tail, act_accum))

            # drain the pipeline: B of the last chunk, tails, accumulates
            filler(len(pending))
            while accum_slot:
                accum_slot.pop(0)()
            while tail_q:
                pool_tail, act_accum = tail_q.pop(0)
                pool_tail()
                act_accum()
            nc.sync.dma_start(out=out[:, :], in_=acc)
    return nc


def _elide_implied_waits(nc):
    """Drop semaphore waits already implied by program order or transitively
    by earlier waits (vector-clock propagation).  Tile's per-instruction wait
    emission is not transitively minimal, and walrus can encode only one sync
    wait on Activation/DMA instructions (and ~4 on control instructions), so
    the redundant waits both break codegen and waste sequencer time.

    Model: each semaphore s carries a snapshot VC at every increment value;
    an engine's observed VC advances via its own instruction stream and via
    the snapshots of the waits it executes.  A wait (s >= v) is dropped iff
    the engine's observed VC already dominates it.  Unknown update modes
    disable elision for that semaphore (conservative).
    """
    join = lambda a, b: {k: max(a.get(k, 0), b.get(k, 0)) for k in set(a) | set(b)}
    sem_val = {}        # sem name -> current value
    sem_snap = {}       # sem name -> list of (value, VC) snapshots
    eng_vc = {}         # engine name -> observed VC
    unsafe = set()      # sems with non-increment updates
    n_drop = 0
    for f in nc.m.functions:
        for bb in f.blocks:
            for ins in bb.instructions:
                eng = str(ins.engine)
                vc = dict(eng_vc.get(eng, {}))
                si = ins.sync_info
                waits = list(si.on_wait) if si is not None and si.on_wait else []
                kept = []
                for w in waits:
                    s, v = w.ant_name, w.wait_value
                    if w.wait_mode != "sem-ge-imm" or s in unsafe:
                        kept.append(w)
                        continue
                    if vc.get(s, 0) >= v:
                        n_drop += 1
                        continue
                    if sem_val.get(s, 0) < v:
                        # increment not yet seen in emission order; keep and
                        # learn nothing (conservative)
                        kept.append(w)
                        continue
                    kept.append(w)
                    snap = {}
                    for sv, svc in sem_snap.get(s, ()):
                        if sv <= v:
                            snap = svc
                        else:
                            break
                    vc = join(vc, snap)
                    vc[s] = max(vc.get(s, 0), v)
                if si is not None and len(kept) != len(waits):
                    si.on_wait = kept
                # apply this instruction's increments
                ups = si.on_update if si is not None and si.on_update else []
                for u in ups:
                    s = u.ant_name
                    if u.update_mode not in ("sem-inc", "sem-add-imm"):
                        unsafe.add(s)
                        continue
                    nv = sem_val.get(s, 0) + (u.update_value or 1)
                    sem_val[s] = nv
                    lst = sem_snap.setdefault(s, [])
                    prev = lst[-1][1] if lst else {}
                    lst.append((nv, join(prev, vc)))
                    # Engine-sem increments fire when the instruction
                    # completes, and the engine is sequential, so later
                    # instructions on this engine observe them.  DMA-queue
                    # increments fire asynchronously at transfer completion:
                    # the issuing engine must NOT absorb those.
                    if "DMA" not in s:
                        vc[s] = max(vc.get(s, 0), nv)
                eng_vc[eng] = vc
    return n_drop


def _spill_excess_waits(nc):
    """walrus encodes at most ONE sync wait on real engine instructions
    (Activation/DVE/DMA); the Tile scheduler can leave more after
    cross-engine reordering.  Keep one wait on the instruction and hoist
    the rest onto preceding InstEventSemaphore control instructions
    (which accept 2 waits each)."""
    from concourse import mybir
    n_spill = 0
    for f in nc.m.functions:
        for bb in f.blocks:
            out = []
            for ins in bb.instructions:
                si = ins.sync_info
                waits = list(si.on_wait) if si is not None and si.on_wait else []
                is_engine_op = bool(ins.ins) or bool(ins.outs)
                if len(waits) > 1 and is_engine_op and \
                        not isinstance(ins, mybir.InstEventSemaphore):
                    spill = waits[:-1]
                    si.on_wait = waits[-1:]
                    while spill:
                        grp, spill = spill[:2], spill[2:]
                        n_spill += 1
                        ev = mybir.InstEventSemaphore(
                            name=f"wspill_{n_spill}", engine=ins.engine,
                            ins=[], outs=[],
                            sync_info=mybir.SyncInfo(on_wait=grp,
                                                     on_update=[]))
                        out.append(ev)
                out.append(ins)
            bb.instructions = out
    return n_spill


_NC_CACHE = None


def kernel(pred: np.ndarray, target: np.ndarray) -> np.ndarray:
    global _NC_CACHE
    from concourse.bass_utils import run_bass_kernel_spmd

    pred = np.ascontiguousarray(np.asarray(pred, dtype=np.float32))
    target = np.ascontiguousarray(np.asarray(target, dtype=np.float32))
    assert pred.shape == (B, 9) and target.shape == (B, 9)

    if _NC_CACHE is None:
        _NC_CACHE = _build_nc()
        _elide_implied_waits(_NC_CACHE)
        _spill_excess_waits(_NC_CACHE)
    nc = _NC_CACHE

    ps = pred.reshape(N_CORES, ROWS_PER_CORE, 9)
    ts = target.reshape(N_CORES, ROWS_PER_CORE, 9)
    in_maps = [{"pred": ps[i], "target": ts[i]} for i in range(N_CORES)]
    res = run_bass_kernel_spmd(nc, in_maps, core_ids=list(range(N_CORES)))
    globals()["_LAST_RESULT"] = res

    mse_sum = 0.0
    rot_sum = 0.0
    for r in res.results:
        part = np.asarray(r["partials"], dtype=np.float64)
        mse_sum += part[:, :NCHUNK].sum()
        rot_sum += part[:, NCHUNK:].sum()
    n = float(B * 9)
    return np.asarray(np.float32(mse_sum / n + 0.5 * (rot_sum / n)))
